# revision 24
# baseline (speedup 1.0000x reference)
"""Trainium2 Bass kernel for nn_DiffGraphTransformerGenGCN.

Strategy (see spec sharding hint): data-parallel over the 64 graphs, 8 graphs
per NeuronCore.  Per core everything is reformulated as dense per-graph
matmuls:

  stage 1 (GCN filter coefficients over the attention graphs):
    colsum/weighted-colsum of each 256x256 attention matrix via PE matmuls
    with the attention tile as the stationary operand -> tanh -> global
    average pool -> tiny block-diagonal linear -> 4 coefficients per
    (graph, head).

  stage 2 (dynamic Chebyshev filtering on the input graph):
    The edge list is converted (host-side, pure integer indexing) into a
    dense 256x256 edge-count matrix C_g per graph.  The scaled Laplacian
    action is  P x = -D C^T D x  with D = diag(rsqrt(indeg)).  The Chebyshev
    coefficients c_(g,h,k) (stage-1 output) are folded into per-(g,h)
    scaled copies of W_cheb, so  acc = sum_k c_k T_k(P) (x0 W_k)  is computed
    with 6 dense propagation matmuls per graph and no transposes.

  final: out = LayerNorm(concat(output, filtered) @ W_cat + b_cat') computed
    as a feat-major matmul (lhsT = transposed operands; 'output' comes in
    pre-transposed via DMA-transpose, 'filtered' is transposed on the PE).

kernel() takes FULL inputs and returns the FULL output; it shards across the
8 cores internally.
"""

import numpy as np

B, N, H, DH, ORDER = 64, 256, 8, 64, 4
D = H * DH
NCORES = 8
GPC = B // NCORES          # graphs per core
NCHUNK = N // 128          # node chunks per graph (2)

_CACHE = {}


# ---------------------------------------------------------------------------
# device kernel construction
# ---------------------------------------------------------------------------

def _build_module(use_f32r=True):
    import concourse.bass as bass
    import concourse.bacc as bacc
    import concourse.tile as tile
    from concourse import mybir
    from concourse.masks import make_identity

    F32 = mybir.dt.float32
    F32R = mybir.dt.float32r
    AF = mybir.ActivationFunctionType
    ALU = mybir.AluOpType

    def r(ap):
        return ap.bitcast(F32R) if use_f32r else ap

    nc = bacc.Bacc(None, target_bir_lowering=False, debug=False)

    # ---- DRAM parameters (per-core shard) ----
    p_attn = nc.declare_dram_parameter("attn_s", [GPC, H, N, N], F32, isOutput=False).ap()
    p_x0 = nc.declare_dram_parameter("x0_s", [GPC, N, H, DH], F32, isOutput=False).ap()
    p_oin = nc.declare_dram_parameter("out_in_s", [N, GPC, D], F32, isOutput=False).ap()
    p_cmat = nc.declare_dram_parameter("cmat_s", [GPC, N, N], F32, isOutput=False).ap()
    p_wcheb = nc.declare_dram_parameter("wcheb", [DH, ORDER * DH], F32, isOutput=False).ap()
    p_wcat = nc.declare_dram_parameter("wcat", [2 * D, D], F32, isOutput=False).ap()
    p_bcat2 = nc.declare_dram_parameter("bcat2", [1, D], F32, isOutput=False).ap()
    p_gamma = nc.declare_dram_parameter("gamma", [1, D], F32, isOutput=False).ap()
    p_beta = nc.declare_dram_parameter("beta", [1, D], F32, isOutput=False).ap()
    p_wlinbd = nc.declare_dram_parameter("wlinbd", [128, 128], F32, isOutput=False).ap()
    p_wrow = nc.declare_dram_parameter("wrow", [1, ORDER], F32, isOutput=False).ap()
    p_bgcn = nc.declare_dram_parameter("bgcn", [1, ORDER], F32, isOutput=False).ap()
    p_blin = nc.declare_dram_parameter("blin", [128, 1], F32, isOutput=False).ap()
    p_outy = nc.declare_dram_parameter("outy", [N, GPC, D], F32, isOutput=True).ap()

    def bcast(dram_ap, parts, inner):
        """AP reading a [1, n] DRAM row broadcast across `parts` partitions."""
        return bass.AP(tensor=dram_ap.tensor, offset=dram_ap.offset,
                       ap=[[0, parts]] + inner)

    with tile.TileContext(nc) as tc:
        import contextlib
        ctx = contextlib.ExitStack()
        with ctx:
            const = ctx.enter_context(tc.tile_pool(name="const", bufs=1))
            persist = ctx.enter_context(tc.tile_pool(name="persist", bufs=1))
            ps = ctx.enter_context(tc.tile_pool(name="ps", bufs=2, space="PSUM"))
            dram = ctx.enter_context(tc.tile_pool(name="dram", bufs=1, space="DRAM"))

            # ---------------- constants ----------------
            ones_col = const.tile([128, 1], F32)
            nc.vector.memset(ones_col, 1.0)
            identity = const.tile([128, 128], F32)
            make_identity(nc, identity)
            wcheb_sb = const.tile([DH, ORDER * DH], F32)
            nc.sync.dma_start(out=wcheb_sb, in_=p_wcheb)
            wlinbd_sb = const.tile([128, 128], F32)
            nc.sync.dma_start(out=wlinbd_sb, in_=p_wlinbd)
            wcat_sb = const.tile([128, 8 * D], F32)
            for fb in range(8):
                nc.sync.dma_start(out=wcat_sb[:, fb * D:(fb + 1) * D],
                                  in_=p_wcat[fb * 128:(fb + 1) * 128, :])
            bcat2_bc = const.tile([128, D], F32)
            nc.sync.dma_start(out=bcat2_bc, in_=bcast(p_bcat2, 128, [[1, D]]))
            gamma_bc = const.tile([128, D], F32)
            nc.sync.dma_start(out=gamma_bc, in_=bcast(p_gamma, 128, [[1, D]]))
            beta_bc = const.tile([128, D], F32)
            nc.sync.dma_start(out=beta_bc, in_=bcast(p_beta, 128, [[1, D]]))
            wrow_bc = const.tile([128, ORDER], F32)
            nc.sync.dma_start(out=wrow_bc, in_=bcast(p_wrow, 128, [[1, ORDER]]))
            bgcn_bc = const.tile([128, ORDER], F32)
            nc.sync.dma_start(out=bgcn_bc, in_=bcast(p_bgcn, 128, [[1, ORDER]]))
            blin_sb = const.tile([128, 1], F32)
            nc.sync.dma_start(out=blin_sb, in_=p_blin)
            eps_col = const.tile([128, 1], F32)
            nc.vector.memset(eps_col, 1e-5)

            # persistent big tiles
            acc_all = persist.tile([128, GPC * NCHUNK * D], F32)    # 4 MB
            outT = persist.tile([128, 4 * GPC * N], F32)            # 4 MB
            dcrow = dram.tile([2, 128], F32)

            # 'output' input, loaded node-major then PE-transposed to
            # feature-major: outT[f in fb-block, fb*2048 + g*256 + n].
            with tc.tile_pool(name="oin", bufs=3) as oinp:
                for nch in range(GPC * NCHUNK):
                    g, dc = divmod(nch, NCHUNK)
                    on = oinp.tile([128, D], F32, tag="on")
                    nc.sync.dma_start(out=on, in_=p_oin[dc * 128:(dc + 1) * 128, g, :])
                    pso = ps.tile([128, 2048], F32, tag="big")
                    for fb in range(4):
                        nc.tensor.transpose(pso[:, fb * 128:(fb + 1) * 128],
                                            on[:, fb * 128:(fb + 1) * 128], identity)
                    # one batched eviction: psum [128, (fb,128)] -> strided outT
                    nc.scalar.copy(
                        out=outT.rearrange("p (f x) -> p f x", f=4)
                            [:, :, nch * 128:(nch + 1) * 128],
                        in_=pso[:, 0:512].rearrange("p (f n) -> p f n", f=4))

            # =====================================================
            # Phase A: stage-1 GCN coefficients
            # =====================================================
            with tc.tile_pool(name="stage1", bufs=3) as s1p, \
                 tc.tile_pool(name="stage1s", bufs=4) as s1s:
                xc_all = persist.tile([128, GPC * 64], F32)  # (g, jc, (h,c))
                for g in range(GPC):
                    At = []
                    for ic in range(2):
                        a = s1p.tile([128, H * N], F32, tag="attn")
                        nc.sync.dma_start(
                            out=a.rearrange("p (h j) -> p h j", h=H),
                            in_=p_attn[g, :, ic * 128:(ic + 1) * 128, :]
                                .rearrange("h i j -> i h j"))
                        At.append(a)
                    psA = ps.tile([128, 2048], F32, tag="big")
                    # round 1: colsum  (deg^T columns, per (jc, h))
                    for jc in range(2):
                        for h in range(H):
                            for ic in range(2):
                                nc.tensor.matmul(
                                    out=psA[:, jc * 8 + h:jc * 8 + h + 1],
                                    lhsT=r(At[ic][:, h * N + jc * 128:h * N + (jc + 1) * 128]),
                                    rhs=r(ones_col),
                                    start=(ic == 0), stop=(ic == 1))
                    dinvT = []
                    for jc in range(2):
                        dsq = s1s.tile([128, 8], F32, tag="dsq")
                        nc.scalar.activation(out=dsq, in_=psA[:, jc * 8:(jc + 1) * 8],
                                             func=AF.Sqrt, bias=1.0)
                        dv = s1s.tile([128, 8], F32, tag="dinvT")
                        nc.vector.reciprocal(out=dv, in_=dsq)
                        dinvT.append(dv)
                    # round 2: dinv-weighted colsum
                    for jc in range(2):
                        for h in range(H):
                            for ic in range(2):
                                nc.tensor.matmul(
                                    out=psA[:, 16 + jc * 8 + h:16 + jc * 8 + h + 1],
                                    lhsT=r(At[ic][:, h * N + jc * 128:h * N + (jc + 1) * 128]),
                                    rhs=r(dinvT[ic][:, h:h + 1]),
                                    start=(ic == 0), stop=(ic == 1))
                    for jc in range(2):
                        sT = s1s.tile([128, 8], F32, tag="sT")
                        nc.vector.tensor_add(sT, psA[:, 16 + jc * 8:16 + (jc + 1) * 8],
                                             dinvT[jc])
                        nc.vector.tensor_mul(sT, sT, dinvT[jc])
                        # xc[:, (h, c)] = tanh(s * w_row[c] + b_gcn[c])
                        # layout: [:, half*256 + jc*128 + g4*32 + (h, c)]
                        half, g4 = divmod(g, 4)
                        off = half * 256 + jc * 128 + g4 * 32
                        xcv = xc_all[:, off:off + 32] \
                            .rearrange("p (h c) -> p h c", c=ORDER)
                        for c in range(ORDER):
                            nc.scalar.activation(
                                out=xcv[:, :, c], in_=sT, func=AF.Tanh,
                                scale=wrow_bc[:, c:c + 1], bias=bgcn_bc[:, c:c + 1])

                # phase A2: GAP + linear -> coefficients
                psA2 = ps.tile([128, 2048], F32, tag="big")
                gap_sb = s1s.tile([128, 2], F32, tag="gap")
                ct_sb = s1s.tile([128, 2], F32, tag="ct")
                for half in range(2):
                    for jc in range(2):
                        nc.tensor.matmul(
                            out=psA2[:, half:half + 1],
                            lhsT=r(xc_all[:, half * 256 + jc * 128:
                                          half * 256 + (jc + 1) * 128]),
                            rhs=r(ones_col),
                            start=(jc == 0), stop=(jc == 1))
                    nc.scalar.copy(out=gap_sb[:, half:half + 1],
                                   in_=psA2[:, half:half + 1])
                    nc.tensor.matmul(out=psA2[:, 4 + half:5 + half],
                                     lhsT=r(wlinbd_sb),
                                     rhs=r(gap_sb[:, half:half + 1]),
                                     start=True, stop=True)
                    nc.vector.tensor_add(ct_sb[:, half:half + 1],
                                         psA2[:, 4 + half:5 + half], blin_sb)
                    # transpose [128,1] -> [1,128] and park in DRAM for
                    # broadcast-reads
                    nc.tensor.transpose(psA2[0:1, 512 + half * 128:512 + (half + 1) * 128],
                                        ct_sb[:, half:half + 1], identity)
                    crow_sb = s1s.tile([1, 128], F32, tag="crow")
                    nc.scalar.copy(out=crow_sb,
                                   in_=psA2[0:1, 512 + half * 128:512 + (half + 1) * 128])
                    nc.sync.dma_start(out=dcrow[half:half + 1, :], in_=crow_sb)

            # =====================================================
            # Phase B: Chebyshev propagation per graph
            # =====================================================
            with tc.tile_pool(name="st2a", bufs=2) as st2a, \
                 tc.tile_pool(name="st2b", bufs=2) as st2b, \
                 tc.tile_pool(name="st2s", bufs=4) as st2s:
                for g in range(GPC):
                    # --- C_g and degree-derived columns ---
                    cm = []
                    for sc in range(2):
                        t = st2a.tile([128, N], F32, tag="cm")
                        nc.sync.dma_start(out=t, in_=p_cmat[g, sc * 128:(sc + 1) * 128, :])
                        cm.append(t)
                    psd = ps.tile([128, 2048], F32, tag="big")
                    for dc in range(2):
                        for sc in range(2):
                            nc.tensor.matmul(
                                out=psd[:, dc:dc + 1],
                                lhsT=r(cm[sc][:, dc * 128:(dc + 1) * 128]),
                                rhs=r(ones_col), start=(sc == 0), stop=(sc == 1))
                    # masked rsqrt: dinv = m / (sqrt(deg) + 1 - m), m = sign(deg)
                    # dvar layout [128, (kind, dc)]: kinds
                    # 0=dinv 1=dsq 2=dneg 3=d3 4=d2 5=dm4
                    dvar = st2s.tile([128, 12], F32, tag="dvar")
                    sg = st2s.tile([128, 2], F32, tag="sg")
                    nc.scalar.activation(out=sg, in_=psd[:, 0:2], func=AF.Sign)
                    sq = st2s.tile([128, 2], F32, tag="sq")
                    nc.scalar.activation(out=sq, in_=psd[:, 0:2], func=AF.Sqrt)
                    den = st2s.tile([128, 2], F32, tag="den")
                    nc.vector.scalar_tensor_tensor(
                        out=den, in0=sq, scalar=1.0, in1=sg,
                        op0=ALU.add, op1=ALU.subtract)
                    rec = st2s.tile([128, 2], F32, tag="rec")
                    nc.vector.reciprocal(out=rec, in_=den)
                    dv = dvar[:, 0:2]
                    nc.vector.tensor_mul(dv, rec, sg)
                    nc.scalar.square(dvar[:, 2:4], dv)
                    nc.scalar.mul(dvar[:, 4:6], dv, -1.0)
                    nc.scalar.mul(dvar[:, 6:8], dv, 3.0)
                    nc.scalar.mul(dvar[:, 8:10], dv, 2.0)
                    nc.scalar.mul(dvar[:, 10:12], dv, -4.0)

                    # --- x0^T via PE transpose; c-scaled W_cheb ---
                    x0t = st2b.tile([DH, H * N], F32, tag="x0t")
                    for nc2 in range(2):
                        x0n = st2s.tile([128, H * DH], F32, tag="x0n")
                        nc.sync.dma_start(
                            out=x0n.rearrange("p (h d) -> p h d", h=H),
                            in_=p_x0[g, nc2 * 128:(nc2 + 1) * 128, :, :])
                        psx = ps.tile([128, 2048], F32, tag="big")
                        for h in range(H):
                            nc.tensor.transpose(psx[0:DH, h * 128:(h + 1) * 128],
                                                x0n[:, h * DH:(h + 1) * DH], identity)
                        nc.vector.tensor_copy(
                            x0t.rearrange("p (h x) -> p h x", h=H)
                               [:, :, nc2 * 128:(nc2 + 1) * 128],
                            psx[0:DH, 0:1024].rearrange("p (h n) -> p h n", h=H))
                    # cexp3[p, (dh', hk)]: c value per (h,k) broadcast over
                    # partitions and dh' (innermost dim continuous for DGE)
                    cexp = st2b.tile([DH, DH * H * ORDER], F32, tag="cexp")
                    nc.sync.dma_start(
                        out=cexp.rearrange("p (d hk) -> p d hk", d=DH),
                        in_=bass.AP(tensor=dcrow.tensor,
                                    offset=dcrow.offset + (g // 4) * 128 + (g % 4) * 32,
                                    ap=[[0, DH], [0, DH], [1, H * ORDER]]))
                    wsc = st2b.tile([DH, H * ORDER * DH], F32, tag="wsc")
                    cexpv = cexp.rearrange("p (d h2 k) -> p h2 k d", d=DH, k=ORDER)
                    for h in range(H):
                        nc.gpsimd.tensor_mul(
                            wsc[:, h * 256:(h + 1) * 256]
                                .rearrange("p (k d) -> p k d", k=ORDER),
                            cexpv[:, h, :, :],
                            wcheb_sb.rearrange("p (k d) -> p k d", k=ORDER))

                    # --- Z matmuls:  Z[(n),(k,dh')] = x0 @ (c_k W_k) ---
                    accv = []
                    v0 = []
                    for dc in range(2):
                        pz = ps.tile([128, 2048], F32, tag="big")
                        for h in range(H):
                            nc.tensor.matmul(
                                out=pz[:, h * 256:(h + 1) * 256],
                                lhsT=r(x0t[:, h * N + dc * 128:h * N + (dc + 1) * 128]),
                                rhs=r(wsc[:, h * 256:(h + 1) * 256]),
                                start=True, stop=True)
                        pzv = pz.rearrange("p (h k d) -> p h k d", h=H, k=ORDER)
                        a = acc_all[:, (g * 2 + dc) * D:(g * 2 + dc + 1) * D] \
                            .rearrange("p (h d) -> p h d", h=H)
                        accv.append(a)
                        # acc = Z0 - Z2  (one PSUM operand per instruction)
                        nc.scalar.mul(a, pzv[:, :, 2, :], -1.0)
                        nc.vector.tensor_tensor(
                            out=a, in0=pzv[:, :, 0, :], in1=a, op=ALU.add)
                        # v0 = dinv * Z[1:4]
                        v = st2a.tile([128, H * 3 * DH], F32, tag="v0")
                        nc.scalar.activation(
                            out=v.rearrange("p (h k d) -> p h k d", h=H, k=3),
                            in_=pzv[:, :, 1:4, :], func=AF.Copy,
                            scale=dvar[:, dc:dc + 1])
                        v0.append(v)

                    # --- propagation round 1: y1 = C^T v0 ---
                    v1 = []
                    for dc in range(2):
                        py = ps.tile([128, 2048], F32, tag="big")
                        for w in range(3):
                            for sc in range(2):
                                nc.tensor.matmul(
                                    out=py[:, w * 512:(w + 1) * 512],
                                    lhsT=r(cm[sc][:, dc * 128:(dc + 1) * 128]),
                                    rhs=r(v0[sc][:, w * 512:(w + 1) * 512]),
                                    start=(sc == 0), stop=(sc == 1))
                        pyv = py[:, 0:1536].rearrange("p (h k d) -> p h k d", h=H, k=3)
                        a = accv[dc]
                        # acc += -dinv * y1[k=1]
                        nc.vector.scalar_tensor_tensor(
                            out=a, in0=pyv[:, :, 0, :],
                            scalar=dvar[:, 4 + dc:5 + dc],
                            in1=a, op0=ALU.mult, op1=ALU.add)
                        # acc += 3 dinv * y1[k=3]
                        nc.vector.scalar_tensor_tensor(
                            out=a, in0=pyv[:, :, 2, :],
                            scalar=dvar[:, 6 + dc:7 + dc],
                            in1=a, op0=ALU.mult, op1=ALU.add)
                        # v1 = dinv^2 * y1[k=2,3]
                        v = st2a.tile([128, H * 2 * DH], F32, tag="v1")
                        nc.scalar.activation(
                            out=v.rearrange("p (h k d) -> p h k d", h=H, k=2),
                            in_=pyv[:, :, 1:3, :], func=AF.Copy,
                            scale=dvar[:, 2 + dc:3 + dc])
                        v1.append(v)

                    # --- round 2 ---
                    v2 = []
                    for dc in range(2):
                        py = ps.tile([128, 2048], F32, tag="big")
                        for w in range(2):
                            for sc in range(2):
                                nc.tensor.matmul(
                                    out=py[:, w * 512:(w + 1) * 512],
                                    lhsT=r(cm[sc][:, dc * 128:(dc + 1) * 128]),
                                    rhs=r(v1[sc][:, w * 512:(w + 1) * 512]),
                                    start=(sc == 0), stop=(sc == 1))
                        pyv = py[:, 0:1024].rearrange("p (h k d) -> p h k d", h=H, k=2)
                        a = accv[dc]
                        # acc += 2 dinv * y2[k=2]
                        nc.vector.scalar_tensor_tensor(
                            out=a, in0=pyv[:, :, 0, :],
                            scalar=dvar[:, 8 + dc:9 + dc],
                            in1=a, op0=ALU.mult, op1=ALU.add)
                        v = st2a.tile([128, H * DH], F32, tag="v2")
                        nc.scalar.activation(
                            out=v.rearrange("p (h d) -> p h d", h=H),
                            in_=pyv[:, :, 1, :], func=AF.Copy,
                            scale=dvar[:, 2 + dc:3 + dc])
                        v2.append(v)

                    # --- round 3 ---
                    for dc in range(2):
                        py = ps.tile([128, 2048], F32, tag="big")
                        for sc in range(2):
                            nc.tensor.matmul(
                                out=py[:, 0:512],
                                lhsT=r(cm[sc][:, dc * 128:(dc + 1) * 128]),
                                rhs=r(v2[sc]), start=(sc == 0), stop=(sc == 1))
                        a = accv[dc]
                        # acc += -4 dinv * y3
                        nc.vector.scalar_tensor_tensor(
                            out=a,
                            in0=py[:, 0:512].rearrange("p (h d) -> p h d", h=H),
                            scalar=dvar[:, 10 + dc:11 + dc],
                            in1=a, op0=ALU.mult, op1=ALU.add)

            # =====================================================
            # Phase C: transpose acc, cat-matmul, LayerNorm, store
            # =====================================================
            with tc.tile_pool(name="ph_c", bufs=1) as phc, \
                 tc.tile_pool(name="ln", bufs=3) as lnp, \
                 tc.tile_pool(name="lns", bufs=4) as lns:
                accT = phc.tile([128, 4 * GPC * N], F32)
                for nch in range(GPC * NCHUNK):
                    pst = ps.tile([128, 2048], F32, tag="big")
                    for fb in range(4):
                        nc.tensor.transpose(
                            pst[:, fb * 128:(fb + 1) * 128],
                            acc_all[:, nch * D + fb * 128:nch * D + (fb + 1) * 128],
                            identity)
                    nc.scalar.copy(
                        out=accT.rearrange("p (f x) -> p f x", f=4)
                            [:, :, nch * 128:(nch + 1) * 128],
                        in_=pst[:, 0:512].rearrange("p (f n) -> p f n", f=4))

                for nch in range(GPC * NCHUNK):
                    g, dc = divmod(nch, NCHUNK)
                    po = ps.tile([128, 2048], F32, tag="big")
                    for fb in range(4):
                        nc.tensor.matmul(
                            out=po[:, 0:512],
                            lhsT=r(outT[:, fb * (GPC * N) + nch * 128:
                                        fb * (GPC * N) + (nch + 1) * 128]),
                            rhs=r(wcat_sb[:, fb * D:(fb + 1) * D]),
                            start=(fb == 0), stop=False)
                    for fb in range(4):
                        nc.tensor.matmul(
                            out=po[:, 0:512],
                            lhsT=r(accT[:, fb * (GPC * N) + nch * 128:
                                        fb * (GPC * N) + (nch + 1) * 128]),
                            rhs=r(wcat_sb[:, (4 + fb) * D:(5 + fb) * D]),
                            start=False, stop=(fb == 3))

                    # LayerNorm
                    t0 = lnp.tile([128, D], F32, tag="t0")
                    musum = lns.tile([128, 1], F32, tag="musum")
                    nc.vector.scalar_tensor_tensor(
                        out=t0, in0=po[:, 0:512], scalar=1.0, in1=bcat2_bc,
                        op0=ALU.mult, op1=ALU.add, accum_out=musum)
                    negmu = lns.tile([128, 1], F32, tag="negmu")
                    nc.scalar.mul(negmu, musum, -1.0 / D)
                    sqs = lns.tile([128, 1], F32, tag="sqs")
                    sq = lnp.tile([128, D], F32, tag="sq")
                    nc.scalar.activation(out=sq, in_=t0, func=AF.Square,
                                         bias=negmu, accum_out=sqs)
                    sd = lns.tile([128, 1], F32, tag="sd")
                    nc.scalar.activation(out=sd, in_=sqs, func=AF.Sqrt,
                                         scale=1.0 / D, bias=eps_col)
                    rstd = lns.tile([128, 1], F32, tag="rstd")
                    nc.vector.reciprocal(out=rstd, in_=sd)
                    nmr = lns.tile([128, 1], F32, tag="nmr")
                    nc.vector.tensor_mul(nmr, negmu, rstd)
                    t1 = lnp.tile([128, D], F32, tag="t1")
                    nc.scalar.activation(out=t1, in_=t0, func=AF.Identity,
                                         scale=rstd, bias=nmr)
                    t2 = lnp.tile([128, D], F32, tag="t2")
                    nc.gpsimd.tensor_mul(t2, t1, gamma_bc)
                    nc.gpsimd.tensor_add(t2, t2, beta_bc)
                    nc.sync.dma_start(out=p_outy[dc * 128:(dc + 1) * 128, g, :], in_=t2)

    if not nc.is_finalized():
        nc.finalize()
    return nc


# ---------------------------------------------------------------------------
# host side
# ---------------------------------------------------------------------------

def _canonical_indices(feature_indices, batch):
    fi = np.asarray(feature_indices)
    bt = np.asarray(batch)
    want0 = np.repeat(np.arange(B), N)
    want1 = np.tile(np.arange(N), B)
    return (fi.shape == (B * N, 2) and bt.shape == (B * N,)
            and np.array_equal(fi[:, 0], want0) and np.array_equal(fi[:, 1], want1)
            and np.array_equal(bt, want0))


def _prep(inputs):
    """Host-side sharding + index preprocessing. Returns per-core input maps."""
    attn = np.ascontiguousarray(np.asarray(inputs["attn"], np.float32))
    oeh = np.ascontiguousarray(np.asarray(inputs["out_each_head"], np.float32))
    outp = np.ascontiguousarray(np.asarray(inputs["output"], np.float32))
    ei = np.asarray(inputs["edge_index"])

    W_gcn = np.asarray(inputs["W_gcn"], np.float32)
    b_gcn = np.asarray(inputs["b_gcn"], np.float32)
    W_lin = np.asarray(inputs["W_lin"], np.float32)
    b_lin = np.asarray(inputs["b_lin"], np.float32)
    W_cheb = np.asarray(inputs["W_cheb"], np.float32)
    b_cheb = np.asarray(inputs["b_cheb"], np.float32)
    W_cat = np.asarray(inputs["W_cat"], np.float32)
    b_cat = np.asarray(inputs["b_cat"], np.float32)
    gamma = np.asarray(inputs["gamma"], np.float32)
    beta = np.asarray(inputs["beta"], np.float32)

    # dense per-graph edge-count matrices (pure integer indexing)
    s_g = (ei[0] // N).astype(np.int64)
    s_l = (ei[0] % N).astype(np.int64)
    d_l = (ei[1] % N).astype(np.int64)
    flat = np.zeros(B * N * N, np.float32)
    np.add.at(flat, s_g * (N * N) + s_l * N + d_l, 1.0)
    cmat = flat.reshape(B, N, N)

    wcheb = np.ascontiguousarray(
        W_cheb.transpose(1, 0, 2).reshape(DH, ORDER * DH))   # [dh, (k, dh')]
    # blockdiag(I_32 (x) W_lin) / 256 : partition (m, c) -> col (m, k)
    wlinbd = (np.kron(np.eye(32, dtype=np.float32), W_lin) / N).astype(np.float32)
    blin = np.tile(b_lin, 32).reshape(128, 1).astype(np.float32)
    wrow = W_gcn.sum(axis=0).reshape(1, ORDER).astype(np.float32)
    bcat2 = (b_cat + np.tile(b_cheb, H) @ W_cat[D:, :]).reshape(1, D).astype(np.float32)

    shared = dict(
        wcheb=wcheb, wcat=np.ascontiguousarray(W_cat),
        bcat2=bcat2, gamma=gamma.reshape(1, D).astype(np.float32),
        beta=beta.reshape(1, D).astype(np.float32),
        wlinbd=wlinbd, wrow=wrow,
        bgcn=b_gcn.reshape(1, ORDER).astype(np.float32), blin=blin)

    in_maps = []
    for c in range(NCORES):
        G = slice(c * GPC, (c + 1) * GPC)
        in_maps.append(dict(
            attn_s=np.ascontiguousarray(attn[G]),
            x0_s=np.ascontiguousarray(oeh[G]),
            out_in_s=np.ascontiguousarray(outp[:, G, :]),
            cmat_s=np.ascontiguousarray(cmat[G]),
            **shared))
    return in_maps


def _fallback_numpy(inputs):
    """Generic (slow) numpy path, used only if the index tensors deviate from
    the canonical layout produced by setup_inputs()."""
    output = np.asarray(inputs["output"], np.float32)
    attn = np.asarray(inputs["attn"], np.float32)
    oeh = np.asarray(inputs["out_each_head"], np.float32)
    ei = np.asarray(inputs["edge_index"])
    fi = np.asarray(inputs["feature_indices"])
    batch = np.asarray(inputs["batch"])
    W_gcn = np.asarray(inputs["W_gcn"], np.float32); b_gcn = np.asarray(inputs["b_gcn"], np.float32)
    W_lin = np.asarray(inputs["W_lin"], np.float32); b_lin = np.asarray(inputs["b_lin"], np.float32)
    W_cheb = np.asarray(inputs["W_cheb"], np.float32); b_cheb = np.asarray(inputs["b_cheb"], np.float32)
    W_cat = np.asarray(inputs["W_cat"], np.float32); b_cat = np.asarray(inputs["b_cat"], np.float32)
    gamma = np.asarray(inputs["gamma"], np.float32); beta = np.asarray(inputs["beta"], np.float32)

    Bn, Hn, Nn, _ = attn.shape
    C = W_gcn.shape[0]
    total = Bn * Nn
    NT = Hn * total
    A = attn.transpose(1, 0, 2, 3).reshape(Hn * Bn, Nn, Nn)
    A_hat = A + np.eye(Nn, dtype=A.dtype)
    deg = A_hat.sum(axis=1)
    dinv = np.where(deg > 0, 1.0 / np.sqrt(deg), 0.0).astype(np.float32)
    w_row = W_gcn.sum(axis=0)
    s = np.einsum('bi,bij->bj', dinv, A_hat) * dinv
    x_c = np.tanh(s[:, :, None] * w_row + b_gcn)
    gap = x_c.mean(axis=1)
    coeff = gap @ W_lin + b_lin

    offsets = (np.arange(Hn) * total).astype(ei.dtype)
    src = (ei[0][None, :] + offsets[:, None]).reshape(-1)
    dst = (ei[1][None, :] + offsets[:, None]).reshape(-1)
    deg_n = np.zeros(NT, np.float32)
    np.add.at(deg_n, dst, 1.0)
    dinv_n = np.where(deg_n > 0, 1.0 / np.sqrt(np.maximum(deg_n, 1e-30)), 0.0).astype(np.float32)
    norm_e = -(dinv_n[src] * dinv_n[dst])

    def prop(x):
        out = np.zeros((NT, x.shape[1]), np.float32)
        np.add.at(out, dst, norm_e[:, None] * x[src])
        return out

    x0 = oeh.transpose(2, 0, 1, 3).reshape(NT, DH)
    batch_all = (batch[None, :] + (np.arange(Hn) * Bn)[:, None]).reshape(-1)
    c_node = coeff[batch_all]
    T_prev, T_cur = x0, prop(x0)
    acc = c_node[:, 0:1] * (T_prev @ W_cheb[0]) + c_node[:, 1:2] * (T_cur @ W_cheb[1])
    for k in range(2, C):
        T_next = 2.0 * prop(T_cur) - T_prev
        acc = acc + c_node[:, k:k + 1] * (T_next @ W_cheb[k])
        T_prev, T_cur = T_cur, T_next
    acc = acc + b_cheb
    filtered = acc.reshape(Hn, total, DH).transpose(1, 0, 2).reshape(total, Hn * DH)
    out_filtered = np.zeros_like(output)
    out_filtered[fi[:, 1], fi[:, 0], :] = filtered
    out_cat = np.concatenate([output, out_filtered], axis=-1)
    out = out_cat @ W_cat + b_cat
    mu = out.mean(axis=-1, keepdims=True)
    var = ((out - mu) ** 2).mean(axis=-1, keepdims=True)
    return ((out - mu) / np.sqrt(var + 1e-5) * gamma + beta).astype(np.float32)


def _get_nc():
    if "nc" not in _CACHE:
        _CACHE["nc"] = _build_module(use_f32r=_CACHE.get("use_f32r", False))
    return _CACHE["nc"]


def kernel(**inputs) -> np.ndarray:
    if not _canonical_indices(inputs["feature_indices"], inputs["batch"]):
        return _fallback_numpy(inputs)

    from concourse.bass_utils import run_bass_kernel_spmd

    nc = _get_nc()
    in_maps = _prep(inputs)
    res = run_bass_kernel_spmd(nc, in_maps, list(range(NCORES)))
    out = np.empty((N, B, D), np.float32)
    for c in range(NCORES):
        out[:, c * GPC:(c + 1) * GPC, :] = res.results[c]["outy"]
    return out


# revision 29
# speedup vs baseline: 1.6337x; 1.6337x over previous
"""Trainium2 Bass kernel for nn_DiffGraphTransformerGenGCN.

Strategy (see spec sharding hint): data-parallel over the 64 graphs, 8 graphs
per NeuronCore.  Per core everything is reformulated as dense per-graph
matmuls:

  stage 1 (GCN filter coefficients over the attention graphs):
    streaming colsum / dinv-weighted colsum of each 256x256 attention matrix
    (ones / block-diagonal dinv as the stationary operand, attention tiles
    streamed once in float32r), masked tanh with accumulate for the global
    average pool, tiny matmuls for the final linear -> 4 coefficients per
    (graph, head).

  stage 2 (dynamic Chebyshev filtering on the input graph):
    The edge list is converted (host-side, pure integer indexing) into a
    dense 256x256 edge-count matrix C_g per graph.  The scaled Laplacian
    action is  P x = -D C^T D x  with D = diag(rsqrt(indeg)).  The Chebyshev
    coefficients c_(g,h,k) (stage-1 output) are folded into per-(g,h)
    scaled copies of W_cheb, so  acc = sum_k c_k T_k(P) (x0 W_k)  is computed
    with 6 dense propagation matmuls per graph and no transposes.

  final: out = LayerNorm(concat(output, filtered) @ W_cat + b_cat') computed
    as a feat-major matmul (lhsT = PE-transposed operands).

All large matmuls run in float32r (1 cycle/row vs 4 for fp32); operands are
rounded to f32r at their producers (engine copy-casts / SWDGE cast-DMA).

kernel() takes FULL inputs and returns the FULL output; it shards across the
8 cores internally.
"""

import numpy as np

B, N, H, DH, ORDER = 64, 256, 8, 64, 4
D = H * DH
NCORES = 8
GPC = B // NCORES          # graphs per core
NCHUNK = N // 128          # node chunks per graph (2)

_CACHE = {}


# ---------------------------------------------------------------------------
# device kernel construction
# ---------------------------------------------------------------------------

def _build_module(use_f32r=True):
    import concourse.bass as bass
    import concourse.bacc as bacc
    import concourse.tile as tile
    from concourse import mybir
    from concourse.masks import make_identity

    F32 = mybir.dt.float32
    F32R = mybir.dt.float32r if use_f32r else mybir.dt.float32
    AF = mybir.ActivationFunctionType
    ALU = mybir.AluOpType

    nc = bacc.Bacc(None, target_bir_lowering=False, debug=False)

    # ---- DRAM parameters (per-core shard) ----
    p_attn = nc.declare_dram_parameter("attn_s", [GPC, H, N, N], F32, isOutput=False).ap()
    p_x0 = nc.declare_dram_parameter("x0_s", [GPC, N, H, DH], F32, isOutput=False).ap()
    p_oin = nc.declare_dram_parameter("out_in_s", [N, GPC, D], F32, isOutput=False).ap()
    p_cmat = nc.declare_dram_parameter("cmat_s", [GPC, N, N], F32, isOutput=False).ap()
    p_wcheb = nc.declare_dram_parameter("wcheb", [DH, ORDER * DH], F32, isOutput=False).ap()
    p_wcat = nc.declare_dram_parameter("wcat", [2 * D, D], F32, isOutput=False).ap()
    p_bcat2 = nc.declare_dram_parameter("bcat2", [1, D], F32, isOutput=False).ap()
    p_gamma = nc.declare_dram_parameter("gamma", [1, D], F32, isOutput=False).ap()
    p_beta = nc.declare_dram_parameter("beta", [1, D], F32, isOutput=False).ap()
    p_wrow = nc.declare_dram_parameter("wrow", [1, ORDER], F32, isOutput=False).ap()
    p_bgcn = nc.declare_dram_parameter("bgcn", [1, ORDER], F32, isOutput=False).ap()
    p_mask = nc.declare_dram_parameter("mask8", [H, H * N], F32, isOutput=False).ap()
    p_wlin = nc.declare_dram_parameter("wlin4", [ORDER, ORDER], F32, isOutput=False).ap()
    p_kcorr = nc.declare_dram_parameter("kcorr", [ORDER, 1], F32, isOutput=False).ap()
    p_blin4 = nc.declare_dram_parameter("blin4", [ORDER, 1], F32, isOutput=False).ap()
    p_outy = nc.declare_dram_parameter("outy", [N, GPC, D], F32, isOutput=True).ap()

    def bcast(dram_ap, parts, inner, extra_off=0):
        return bass.AP(tensor=dram_ap.tensor, offset=dram_ap.offset + extra_off,
                       ap=[[0, parts]] + inner)

    with tile.TileContext(nc) as tc:
        import contextlib
        ctx = contextlib.ExitStack()
        with ctx:
            const = ctx.enter_context(tc.tile_pool(name="const", bufs=1))
            persist = ctx.enter_context(tc.tile_pool(name="persist", bufs=1))
            ps = ctx.enter_context(tc.tile_pool(name="ps", bufs=2, space="PSUM"))
            dram = ctx.enter_context(tc.tile_pool(name="dram", bufs=1, space="DRAM"))

            # ---------------- constants ----------------
            ones_col = const.tile([128, 1], F32)
            nc.vector.memset(ones_col, 1.0)
            ones_r = const.tile([128, 1], F32R)
            nc.vector.tensor_copy(ones_r, ones_col)
            ones2 = const.tile([128, 2], F32)
            nc.vector.memset(ones2, 1.0)
            ones2_r = const.tile([128, 2], F32R)
            nc.vector.tensor_copy(ones2_r, ones2)
            identity = const.tile([128, 128], F32)
            make_identity(nc, identity)
            wcheb_sb = const.tile([DH, ORDER * DH], F32)
            nc.sync.dma_start(out=wcheb_sb, in_=p_wcheb)
            wcat_sb = const.tile([128, 8 * D], F32R)
            for fb in range(8):
                nc.gpsimd.dma_start(out=wcat_sb[:, fb * D:(fb + 1) * D],
                                    in_=p_wcat[fb * 128:(fb + 1) * 128, :])
            bcat2_bc = const.tile([128, D], F32)
            nc.sync.dma_start(out=bcat2_bc, in_=bcast(p_bcat2, 128, [[1, D]]))
            gamma_bc = const.tile([128, D], F32)
            nc.sync.dma_start(out=gamma_bc, in_=bcast(p_gamma, 128, [[1, D]]))
            beta_bc = const.tile([128, D], F32)
            nc.sync.dma_start(out=beta_bc, in_=bcast(p_beta, 128, [[1, D]]))
            wrow_bc = const.tile([128, ORDER], F32)
            nc.sync.dma_start(out=wrow_bc, in_=bcast(p_wrow, 128, [[1, ORDER]]))
            bgcn_bc = const.tile([128, ORDER], F32)
            nc.sync.dma_start(out=bgcn_bc, in_=bcast(p_bgcn, 128, [[1, ORDER]]))
            mask_sb = const.tile([H, H * N], F32)
            nc.sync.dma_start(out=mask_sb, in_=p_mask)
            wlin_sb = const.tile([ORDER, ORDER], F32)
            nc.sync.dma_start(out=wlin_sb, in_=p_wlin)
            kcorr_sb = const.tile([ORDER, 1], F32)
            nc.sync.dma_start(out=kcorr_sb, in_=p_kcorr)
            blin4_sb = const.tile([ORDER, 1], F32)
            nc.sync.dma_start(out=blin4_sb, in_=p_blin4)
            eps_col = const.tile([128, 1], F32)
            nc.vector.memset(eps_col, 1e-5)

            # persistent big tiles
            acc_all = persist.tile([128, GPC * NCHUNK * D], F32)    # 4 MB
            outT = persist.tile([128, 4 * GPC * N], F32R)           # 4 MB
            dcrow2 = dram.tile([GPC, H * ORDER], F32)
            dcrow_exp = dram.tile([GPC, DH * H * ORDER], F32)
            ddinv = dram.tile([GPC, H * N], F32)

            # 'output' input, loaded node-major then PE-transposed to
            # feature-major: outT[f in fb-block, fb*2048 + g*256 + n].
            with tc.tile_pool(name="oin", bufs=3) as oinp:
                for nch in range(GPC * NCHUNK):
                    g, dc = divmod(nch, NCHUNK)
                    on = oinp.tile([128, D], F32, tag="on")
                    nc.sync.dma_start(out=on, in_=p_oin[dc * 128:(dc + 1) * 128, g, :])
                    pso = ps.tile([128, 2048], F32, tag="big")
                    for fb in range(4):
                        nc.tensor.transpose(pso[:, fb * 128:(fb + 1) * 128],
                                            on[:, fb * 128:(fb + 1) * 128], identity)
                    nc.scalar.copy(
                        out=outT.rearrange("p (f x) -> p f x", f=4)
                            [:, :, nch * 128:(nch + 1) * 128],
                        in_=pso[:, 0:512].rearrange("p (f n) -> p f n", f=4))

            # =====================================================
            # Phase A: stage-1 GCN coefficients (streaming form)
            # =====================================================
            with tc.tile_pool(name="stage1", bufs=3) as s1p, \
                 tc.tile_pool(name="stage1s", bufs=4) as s1s, \
                 tc.tile_pool(name="s1w", bufs=1) as s1w:
                for g in range(GPC):
                    Atr = []
                    for ic in range(2):
                        a = s1p.tile([128, H * N], F32, tag="attn")
                        nc.sync.dma_start(
                            out=a.rearrange("p (h j) -> p h j", h=H),
                            in_=p_attn[g, :, ic * 128:(ic + 1) * 128, :]
                                .rearrange("h i j -> i h j"))
                        ar = s1p.tile([128, H * N], F32R, tag="attnr")
                        nc.gpsimd.tensor_copy(out=ar, in_=a)
                        Atr.append(ar)
                    # r1: colsum rows [1, (h, j)]
                    psA = ps.tile([128, 2048], F32, tag="big")
                    for w in range(4):
                        for ic in range(2):
                            nc.tensor.matmul(
                                out=psA[0:1, w * 512:(w + 1) * 512],
                                lhsT=ones_r, rhs=Atr[ic][:, w * 512:(w + 1) * 512],
                                start=(ic == 0), stop=(ic == 1))
                    # dinv row = 1/sqrt(colsum + 1)
                    sqrow = s1w.tile([1, H * N], F32, tag="sqrow")
                    nc.scalar.activation(out=sqrow, in_=psA[0:1, 0:2048],
                                         func=AF.Sqrt, bias=1.0)
                    dinvrow = s1w.tile([1, H * N], F32, tag="dinvrow")
                    nc.vector.reciprocal(out=dinvrow, in_=sqrow)
                    nc.sync.dma_start(out=ddinv[g:g + 1, :], in_=dinvrow)
                    # transpose dinv row-chunks -> columns [128, (ic, h)]
                    psT = ps.tile([128, 2048], F32, tag="big")
                    for ic in range(2):
                        for h in range(H):
                            nc.tensor.transpose(
                                psT[:, (ic * 8 + h) * 128:(ic * 8 + h) * 128 + 1],
                                dinvrow[0:1, h * N + ic * 128:h * N + (ic + 1) * 128],
                                identity[0:1, 0:1])
                    dinvT = s1s.tile([128, 16], F32R, tag="dinvT")
                    nc.scalar.copy(
                        out=dinvT,
                        in_=psT[:, 0:2048].rearrange("p (c x) -> p c x", c=16)[:, :, 0])
                    # r2: dot rows [8, (h', j)] via block-diag dinv lhsT
                    psD = ps.tile([128, 2048], F32, tag="big")
                    for w in range(4):
                        for ic in range(2):
                            nc.tensor.matmul(
                                out=psD[0:8, w * 512:(w + 1) * 512],
                                lhsT=dinvT[:, ic * 8:(ic + 1) * 8],
                                rhs=Atr[ic][:, w * 512:(w + 1) * 512],
                                start=(ic == 0), stop=(ic == 1))
                    # s = (dot + dinv) * dinv, diag-masked
                    dinvB = s1w.tile([H, H * N], F32, tag="dinvB")
                    nc.sync.dma_start(out=dinvB,
                                      in_=bcast(ddinv[0], H, [[1, H * N]],
                                                extra_off=g * H * N))
                    maskD = s1w.tile([H, H * N], F32, tag="maskD")
                    nc.vector.tensor_mul(maskD, dinvB, mask_sb)
                    t8 = s1w.tile([H, H * N], F32, tag="t8")
                    nc.vector.tensor_add(t8, psD[0:8, 0:2048], dinvB)
                    sm = s1w.tile([H, H * N], F32, tag="sm")
                    nc.vector.tensor_mul(sm, t8, maskD)
                    # tanh + GAP (accumulate); off-diag contributes tanh(b_c)
                    gacc = s1s.tile([H, ORDER], F32, tag="gacc")
                    for c in range(ORDER):
                        junk = s1w.tile([H, H * N], F32, tag="junk")
                        nc.scalar.activation(
                            out=junk, in_=sm, func=AF.Tanh,
                            scale=wrow_bc[0:H, c:c + 1], bias=bgcn_bc[0:H, c:c + 1],
                            accum_out=gacc[:, c:c + 1])
                    # coeff = (gacc - corr)/256 @ W_lin + b_lin
                    psC = ps.tile([128, 2048], F32, tag="big")
                    nc.tensor.transpose(psC[0:ORDER, 0:H], gacc, identity[0:H, 0:H])
                    gapT = s1s.tile([ORDER, H], F32, tag="gapT")
                    nc.vector.tensor_scalar_sub(gapT, psC[0:ORDER, 0:H], kcorr_sb)
                    nc.tensor.matmul(out=psC[0:ORDER, 512:512 + H], lhsT=wlin_sb,
                                     rhs=gapT, start=True, stop=True)
                    coefT = s1s.tile([ORDER, H], F32, tag="coefT")
                    nc.vector.tensor_scalar_add(coefT, psC[0:ORDER, 512:512 + H],
                                                blin4_sb)
                    nc.tensor.transpose(psC[0:H, 1024:1024 + ORDER], coefT,
                                        identity[0:ORDER, 0:ORDER])
                    crow_sb = s1s.tile([H, ORDER], F32, tag="crow_sb")
                    nc.scalar.copy(out=crow_sb, in_=psC[0:H, 1024:1024 + ORDER])
                    nc.sync.dma_start(
                        out=dcrow2[g:g + 1, :].rearrange("o (h k) -> o h k", k=ORDER),
                        in_=crow_sb)

            # expand coefficients: dcrow_exp[g, (dh', h, k)] = c[g, h, k]
            nc.sync.dma_start(
                out=dcrow_exp.rearrange("g (d hk) -> g d hk", d=DH),
                in_=bass.AP(tensor=dcrow2.tensor, offset=dcrow2.offset,
                            ap=[[H * ORDER, GPC], [0, DH], [1, H * ORDER]]))

            # =====================================================
            # Phase B: Chebyshev propagation per graph
            # =====================================================
            with tc.tile_pool(name="st2a", bufs=2) as st2a, \
                 tc.tile_pool(name="st2b", bufs=2) as st2b, \
                 tc.tile_pool(name="st2s", bufs=4) as st2s:
                for g in range(GPC):
                    # --- C_g (cast to f32r during DMA) and degree columns ---
                    cm = []
                    for sc in range(2):
                        t = st2a.tile([128, N], F32R, tag="cm")
                        nc.gpsimd.dma_start(out=t,
                                            in_=p_cmat[g, sc * 128:(sc + 1) * 128, :])
                        cm.append(t)
                    psd = ps.tile([128, 2048], F32, tag="big")
                    for dc in range(2):
                        for sc in range(2):
                            nc.tensor.matmul(
                                out=psd[:, dc * 2:dc * 2 + 2],
                                lhsT=cm[sc][:, dc * 128:(dc + 1) * 128],
                                rhs=ones2_r, start=(sc == 0), stop=(sc == 1))
                    # masked rsqrt: dinv = m / (sqrt(deg) + 1 - m), m = sign(deg)
                    # dvar layout [128, (kind, dc)]: kinds
                    # 0=dinv 1=dsq 2=dneg 3=d3 4=d2 5=dm4
                    dvar = st2s.tile([128, 12], F32, tag="dvar")
                    sg = st2s.tile([128, 2], F32, tag="sg")
                    psdv = psd[:, 0:4].rearrange('p (d two) -> p d two', two=2)[:, :, 0]
                    nc.scalar.activation(out=sg, in_=psdv, func=AF.Sign)
                    sq = st2s.tile([128, 2], F32, tag="sq")
                    nc.scalar.activation(out=sq, in_=psdv, func=AF.Sqrt)
                    den = st2s.tile([128, 2], F32, tag="den")
                    nc.vector.scalar_tensor_tensor(
                        out=den, in0=sq, scalar=1.0, in1=sg,
                        op0=ALU.add, op1=ALU.subtract)
                    rec = st2s.tile([128, 2], F32, tag="rec")
                    nc.vector.reciprocal(out=rec, in_=den)
                    dv = dvar[:, 0:2]
                    nc.vector.tensor_mul(dv, rec, sg)
                    nc.scalar.square(dvar[:, 2:4], dv)
                    nc.scalar.mul(dvar[:, 4:6], dv, -1.0)
                    nc.scalar.mul(dvar[:, 6:8], dv, 3.0)
                    nc.scalar.mul(dvar[:, 8:10], dv, 2.0)
                    nc.scalar.mul(dvar[:, 10:12], dv, -4.0)

                    # --- x0^T via PE transpose (cast to f32r on eviction) ---
                    x0t = st2b.tile([DH, H * N], F32R, tag="x0t")
                    for nc2 in range(2):
                        x0n = st2s.tile([128, H * DH], F32, tag="x0n")
                        nc.sync.dma_start(
                            out=x0n.rearrange("p (h d) -> p h d", h=H),
                            in_=p_x0[g, nc2 * 128:(nc2 + 1) * 128, :, :])
                        psx = ps.tile([128, 2048], F32, tag="big")
                        for h in range(H):
                            nc.tensor.transpose(psx[0:DH, h * 128:(h + 1) * 128],
                                                x0n[:, h * DH:(h + 1) * DH], identity)
                        nc.vector.tensor_copy(
                            x0t.rearrange("p (h x) -> p h x", h=H)
                               [:, :, nc2 * 128:(nc2 + 1) * 128],
                            psx[0:DH, 0:1024].rearrange("p (h n) -> p h n", h=H))

                    # --- c-scaled W_cheb:
                    # cexp[p, (dh', hk)] broadcast from dcrow_exp[g]
                    cexp = st2b.tile([DH, DH * H * ORDER], F32, tag="cexp")
                    nc.sync.dma_start(
                        out=cexp,
                        in_=bcast(dcrow_exp[0], DH, [[1, DH * H * ORDER]],
                                  extra_off=g * DH * H * ORDER))
                    wsc = st2b.tile([DH, H * ORDER * DH], F32R, tag="wsc")
                    cexpv = cexp.rearrange("p (d h2 k) -> p h2 k d", d=DH, k=ORDER)
                    for h in range(H):
                        nc.vector.tensor_mul(
                            wsc[:, h * 256:(h + 1) * 256]
                                .rearrange("p (k d) -> p k d", k=ORDER),
                            cexpv[:, h, :, :],
                            wcheb_sb.rearrange("p (k d) -> p k d", k=ORDER))

                    # --- Z matmuls:  Z[(n),(k,dh')] = x0 @ (c_k W_k) ---
                    accv = []
                    v0 = []
                    for dc in range(2):
                        pz = ps.tile([128, 2048], F32, tag="big")
                        for h in range(H):
                            nc.tensor.matmul(
                                out=pz[:, h * 256:(h + 1) * 256],
                                lhsT=x0t[:, h * N + dc * 128:h * N + (dc + 1) * 128],
                                rhs=wsc[:, h * 256:(h + 1) * 256],
                                start=True, stop=True)
                        pzv = pz.rearrange("p (h k d) -> p h k d", h=H, k=ORDER)
                        a = acc_all[:, (g * 2 + dc) * D:(g * 2 + dc + 1) * D] \
                            .rearrange("p (h d) -> p h d", h=H)
                        accv.append(a)
                        # acc = Z0 - Z2  (one PSUM operand per instruction)
                        nc.scalar.mul(a, pzv[:, :, 2, :], -1.0)
                        nc.vector.tensor_tensor(
                            out=a, in0=pzv[:, :, 0, :], in1=a, op=ALU.add)
                        # v0 = dinv * Z[1:4]
                        v = st2a.tile([128, H * 3 * DH], F32R, tag="v0")
                        nc.scalar.activation(
                            out=v.rearrange("p (h k d) -> p h k d", h=H, k=3),
                            in_=pzv[:, :, 1:4, :], func=AF.Copy,
                            scale=dvar[:, dc:dc + 1])
                        v0.append(v)

                    # --- propagation round 1: y1 = C^T v0 ---
                    v1 = []
                    for dc in range(2):
                        py = ps.tile([128, 2048], F32, tag="big")
                        for w in range(3):
                            for sc in range(2):
                                nc.tensor.matmul(
                                    out=py[:, w * 512:(w + 1) * 512],
                                    lhsT=cm[sc][:, dc * 128:(dc + 1) * 128],
                                    rhs=v0[sc][:, w * 512:(w + 1) * 512],
                                    start=(sc == 0), stop=(sc == 1))
                        pyv = py[:, 0:1536].rearrange("p (h k d) -> p h k d", h=H, k=3)
                        a = accv[dc]
                        # acc += -dinv * y1[k=1]
                        nc.vector.scalar_tensor_tensor(
                            out=a, in0=pyv[:, :, 0, :],
                            scalar=dvar[:, 4 + dc:5 + dc],
                            in1=a, op0=ALU.mult, op1=ALU.add)
                        # acc += 3 dinv * y1[k=3]
                        nc.vector.scalar_tensor_tensor(
                            out=a, in0=pyv[:, :, 2, :],
                            scalar=dvar[:, 6 + dc:7 + dc],
                            in1=a, op0=ALU.mult, op1=ALU.add)
                        # v1 = dinv^2 * y1[k=2,3]
                        v = st2a.tile([128, H * 2 * DH], F32R, tag="v1")
                        nc.scalar.activation(
                            out=v.rearrange("p (h k d) -> p h k d", h=H, k=2),
                            in_=pyv[:, :, 1:3, :], func=AF.Copy,
                            scale=dvar[:, 2 + dc:3 + dc])
                        v1.append(v)

                    # --- round 2 ---
                    v2 = []
                    for dc in range(2):
                        py = ps.tile([128, 2048], F32, tag="big")
                        for w in range(2):
                            for sc in range(2):
                                nc.tensor.matmul(
                                    out=py[:, w * 512:(w + 1) * 512],
                                    lhsT=cm[sc][:, dc * 128:(dc + 1) * 128],
                                    rhs=v1[sc][:, w * 512:(w + 1) * 512],
                                    start=(sc == 0), stop=(sc == 1))
                        pyv = py[:, 0:1024].rearrange("p (h k d) -> p h k d", h=H, k=2)
                        a = accv[dc]
                        # acc += 2 dinv * y2[k=2]
                        nc.vector.scalar_tensor_tensor(
                            out=a, in0=pyv[:, :, 0, :],
                            scalar=dvar[:, 8 + dc:9 + dc],
                            in1=a, op0=ALU.mult, op1=ALU.add)
                        v = st2a.tile([128, H * DH], F32R, tag="v2")
                        nc.scalar.activation(
                            out=v.rearrange("p (h d) -> p h d", h=H),
                            in_=pyv[:, :, 1, :], func=AF.Copy,
                            scale=dvar[:, 2 + dc:3 + dc])
                        v2.append(v)

                    # --- round 3 ---
                    for dc in range(2):
                        py = ps.tile([128, 2048], F32, tag="big")
                        for sc in range(2):
                            nc.tensor.matmul(
                                out=py[:, 0:512],
                                lhsT=cm[sc][:, dc * 128:(dc + 1) * 128],
                                rhs=v2[sc], start=(sc == 0), stop=(sc == 1))
                        a = accv[dc]
                        # acc += -4 dinv * y3
                        nc.vector.scalar_tensor_tensor(
                            out=a,
                            in0=py[:, 0:512].rearrange("p (h d) -> p h d", h=H),
                            scalar=dvar[:, 10 + dc:11 + dc],
                            in1=a, op0=ALU.mult, op1=ALU.add)

            # =====================================================
            # Phase C: transpose acc, cat-matmul, LayerNorm, store
            # =====================================================
            with tc.tile_pool(name="ph_c", bufs=1) as phc, \
                 tc.tile_pool(name="ln", bufs=3) as lnp, \
                 tc.tile_pool(name="lns", bufs=4) as lns:
                accT = phc.tile([128, 4 * GPC * N], F32R)
                for nch in range(GPC * NCHUNK):
                    pst = ps.tile([128, 2048], F32, tag="big")
                    for fb in range(4):
                        nc.tensor.transpose(
                            pst[:, fb * 128:(fb + 1) * 128],
                            acc_all[:, nch * D + fb * 128:nch * D + (fb + 1) * 128],
                            identity)
                    nc.scalar.copy(
                        out=accT.rearrange("p (f x) -> p f x", f=4)
                            [:, :, nch * 128:(nch + 1) * 128],
                        in_=pst[:, 0:512].rearrange("p (f n) -> p f n", f=4))

                for nch in range(GPC * NCHUNK):
                    g, dc = divmod(nch, NCHUNK)
                    po = ps.tile([128, 2048], F32, tag="big")
                    for fb in range(4):
                        nc.tensor.matmul(
                            out=po[:, 0:512],
                            lhsT=outT[:, fb * (GPC * N) + nch * 128:
                                      fb * (GPC * N) + (nch + 1) * 128],
                            rhs=wcat_sb[:, fb * D:(fb + 1) * D],
                            start=(fb == 0), stop=False)
                    for fb in range(4):
                        nc.tensor.matmul(
                            out=po[:, 0:512],
                            lhsT=accT[:, fb * (GPC * N) + nch * 128:
                                      fb * (GPC * N) + (nch + 1) * 128],
                            rhs=wcat_sb[:, (4 + fb) * D:(5 + fb) * D],
                            start=False, stop=(fb == 3))

                    # LayerNorm
                    t0 = lnp.tile([128, D], F32, tag="t0")
                    musum = lns.tile([128, 1], F32, tag="musum")
                    nc.vector.scalar_tensor_tensor(
                        out=t0, in0=po[:, 0:512], scalar=1.0, in1=bcat2_bc,
                        op0=ALU.mult, op1=ALU.add, accum_out=musum)
                    negmu = lns.tile([128, 1], F32, tag="negmu")
                    nc.scalar.mul(negmu, musum, -1.0 / D)
                    sqs = lns.tile([128, 1], F32, tag="sqs")
                    sq = lnp.tile([128, D], F32, tag="sq")
                    nc.scalar.activation(out=sq, in_=t0, func=AF.Square,
                                         bias=negmu, accum_out=sqs)
                    sd = lns.tile([128, 1], F32, tag="sd")
                    nc.scalar.activation(out=sd, in_=sqs, func=AF.Sqrt,
                                         scale=1.0 / D, bias=eps_col)
                    rstd = lns.tile([128, 1], F32, tag="rstd")
                    nc.vector.reciprocal(out=rstd, in_=sd)
                    nmr = lns.tile([128, 1], F32, tag="nmr")
                    nc.vector.tensor_mul(nmr, negmu, rstd)
                    t1 = lnp.tile([128, D], F32, tag="t1")
                    nc.scalar.activation(out=t1, in_=t0, func=AF.Identity,
                                         scale=rstd, bias=nmr)
                    t2 = lnp.tile([128, D], F32, tag="t2")
                    nc.vector.tensor_mul(t2, t1, gamma_bc)
                    nc.gpsimd.tensor_add(t2, t2, beta_bc)
                    nc.sync.dma_start(out=p_outy[dc * 128:(dc + 1) * 128, g, :], in_=t2)

    if not nc.is_finalized():
        nc.finalize()
    return nc


# ---------------------------------------------------------------------------
# host side
# ---------------------------------------------------------------------------

def _canonical_indices(feature_indices, batch):
    fi = np.asarray(feature_indices)
    bt = np.asarray(batch)
    want0 = np.repeat(np.arange(B), N)
    want1 = np.tile(np.arange(N), B)
    return (fi.shape == (B * N, 2) and bt.shape == (B * N,)
            and np.array_equal(fi[:, 0], want0) and np.array_equal(fi[:, 1], want1)
            and np.array_equal(bt, want0))


def _prep(inputs):
    """Host-side sharding + index preprocessing. Returns per-core input maps."""
    attn = np.ascontiguousarray(np.asarray(inputs["attn"], np.float32))
    oeh = np.ascontiguousarray(np.asarray(inputs["out_each_head"], np.float32))
    outp = np.ascontiguousarray(np.asarray(inputs["output"], np.float32))
    ei = np.asarray(inputs["edge_index"])

    W_gcn = np.asarray(inputs["W_gcn"], np.float32)
    b_gcn = np.asarray(inputs["b_gcn"], np.float32)
    W_lin = np.asarray(inputs["W_lin"], np.float32)
    b_lin = np.asarray(inputs["b_lin"], np.float32)
    W_cheb = np.asarray(inputs["W_cheb"], np.float32)
    b_cheb = np.asarray(inputs["b_cheb"], np.float32)
    W_cat = np.asarray(inputs["W_cat"], np.float32)
    b_cat = np.asarray(inputs["b_cat"], np.float32)
    gamma = np.asarray(inputs["gamma"], np.float32)
    beta = np.asarray(inputs["beta"], np.float32)

    # dense per-graph edge-count matrices (pure integer indexing)
    s_g = (ei[0] // N).astype(np.int64)
    s_l = (ei[0] % N).astype(np.int64)
    d_l = (ei[1] % N).astype(np.int64)
    flat = np.zeros(B * N * N, np.float32)
    np.add.at(flat, s_g * (N * N) + s_l * N + d_l, 1.0)
    cmat = flat.reshape(B, N, N)

    wcheb = np.ascontiguousarray(
        W_cheb.transpose(1, 0, 2).reshape(DH, ORDER * DH))   # [dh, (k, dh')]
    wrow = W_gcn.sum(axis=0).reshape(1, ORDER).astype(np.float32)
    bcat2 = (b_cat + np.tile(b_cheb, H) @ W_cat[D:, :]).reshape(1, D).astype(np.float32)

    # stage-1 streaming-form constants
    mask8 = np.zeros((H, H * N), np.float32)
    for h in range(H):
        mask8[h, h * N:(h + 1) * N] = 1.0
    kcorr = ((H * N - N) * np.tanh(b_gcn)).reshape(ORDER, 1).astype(np.float32)
    wlin4 = (W_lin / N).astype(np.float32)
    blin4 = b_lin.reshape(ORDER, 1).astype(np.float32)

    shared = dict(
        wcheb=wcheb, wcat=np.ascontiguousarray(W_cat),
        bcat2=bcat2, gamma=gamma.reshape(1, D).astype(np.float32),
        beta=beta.reshape(1, D).astype(np.float32),
        wrow=wrow, bgcn=b_gcn.reshape(1, ORDER).astype(np.float32),
        mask8=mask8, wlin4=wlin4, kcorr=kcorr, blin4=blin4)

    in_maps = []
    for c in range(NCORES):
        G = slice(c * GPC, (c + 1) * GPC)
        in_maps.append(dict(
            attn_s=np.ascontiguousarray(attn[G]),
            x0_s=np.ascontiguousarray(oeh[G]),
            out_in_s=np.ascontiguousarray(outp[:, G, :]),
            cmat_s=np.ascontiguousarray(cmat[G]),
            **shared))
    return in_maps


def _fallback_numpy(inputs):
    """Generic (slow) numpy path, used only if the index tensors deviate from
    the canonical layout produced by setup_inputs()."""
    output = np.asarray(inputs["output"], np.float32)
    attn = np.asarray(inputs["attn"], np.float32)
    oeh = np.asarray(inputs["out_each_head"], np.float32)
    ei = np.asarray(inputs["edge_index"])
    fi = np.asarray(inputs["feature_indices"])
    batch = np.asarray(inputs["batch"])
    W_gcn = np.asarray(inputs["W_gcn"], np.float32); b_gcn = np.asarray(inputs["b_gcn"], np.float32)
    W_lin = np.asarray(inputs["W_lin"], np.float32); b_lin = np.asarray(inputs["b_lin"], np.float32)
    W_cheb = np.asarray(inputs["W_cheb"], np.float32); b_cheb = np.asarray(inputs["b_cheb"], np.float32)
    W_cat = np.asarray(inputs["W_cat"], np.float32); b_cat = np.asarray(inputs["b_cat"], np.float32)
    gamma = np.asarray(inputs["gamma"], np.float32); beta = np.asarray(inputs["beta"], np.float32)

    Bn, Hn, Nn, _ = attn.shape
    C = W_gcn.shape[0]
    total = Bn * Nn
    NT = Hn * total
    A = attn.transpose(1, 0, 2, 3).reshape(Hn * Bn, Nn, Nn)
    A_hat = A + np.eye(Nn, dtype=A.dtype)
    deg = A_hat.sum(axis=1)
    dinv = np.where(deg > 0, 1.0 / np.sqrt(deg), 0.0).astype(np.float32)
    w_row = W_gcn.sum(axis=0)
    s = np.einsum('bi,bij->bj', dinv, A_hat) * dinv
    x_c = np.tanh(s[:, :, None] * w_row + b_gcn)
    gap = x_c.mean(axis=1)
    coeff = gap @ W_lin + b_lin

    offsets = (np.arange(Hn) * total).astype(ei.dtype)
    src = (ei[0][None, :] + offsets[:, None]).reshape(-1)
    dst = (ei[1][None, :] + offsets[:, None]).reshape(-1)
    deg_n = np.zeros(NT, np.float32)
    np.add.at(deg_n, dst, 1.0)
    dinv_n = np.where(deg_n > 0, 1.0 / np.sqrt(np.maximum(deg_n, 1e-30)), 0.0).astype(np.float32)
    norm_e = -(dinv_n[src] * dinv_n[dst])

    def prop(x):
        out = np.zeros((NT, x.shape[1]), np.float32)
        np.add.at(out, dst, norm_e[:, None] * x[src])
        return out

    x0 = oeh.transpose(2, 0, 1, 3).reshape(NT, DH)
    batch_all = (batch[None, :] + (np.arange(Hn) * Bn)[:, None]).reshape(-1)
    c_node = coeff[batch_all]
    T_prev, T_cur = x0, prop(x0)
    acc = c_node[:, 0:1] * (T_prev @ W_cheb[0]) + c_node[:, 1:2] * (T_cur @ W_cheb[1])
    for k in range(2, C):
        T_next = 2.0 * prop(T_cur) - T_prev
        acc = acc + c_node[:, k:k + 1] * (T_next @ W_cheb[k])
        T_prev, T_cur = T_cur, T_next
    acc = acc + b_cheb
    filtered = acc.reshape(Hn, total, DH).transpose(1, 0, 2).reshape(total, Hn * DH)
    out_filtered = np.zeros_like(output)
    out_filtered[fi[:, 1], fi[:, 0], :] = filtered
    out_cat = np.concatenate([output, out_filtered], axis=-1)
    out = out_cat @ W_cat + b_cat
    mu = out.mean(axis=-1, keepdims=True)
    var = ((out - mu) ** 2).mean(axis=-1, keepdims=True)
    return ((out - mu) / np.sqrt(var + 1e-5) * gamma + beta).astype(np.float32)


def _get_nc():
    if "nc" not in _CACHE:
        _CACHE["nc"] = _build_module(use_f32r=_CACHE.get("use_f32r", True))
    return _CACHE["nc"]


def kernel(**inputs) -> np.ndarray:
    if not _canonical_indices(inputs["feature_indices"], inputs["batch"]):
        return _fallback_numpy(inputs)

    from concourse.bass_utils import run_bass_kernel_spmd

    nc = _get_nc()
    in_maps = _prep(inputs)
    res = run_bass_kernel_spmd(nc, in_maps, list(range(NCORES)))
    out = np.empty((N, B, D), np.float32)
    for c in range(NCORES):
        out[:, c * GPC:(c + 1) * GPC, :] = res.results[c]["outy"]
    return out


# revision 30
# speedup vs baseline: 1.9215x; 1.1762x over previous
"""Trainium2 Bass kernel for nn_DiffGraphTransformerGenGCN.

Strategy (see spec sharding hint): data-parallel over the 64 graphs, 8 graphs
per NeuronCore.  Per core everything is reformulated as dense per-graph
matmuls:

  stage 1 (GCN filter coefficients over the attention graphs):
    streaming colsum / dinv-weighted colsum of each 256x256 attention matrix
    (ones / block-diagonal dinv as the stationary operand, attention tiles
    streamed once in float32r), masked tanh with accumulate for the global
    average pool, tiny matmuls for the final linear -> 4 coefficients per
    (graph, head).

  stage 2 (dynamic Chebyshev filtering on the input graph):
    The edge list is converted (host-side, pure integer indexing) into a
    dense 256x256 edge-count matrix C_g per graph.  The scaled Laplacian
    action is  P x = -D C^T D x  with D = diag(rsqrt(indeg)).  The Chebyshev
    coefficients c_(g,h,k) (stage-1 output) are folded into per-(g,h)
    scaled copies of W_cheb, so  acc = sum_k c_k T_k(P) (x0 W_k)  is computed
    with 6 dense propagation matmuls per graph and no transposes.

  final: out = LayerNorm(concat(output, filtered) @ W_cat + b_cat') computed
    as a feat-major matmul (lhsT = PE-transposed operands).

All large matmuls run in float32r (1 cycle/row vs 4 for fp32); operands are
rounded to f32r at their producers (engine copy-casts / SWDGE cast-DMA).

kernel() takes FULL inputs and returns the FULL output; it shards across the
8 cores internally.
"""

import numpy as np

B, N, H, DH, ORDER = 64, 256, 8, 64, 4
D = H * DH
NCORES = 8
GPC = B // NCORES          # graphs per core
NCHUNK = N // 128          # node chunks per graph (2)

_CACHE = {}


# ---------------------------------------------------------------------------
# device kernel construction
# ---------------------------------------------------------------------------

def _build_module(use_f32r=True):
    import concourse.bass as bass
    import concourse.bacc as bacc
    import concourse.tile as tile
    from concourse import mybir
    from concourse.masks import make_identity

    F32 = mybir.dt.float32
    F32R = mybir.dt.float32r if use_f32r else mybir.dt.float32
    AF = mybir.ActivationFunctionType
    ALU = mybir.AluOpType

    nc = bacc.Bacc(None, target_bir_lowering=False, debug=False)

    # ---- DRAM parameters (per-core shard) ----
    p_attn = nc.declare_dram_parameter("attn_s", [GPC, H, N, N], F32, isOutput=False).ap()
    p_x0 = nc.declare_dram_parameter("x0_s", [GPC, N, H, DH], F32, isOutput=False).ap()
    p_oin = nc.declare_dram_parameter("out_in_s", [N, GPC, D], F32, isOutput=False).ap()
    p_cmat = nc.declare_dram_parameter("cmat_s", [GPC, N, N], F32, isOutput=False).ap()
    p_wcheb = nc.declare_dram_parameter("wcheb", [DH, ORDER * DH], F32, isOutput=False).ap()
    p_wcat = nc.declare_dram_parameter("wcat", [2 * D, D], F32, isOutput=False).ap()
    p_bcat2 = nc.declare_dram_parameter("bcat2", [1, D], F32, isOutput=False).ap()
    p_gamma = nc.declare_dram_parameter("gamma", [1, D], F32, isOutput=False).ap()
    p_beta = nc.declare_dram_parameter("beta", [1, D], F32, isOutput=False).ap()
    p_wrow = nc.declare_dram_parameter("wrow", [1, ORDER], F32, isOutput=False).ap()
    p_bgcn = nc.declare_dram_parameter("bgcn", [1, ORDER], F32, isOutput=False).ap()
    p_mask = nc.declare_dram_parameter("mask8", [H, H * N], F32, isOutput=False).ap()
    p_wlin = nc.declare_dram_parameter("wlin4", [ORDER, ORDER], F32, isOutput=False).ap()
    p_kcorr = nc.declare_dram_parameter("kcorr", [ORDER, 1], F32, isOutput=False).ap()
    p_blin4 = nc.declare_dram_parameter("blin4", [ORDER, 1], F32, isOutput=False).ap()
    p_outy = nc.declare_dram_parameter("outy", [N, GPC, D], F32, isOutput=True).ap()

    def bcast(dram_ap, parts, inner, extra_off=0):
        return bass.AP(tensor=dram_ap.tensor, offset=dram_ap.offset + extra_off,
                       ap=[[0, parts]] + inner)

    with tile.TileContext(nc) as tc:
        import contextlib
        ctx = contextlib.ExitStack()
        with ctx:
            const = ctx.enter_context(tc.tile_pool(name="const", bufs=1))
            persist = ctx.enter_context(tc.tile_pool(name="persist", bufs=1))
            ps = ctx.enter_context(tc.tile_pool(name="ps", bufs=2, space="PSUM"))
            dram = ctx.enter_context(tc.tile_pool(name="dram", bufs=1, space="DRAM"))

            # ---------------- constants ----------------
            ones_col = const.tile([128, 1], F32)
            nc.vector.memset(ones_col, 1.0)
            ones_r = const.tile([128, 1], F32R)
            nc.vector.tensor_copy(ones_r, ones_col)
            ones2 = const.tile([128, 2], F32)
            nc.vector.memset(ones2, 1.0)
            ones2_r = const.tile([128, 2], F32R)
            nc.vector.tensor_copy(ones2_r, ones2)
            identity = const.tile([128, 128], F32)
            make_identity(nc, identity)
            wcheb_sb = const.tile([DH, ORDER * DH], F32)
            nc.sync.dma_start(out=wcheb_sb, in_=p_wcheb)
            wcat_sb = const.tile([128, 8 * D], F32R)
            for fb in range(8):
                nc.gpsimd.dma_start(out=wcat_sb[:, fb * D:(fb + 1) * D],
                                    in_=p_wcat[fb * 128:(fb + 1) * 128, :])
            bcat2_bc = const.tile([128, D], F32)
            nc.sync.dma_start(out=bcat2_bc, in_=bcast(p_bcat2, 128, [[1, D]]))
            gamma_bc = const.tile([128, D], F32)
            nc.sync.dma_start(out=gamma_bc, in_=bcast(p_gamma, 128, [[1, D]]))
            beta_bc = const.tile([128, D], F32)
            nc.sync.dma_start(out=beta_bc, in_=bcast(p_beta, 128, [[1, D]]))
            wrow_bc = const.tile([128, ORDER], F32)
            nc.sync.dma_start(out=wrow_bc, in_=bcast(p_wrow, 128, [[1, ORDER]]))
            bgcn_bc = const.tile([128, ORDER], F32)
            nc.sync.dma_start(out=bgcn_bc, in_=bcast(p_bgcn, 128, [[1, ORDER]]))
            mask_sb = const.tile([H, H * N], F32)
            nc.sync.dma_start(out=mask_sb, in_=p_mask)
            wlin_sb = const.tile([ORDER, ORDER], F32)
            nc.sync.dma_start(out=wlin_sb, in_=p_wlin)
            kcorr_sb = const.tile([ORDER, 1], F32)
            nc.sync.dma_start(out=kcorr_sb, in_=p_kcorr)
            blin4_sb = const.tile([ORDER, 1], F32)
            nc.sync.dma_start(out=blin4_sb, in_=p_blin4)
            eps_col = const.tile([128, 1], F32)
            nc.vector.memset(eps_col, 1e-5)

            # persistent big tiles
            acc_all = persist.tile([128, GPC * NCHUNK * D], F32)    # 4 MB
            outT = persist.tile([128, 4 * GPC * N], F32R)           # 4 MB
            dcrow2 = dram.tile([GPC, H * ORDER], F32)
            dcrow_exp = dram.tile([GPC, DH * H * ORDER], F32)
            ddinv = dram.tile([GPC, H * N], F32)

            # 'output' input, loaded node-major then PE-transposed to
            # feature-major: outT[f in fb-block, fb*2048 + g*256 + n].
            with tc.tile_pool(name="oin", bufs=3) as oinp:
                for nch in range(GPC * NCHUNK):
                    g, dc = divmod(nch, NCHUNK)
                    on = oinp.tile([128, D], F32, tag="on")
                    nc.sync.dma_start(out=on, in_=p_oin[dc * 128:(dc + 1) * 128, g, :])
                    pso = ps.tile([128, 2048], F32, tag="big")
                    for fb in range(4):
                        nc.tensor.transpose(pso[:, fb * 128:(fb + 1) * 128],
                                            on[:, fb * 128:(fb + 1) * 128], identity)
                    nc.scalar.copy(
                        out=outT.rearrange("p (f x) -> p f x", f=4)
                            [:, :, nch * 128:(nch + 1) * 128],
                        in_=pso[:, 0:512].rearrange("p (f n) -> p f n", f=4))

            # =====================================================
            # Phase A: stage-1 GCN coefficients (streaming form)
            # =====================================================
            with tc.tile_pool(name="stage1", bufs=3) as s1p, \
                 tc.tile_pool(name="stage1s", bufs=4) as s1s, \
                 tc.tile_pool(name="s1w", bufs=1) as s1w:
                for g in range(GPC):
                    Atr = []
                    for ic in range(2):
                        a = s1p.tile([128, H * N], F32, tag="attn")
                        nc.sync.dma_start(
                            out=a.rearrange("p (h j) -> p h j", h=H),
                            in_=p_attn[g, :, ic * 128:(ic + 1) * 128, :]
                                .rearrange("h i j -> i h j"))
                        ar = s1p.tile([128, H * N], F32R, tag="attnr")
                        nc.scalar.copy(out=ar, in_=a)
                        Atr.append(ar)
                    # r1: colsum rows [1, (h, j)]  (one psum tile per graph)
                    psA = ps.tile([128, 2048], F32, tag="big")
                    psT = psA
                    psD = psA
                    psC = psA
                    for w in range(4):
                        for ic in range(2):
                            nc.tensor.matmul(
                                out=psA[0:1, w * 512:(w + 1) * 512],
                                lhsT=ones_r, rhs=Atr[ic][:, w * 512:(w + 1) * 512],
                                start=(ic == 0), stop=(ic == 1))
                    # dinv row = 1/sqrt(colsum + 1)
                    sqrow = s1w.tile([1, H * N], F32, tag="sqrow")
                    nc.scalar.activation(out=sqrow, in_=psA[0:1, 0:2048],
                                         func=AF.Sqrt, bias=1.0)
                    dinvrow = s1w.tile([1, H * N], F32, tag="dinvrow")
                    nc.vector.reciprocal(out=dinvrow, in_=sqrow)
                    nc.sync.dma_start(out=ddinv[g:g + 1, :], in_=dinvrow)
                    # transpose dinv row-chunks -> columns [128, (ic, h)]
                    for ic in range(2):
                        for h in range(H):
                            nc.tensor.transpose(
                                psT[:, (ic * 8 + h) * 128:(ic * 8 + h) * 128 + 1],
                                dinvrow[0:1, h * N + ic * 128:h * N + (ic + 1) * 128],
                                identity[0:1, 0:1])
                    dinvT = s1s.tile([128, 16], F32R, tag="dinvT")
                    nc.scalar.copy(
                        out=dinvT,
                        in_=psT[:, 0:2048].rearrange("p (c x) -> p c x", c=16)[:, :, 0])
                    # r2: dot rows [8, (h', j)] via block-diag dinv lhsT
                    for w in range(4):
                        for ic in range(2):
                            nc.tensor.matmul(
                                out=psD[0:8, w * 512:(w + 1) * 512],
                                lhsT=dinvT[:, ic * 8:(ic + 1) * 8],
                                rhs=Atr[ic][:, w * 512:(w + 1) * 512],
                                start=(ic == 0), stop=(ic == 1))
                    # s = (dot + dinv) * dinv, diag-masked
                    dinvB = s1w.tile([H, H * N], F32, tag="dinvB")
                    nc.sync.dma_start(out=dinvB,
                                      in_=bcast(ddinv[0], H, [[1, H * N]],
                                                extra_off=g * H * N))
                    maskD = s1w.tile([H, H * N], F32, tag="maskD")
                    nc.vector.tensor_mul(maskD, dinvB, mask_sb)
                    t8 = s1w.tile([H, H * N], F32, tag="t8")
                    nc.vector.tensor_add(t8, psD[0:8, 0:2048], dinvB)
                    sm = s1w.tile([H, H * N], F32, tag="sm")
                    nc.vector.tensor_mul(sm, t8, maskD)
                    # tanh + GAP (accumulate); off-diag contributes tanh(b_c)
                    gacc = s1s.tile([H, ORDER], F32, tag="gacc")
                    for c in range(ORDER):
                        junk = s1w.tile([H, H * N], F32, tag="junk")
                        nc.scalar.activation(
                            out=junk, in_=sm, func=AF.Tanh,
                            scale=wrow_bc[0:H, c:c + 1], bias=bgcn_bc[0:H, c:c + 1],
                            accum_out=gacc[:, c:c + 1])
                    # coeff = (gacc - corr)/256 @ W_lin + b_lin
                    nc.tensor.transpose(psC[0:ORDER, 0:H], gacc, identity[0:H, 0:H])
                    gapT = s1s.tile([ORDER, H], F32, tag="gapT")
                    nc.vector.tensor_scalar_sub(gapT, psC[0:ORDER, 0:H], kcorr_sb)
                    nc.tensor.matmul(out=psC[0:ORDER, 512:512 + H], lhsT=wlin_sb,
                                     rhs=gapT, start=True, stop=True)
                    coefT = s1s.tile([ORDER, H], F32, tag="coefT")
                    nc.vector.tensor_scalar_add(coefT, psC[0:ORDER, 512:512 + H],
                                                blin4_sb)
                    nc.tensor.transpose(psC[0:H, 1024:1024 + ORDER], coefT,
                                        identity[0:ORDER, 0:ORDER])
                    crow_sb = s1s.tile([H, ORDER], F32, tag="crow_sb")
                    nc.scalar.copy(out=crow_sb, in_=psC[0:H, 1024:1024 + ORDER])
                    nc.sync.dma_start(
                        out=dcrow2[g:g + 1, :].rearrange("o (h k) -> o h k", k=ORDER),
                        in_=crow_sb)

            # expand coefficients: dcrow_exp[g, (dh', h, k)] = c[g, h, k]
            nc.sync.dma_start(
                out=dcrow_exp.rearrange("g (d hk) -> g d hk", d=DH),
                in_=bass.AP(tensor=dcrow2.tensor, offset=dcrow2.offset,
                            ap=[[H * ORDER, GPC], [0, DH], [1, H * ORDER]]))

            # =====================================================
            # Phase B: Chebyshev propagation per graph
            # =====================================================
            with tc.tile_pool(name="st2a", bufs=2) as st2a, \
                 tc.tile_pool(name="st2b", bufs=2) as st2b, \
                 tc.tile_pool(name="st2s", bufs=4) as st2s:
                for g in range(GPC):
                    # --- C_g (cast to f32r during DMA) and degree columns ---
                    cm = []
                    for sc in range(2):
                        t = st2a.tile([128, N], F32R, tag="cm")
                        nc.gpsimd.dma_start(out=t,
                                            in_=p_cmat[g, sc * 128:(sc + 1) * 128, :])
                        cm.append(t)
                    psd = ps.tile([128, 2048], F32, tag="big")
                    for dc in range(2):
                        for sc in range(2):
                            nc.tensor.matmul(
                                out=psd[:, dc * 2:dc * 2 + 2],
                                lhsT=cm[sc][:, dc * 128:(dc + 1) * 128],
                                rhs=ones2_r, start=(sc == 0), stop=(sc == 1))
                    # masked rsqrt: dinv = m / (sqrt(deg) + 1 - m), m = sign(deg)
                    # dvar layout [128, (kind, dc)]: kinds
                    # 0=dinv 1=dsq 2=dneg 3=d3 4=d2 5=dm4
                    dvar = st2s.tile([128, 12], F32, tag="dvar")
                    sg = st2s.tile([128, 2], F32, tag="sg")
                    psdv = psd[:, 0:4].rearrange('p (d two) -> p d two', two=2)[:, :, 0]
                    nc.scalar.activation(out=sg, in_=psdv, func=AF.Sign)
                    sq = st2s.tile([128, 2], F32, tag="sq")
                    nc.scalar.activation(out=sq, in_=psdv, func=AF.Sqrt)
                    den = st2s.tile([128, 2], F32, tag="den")
                    nc.vector.scalar_tensor_tensor(
                        out=den, in0=sq, scalar=1.0, in1=sg,
                        op0=ALU.add, op1=ALU.subtract)
                    rec = st2s.tile([128, 2], F32, tag="rec")
                    nc.vector.reciprocal(out=rec, in_=den)
                    dv = dvar[:, 0:2]
                    nc.vector.tensor_mul(dv, rec, sg)
                    nc.scalar.square(dvar[:, 2:4], dv)
                    nc.scalar.mul(dvar[:, 4:6], dv, -1.0)
                    nc.scalar.mul(dvar[:, 6:8], dv, 3.0)
                    nc.scalar.mul(dvar[:, 8:10], dv, 2.0)
                    nc.scalar.mul(dvar[:, 10:12], dv, -4.0)

                    # --- x0^T via PE transpose (cast to f32r on eviction) ---
                    x0t = st2b.tile([DH, H * N], F32R, tag="x0t")
                    for nc2 in range(2):
                        x0n = st2s.tile([128, H * DH], F32, tag="x0n")
                        nc.sync.dma_start(
                            out=x0n.rearrange("p (h d) -> p h d", h=H),
                            in_=p_x0[g, nc2 * 128:(nc2 + 1) * 128, :, :])
                        psx = ps.tile([128, 2048], F32, tag="big")
                        for h in range(H):
                            nc.tensor.transpose(psx[0:DH, h * 128:(h + 1) * 128],
                                                x0n[:, h * DH:(h + 1) * DH], identity)
                        nc.vector.tensor_copy(
                            x0t.rearrange("p (h x) -> p h x", h=H)
                               [:, :, nc2 * 128:(nc2 + 1) * 128],
                            psx[0:DH, 0:1024].rearrange("p (h n) -> p h n", h=H))

                    # --- c-scaled W_cheb:
                    # cexp[p, (dh', hk)] broadcast from dcrow_exp[g]
                    cexp = st2b.tile([DH, DH * H * ORDER], F32, tag="cexp")
                    nc.sync.dma_start(
                        out=cexp,
                        in_=bcast(dcrow_exp[0], DH, [[1, DH * H * ORDER]],
                                  extra_off=g * DH * H * ORDER))
                    wsc = st2b.tile([DH, H * ORDER * DH], F32R, tag="wsc")
                    cexpv = cexp.rearrange("p (d h2 k) -> p h2 k d", d=DH, k=ORDER)
                    for h in range(H):
                        nc.vector.tensor_mul(
                            wsc[:, h * 256:(h + 1) * 256]
                                .rearrange("p (k d) -> p k d", k=ORDER),
                            cexpv[:, h, :, :],
                            wcheb_sb.rearrange("p (k d) -> p k d", k=ORDER))

                    # --- Z matmuls:  Z[(n),(k,dh')] = x0 @ (c_k W_k) ---
                    accv = []
                    v0 = []
                    for dc in range(2):
                        pz = ps.tile([128, 2048], F32, tag="big")
                        for h in range(H):
                            nc.tensor.matmul(
                                out=pz[:, h * 256:(h + 1) * 256],
                                lhsT=x0t[:, h * N + dc * 128:h * N + (dc + 1) * 128],
                                rhs=wsc[:, h * 256:(h + 1) * 256],
                                start=True, stop=True)
                        pzv = pz.rearrange("p (h k d) -> p h k d", h=H, k=ORDER)
                        a = acc_all[:, (g * 2 + dc) * D:(g * 2 + dc + 1) * D] \
                            .rearrange("p (h d) -> p h d", h=H)
                        accv.append(a)
                        # acc = Z0 - Z2  (one PSUM operand per instruction)
                        nc.scalar.mul(a, pzv[:, :, 2, :], -1.0)
                        nc.vector.tensor_tensor(
                            out=a, in0=pzv[:, :, 0, :], in1=a, op=ALU.add)
                        # v0 = dinv * Z[1:4]
                        v = st2a.tile([128, H * 3 * DH], F32R, tag="v0")
                        nc.scalar.activation(
                            out=v.rearrange("p (h k d) -> p h k d", h=H, k=3),
                            in_=pzv[:, :, 1:4, :], func=AF.Copy,
                            scale=dvar[:, dc:dc + 1])
                        v0.append(v)

                    # --- propagation round 1: y1 = C^T v0 ---
                    v1 = []
                    for dc in range(2):
                        py = ps.tile([128, 2048], F32, tag="big")
                        for w in range(3):
                            for sc in range(2):
                                nc.tensor.matmul(
                                    out=py[:, w * 512:(w + 1) * 512],
                                    lhsT=cm[sc][:, dc * 128:(dc + 1) * 128],
                                    rhs=v0[sc][:, w * 512:(w + 1) * 512],
                                    start=(sc == 0), stop=(sc == 1))
                        pyv = py[:, 0:1536].rearrange("p (h k d) -> p h k d", h=H, k=3)
                        a = accv[dc]
                        # acc += -dinv * y1[k=1]
                        nc.vector.scalar_tensor_tensor(
                            out=a, in0=pyv[:, :, 0, :],
                            scalar=dvar[:, 4 + dc:5 + dc],
                            in1=a, op0=ALU.mult, op1=ALU.add)
                        # acc += 3 dinv * y1[k=3]
                        nc.vector.scalar_tensor_tensor(
                            out=a, in0=pyv[:, :, 2, :],
                            scalar=dvar[:, 6 + dc:7 + dc],
                            in1=a, op0=ALU.mult, op1=ALU.add)
                        # v1 = dinv^2 * y1[k=2,3]
                        v = st2a.tile([128, H * 2 * DH], F32R, tag="v1")
                        nc.scalar.activation(
                            out=v.rearrange("p (h k d) -> p h k d", h=H, k=2),
                            in_=pyv[:, :, 1:3, :], func=AF.Copy,
                            scale=dvar[:, 2 + dc:3 + dc])
                        v1.append(v)

                    # --- round 2 ---
                    v2 = []
                    for dc in range(2):
                        py = ps.tile([128, 2048], F32, tag="big")
                        for w in range(2):
                            for sc in range(2):
                                nc.tensor.matmul(
                                    out=py[:, w * 512:(w + 1) * 512],
                                    lhsT=cm[sc][:, dc * 128:(dc + 1) * 128],
                                    rhs=v1[sc][:, w * 512:(w + 1) * 512],
                                    start=(sc == 0), stop=(sc == 1))
                        pyv = py[:, 0:1024].rearrange("p (h k d) -> p h k d", h=H, k=2)
                        a = accv[dc]
                        # acc += 2 dinv * y2[k=2]
                        nc.vector.scalar_tensor_tensor(
                            out=a, in0=pyv[:, :, 0, :],
                            scalar=dvar[:, 8 + dc:9 + dc],
                            in1=a, op0=ALU.mult, op1=ALU.add)
                        v = st2a.tile([128, H * DH], F32R, tag="v2")
                        nc.scalar.activation(
                            out=v.rearrange("p (h d) -> p h d", h=H),
                            in_=pyv[:, :, 1, :], func=AF.Copy,
                            scale=dvar[:, 2 + dc:3 + dc])
                        v2.append(v)

                    # --- round 3 ---
                    for dc in range(2):
                        py = ps.tile([128, 2048], F32, tag="big")
                        for sc in range(2):
                            nc.tensor.matmul(
                                out=py[:, 0:512],
                                lhsT=cm[sc][:, dc * 128:(dc + 1) * 128],
                                rhs=v2[sc], start=(sc == 0), stop=(sc == 1))
                        a = accv[dc]
                        # acc += -4 dinv * y3
                        nc.vector.scalar_tensor_tensor(
                            out=a,
                            in0=py[:, 0:512].rearrange("p (h d) -> p h d", h=H),
                            scalar=dvar[:, 10 + dc:11 + dc],
                            in1=a, op0=ALU.mult, op1=ALU.add)

            # =====================================================
            # Phase C: transpose acc, cat-matmul, LayerNorm, store
            # =====================================================
            with tc.tile_pool(name="ph_c", bufs=1) as phc, \
                 tc.tile_pool(name="ln", bufs=3) as lnp, \
                 tc.tile_pool(name="lns", bufs=4) as lns:
                accT = phc.tile([128, 4 * GPC * N], F32R)
                for nch in range(GPC * NCHUNK):
                    pst = ps.tile([128, 2048], F32, tag="big")
                    for fb in range(4):
                        nc.tensor.transpose(
                            pst[:, fb * 128:(fb + 1) * 128],
                            acc_all[:, nch * D + fb * 128:nch * D + (fb + 1) * 128],
                            identity)
                    nc.scalar.copy(
                        out=accT.rearrange("p (f x) -> p f x", f=4)
                            [:, :, nch * 128:(nch + 1) * 128],
                        in_=pst[:, 0:512].rearrange("p (f n) -> p f n", f=4))

                for nch in range(GPC * NCHUNK):
                    g, dc = divmod(nch, NCHUNK)
                    po = ps.tile([128, 2048], F32, tag="big")
                    for fb in range(4):
                        nc.tensor.matmul(
                            out=po[:, 0:512],
                            lhsT=outT[:, fb * (GPC * N) + nch * 128:
                                      fb * (GPC * N) + (nch + 1) * 128],
                            rhs=wcat_sb[:, fb * D:(fb + 1) * D],
                            start=(fb == 0), stop=False)
                    for fb in range(4):
                        nc.tensor.matmul(
                            out=po[:, 0:512],
                            lhsT=accT[:, fb * (GPC * N) + nch * 128:
                                      fb * (GPC * N) + (nch + 1) * 128],
                            rhs=wcat_sb[:, (4 + fb) * D:(5 + fb) * D],
                            start=False, stop=(fb == 3))

                    # LayerNorm
                    t0 = lnp.tile([128, D], F32, tag="t0")
                    musum = lns.tile([128, 1], F32, tag="musum")
                    nc.vector.scalar_tensor_tensor(
                        out=t0, in0=po[:, 0:512], scalar=1.0, in1=bcat2_bc,
                        op0=ALU.mult, op1=ALU.add, accum_out=musum)
                    negmu = lns.tile([128, 1], F32, tag="negmu")
                    nc.scalar.mul(negmu, musum, -1.0 / D)
                    sqs = lns.tile([128, 1], F32, tag="sqs")
                    sq = lnp.tile([128, D], F32, tag="sq")
                    nc.scalar.activation(out=sq, in_=t0, func=AF.Square,
                                         bias=negmu, accum_out=sqs)
                    sd = lns.tile([128, 1], F32, tag="sd")
                    nc.scalar.activation(out=sd, in_=sqs, func=AF.Sqrt,
                                         scale=1.0 / D, bias=eps_col)
                    rstd = lns.tile([128, 1], F32, tag="rstd")
                    nc.vector.reciprocal(out=rstd, in_=sd)
                    nmr = lns.tile([128, 1], F32, tag="nmr")
                    nc.vector.tensor_mul(nmr, negmu, rstd)
                    t1 = lnp.tile([128, D], F32, tag="t1")
                    nc.scalar.activation(out=t1, in_=t0, func=AF.Identity,
                                         scale=rstd, bias=nmr)
                    t2 = lnp.tile([128, D], F32, tag="t2")
                    nc.vector.tensor_mul(t2, t1, gamma_bc)
                    nc.vector.tensor_add(t2, t2, beta_bc)
                    nc.sync.dma_start(out=p_outy[dc * 128:(dc + 1) * 128, g, :], in_=t2)

    if not nc.is_finalized():
        nc.finalize()
    return nc


# ---------------------------------------------------------------------------
# host side
# ---------------------------------------------------------------------------

def _canonical_indices(feature_indices, batch):
    fi = np.asarray(feature_indices)
    bt = np.asarray(batch)
    want0 = np.repeat(np.arange(B), N)
    want1 = np.tile(np.arange(N), B)
    return (fi.shape == (B * N, 2) and bt.shape == (B * N,)
            and np.array_equal(fi[:, 0], want0) and np.array_equal(fi[:, 1], want1)
            and np.array_equal(bt, want0))


def _prep(inputs):
    """Host-side sharding + index preprocessing. Returns per-core input maps."""
    attn = np.ascontiguousarray(np.asarray(inputs["attn"], np.float32))
    oeh = np.ascontiguousarray(np.asarray(inputs["out_each_head"], np.float32))
    outp = np.ascontiguousarray(np.asarray(inputs["output"], np.float32))
    ei = np.asarray(inputs["edge_index"])

    W_gcn = np.asarray(inputs["W_gcn"], np.float32)
    b_gcn = np.asarray(inputs["b_gcn"], np.float32)
    W_lin = np.asarray(inputs["W_lin"], np.float32)
    b_lin = np.asarray(inputs["b_lin"], np.float32)
    W_cheb = np.asarray(inputs["W_cheb"], np.float32)
    b_cheb = np.asarray(inputs["b_cheb"], np.float32)
    W_cat = np.asarray(inputs["W_cat"], np.float32)
    b_cat = np.asarray(inputs["b_cat"], np.float32)
    gamma = np.asarray(inputs["gamma"], np.float32)
    beta = np.asarray(inputs["beta"], np.float32)

    # dense per-graph edge-count matrices (pure integer indexing)
    s_g = (ei[0] // N).astype(np.int64)
    s_l = (ei[0] % N).astype(np.int64)
    d_l = (ei[1] % N).astype(np.int64)
    flat = np.zeros(B * N * N, np.float32)
    np.add.at(flat, s_g * (N * N) + s_l * N + d_l, 1.0)
    cmat = flat.reshape(B, N, N)

    wcheb = np.ascontiguousarray(
        W_cheb.transpose(1, 0, 2).reshape(DH, ORDER * DH))   # [dh, (k, dh')]
    wrow = W_gcn.sum(axis=0).reshape(1, ORDER).astype(np.float32)
    bcat2 = (b_cat + np.tile(b_cheb, H) @ W_cat[D:, :]).reshape(1, D).astype(np.float32)

    # stage-1 streaming-form constants
    mask8 = np.zeros((H, H * N), np.float32)
    for h in range(H):
        mask8[h, h * N:(h + 1) * N] = 1.0
    kcorr = ((H * N - N) * np.tanh(b_gcn)).reshape(ORDER, 1).astype(np.float32)
    wlin4 = (W_lin / N).astype(np.float32)
    blin4 = b_lin.reshape(ORDER, 1).astype(np.float32)

    shared = dict(
        wcheb=wcheb, wcat=np.ascontiguousarray(W_cat),
        bcat2=bcat2, gamma=gamma.reshape(1, D).astype(np.float32),
        beta=beta.reshape(1, D).astype(np.float32),
        wrow=wrow, bgcn=b_gcn.reshape(1, ORDER).astype(np.float32),
        mask8=mask8, wlin4=wlin4, kcorr=kcorr, blin4=blin4)

    in_maps = []
    for c in range(NCORES):
        G = slice(c * GPC, (c + 1) * GPC)
        in_maps.append(dict(
            attn_s=np.ascontiguousarray(attn[G]),
            x0_s=np.ascontiguousarray(oeh[G]),
            out_in_s=np.ascontiguousarray(outp[:, G, :]),
            cmat_s=np.ascontiguousarray(cmat[G]),
            **shared))
    return in_maps


def _fallback_numpy(inputs):
    """Generic (slow) numpy path, used only if the index tensors deviate from
    the canonical layout produced by setup_inputs()."""
    output = np.asarray(inputs["output"], np.float32)
    attn = np.asarray(inputs["attn"], np.float32)
    oeh = np.asarray(inputs["out_each_head"], np.float32)
    ei = np.asarray(inputs["edge_index"])
    fi = np.asarray(inputs["feature_indices"])
    batch = np.asarray(inputs["batch"])
    W_gcn = np.asarray(inputs["W_gcn"], np.float32); b_gcn = np.asarray(inputs["b_gcn"], np.float32)
    W_lin = np.asarray(inputs["W_lin"], np.float32); b_lin = np.asarray(inputs["b_lin"], np.float32)
    W_cheb = np.asarray(inputs["W_cheb"], np.float32); b_cheb = np.asarray(inputs["b_cheb"], np.float32)
    W_cat = np.asarray(inputs["W_cat"], np.float32); b_cat = np.asarray(inputs["b_cat"], np.float32)
    gamma = np.asarray(inputs["gamma"], np.float32); beta = np.asarray(inputs["beta"], np.float32)

    Bn, Hn, Nn, _ = attn.shape
    C = W_gcn.shape[0]
    total = Bn * Nn
    NT = Hn * total
    A = attn.transpose(1, 0, 2, 3).reshape(Hn * Bn, Nn, Nn)
    A_hat = A + np.eye(Nn, dtype=A.dtype)
    deg = A_hat.sum(axis=1)
    dinv = np.where(deg > 0, 1.0 / np.sqrt(deg), 0.0).astype(np.float32)
    w_row = W_gcn.sum(axis=0)
    s = np.einsum('bi,bij->bj', dinv, A_hat) * dinv
    x_c = np.tanh(s[:, :, None] * w_row + b_gcn)
    gap = x_c.mean(axis=1)
    coeff = gap @ W_lin + b_lin

    offsets = (np.arange(Hn) * total).astype(ei.dtype)
    src = (ei[0][None, :] + offsets[:, None]).reshape(-1)
    dst = (ei[1][None, :] + offsets[:, None]).reshape(-1)
    deg_n = np.zeros(NT, np.float32)
    np.add.at(deg_n, dst, 1.0)
    dinv_n = np.where(deg_n > 0, 1.0 / np.sqrt(np.maximum(deg_n, 1e-30)), 0.0).astype(np.float32)
    norm_e = -(dinv_n[src] * dinv_n[dst])

    def prop(x):
        out = np.zeros((NT, x.shape[1]), np.float32)
        np.add.at(out, dst, norm_e[:, None] * x[src])
        return out

    x0 = oeh.transpose(2, 0, 1, 3).reshape(NT, DH)
    batch_all = (batch[None, :] + (np.arange(Hn) * Bn)[:, None]).reshape(-1)
    c_node = coeff[batch_all]
    T_prev, T_cur = x0, prop(x0)
    acc = c_node[:, 0:1] * (T_prev @ W_cheb[0]) + c_node[:, 1:2] * (T_cur @ W_cheb[1])
    for k in range(2, C):
        T_next = 2.0 * prop(T_cur) - T_prev
        acc = acc + c_node[:, k:k + 1] * (T_next @ W_cheb[k])
        T_prev, T_cur = T_cur, T_next
    acc = acc + b_cheb
    filtered = acc.reshape(Hn, total, DH).transpose(1, 0, 2).reshape(total, Hn * DH)
    out_filtered = np.zeros_like(output)
    out_filtered[fi[:, 1], fi[:, 0], :] = filtered
    out_cat = np.concatenate([output, out_filtered], axis=-1)
    out = out_cat @ W_cat + b_cat
    mu = out.mean(axis=-1, keepdims=True)
    var = ((out - mu) ** 2).mean(axis=-1, keepdims=True)
    return ((out - mu) / np.sqrt(var + 1e-5) * gamma + beta).astype(np.float32)


def _get_nc():
    if "nc" not in _CACHE:
        _CACHE["nc"] = _build_module(use_f32r=_CACHE.get("use_f32r", True))
    return _CACHE["nc"]


def kernel(**inputs) -> np.ndarray:
    if not _canonical_indices(inputs["feature_indices"], inputs["batch"]):
        return _fallback_numpy(inputs)

    from concourse.bass_utils import run_bass_kernel_spmd

    nc = _get_nc()
    in_maps = _prep(inputs)
    res = run_bass_kernel_spmd(nc, in_maps, list(range(NCORES)))
    out = np.empty((N, B, D), np.float32)
    for c in range(NCORES):
        out[:, c * GPC:(c + 1) * GPC, :] = res.results[c]["outy"]
    return out


# revision 32
# speedup vs baseline: 2.0108x; 1.0465x over previous
"""Trainium2 Bass kernel for nn_DiffGraphTransformerGenGCN.

Strategy (see spec sharding hint): data-parallel over the 64 graphs, 8 graphs
per NeuronCore.  Per core everything is reformulated as dense per-graph
matmuls:

  stage 1 (GCN filter coefficients over the attention graphs):
    streaming colsum / dinv-weighted colsum of each 256x256 attention matrix
    (ones / block-diagonal dinv as the stationary operand, attention tiles
    streamed once in float32r), masked tanh with accumulate for the global
    average pool, tiny matmuls for the final linear -> 4 coefficients per
    (graph, head).

  stage 2 (dynamic Chebyshev filtering on the input graph):
    The edge list is converted (host-side, pure integer indexing) into a
    dense 256x256 edge-count matrix C_g per graph.  The scaled Laplacian
    action is  P x = -D C^T D x  with D = diag(rsqrt(indeg)).  The Chebyshev
    coefficients c_(g,h,k) (stage-1 output) are folded into per-(g,h)
    scaled copies of W_cheb, so  acc = sum_k c_k T_k(P) (x0 W_k)  is computed
    with 6 dense propagation matmuls per graph and no transposes.

  final: out = LayerNorm(concat(output, filtered) @ W_cat + b_cat') computed
    as a feat-major matmul (lhsT = PE-transposed operands).

All large matmuls run in float32r (1 cycle/row vs 4 for fp32); operands are
rounded to f32r at their producers (engine copy-casts / SWDGE cast-DMA).

kernel() takes FULL inputs and returns the FULL output; it shards across the
8 cores internally.
"""

import numpy as np

B, N, H, DH, ORDER = 64, 256, 8, 64, 4
D = H * DH
NCORES = 8
GPC = B // NCORES          # graphs per core
NCHUNK = N // 128          # node chunks per graph (2)

_CACHE = {}


# ---------------------------------------------------------------------------
# device kernel construction
# ---------------------------------------------------------------------------

def _build_module(use_f32r=True):
    import concourse.bass as bass
    import concourse.bacc as bacc
    import concourse.tile as tile
    from concourse import mybir
    from concourse.masks import make_identity

    F32 = mybir.dt.float32
    F32R = mybir.dt.float32r if use_f32r else mybir.dt.float32
    AF = mybir.ActivationFunctionType
    ALU = mybir.AluOpType

    nc = bacc.Bacc(None, target_bir_lowering=False, debug=False)

    # ---- DRAM parameters (per-core shard) ----
    p_attn = nc.declare_dram_parameter("attn_s", [GPC, H, N, N], F32, isOutput=False).ap()
    p_x0 = nc.declare_dram_parameter("x0_s", [GPC, N, H, DH], F32, isOutput=False).ap()
    p_oin = nc.declare_dram_parameter("out_in_s", [N, GPC, D], F32, isOutput=False).ap()
    p_cmat = nc.declare_dram_parameter("cmat_s", [GPC, N, N], F32, isOutput=False).ap()
    p_wcheb = nc.declare_dram_parameter("wcheb", [DH, ORDER * DH], F32, isOutput=False).ap()
    p_wcat = nc.declare_dram_parameter("wcat", [2 * D, D], F32, isOutput=False).ap()
    p_bcat2 = nc.declare_dram_parameter("bcat2", [1, D], F32, isOutput=False).ap()
    p_gamma = nc.declare_dram_parameter("gamma", [1, D], F32, isOutput=False).ap()
    p_beta = nc.declare_dram_parameter("beta", [1, D], F32, isOutput=False).ap()
    p_wrow = nc.declare_dram_parameter("wrow", [1, ORDER], F32, isOutput=False).ap()
    p_bgcn = nc.declare_dram_parameter("bgcn", [1, ORDER], F32, isOutput=False).ap()
    p_mask = nc.declare_dram_parameter("mask8", [H, H * N], F32, isOutput=False).ap()
    p_wlin = nc.declare_dram_parameter("wlin4", [ORDER, ORDER], F32, isOutput=False).ap()
    p_kcorr = nc.declare_dram_parameter("kcorr", [ORDER, 1], F32, isOutput=False).ap()
    p_blin4 = nc.declare_dram_parameter("blin4", [ORDER, 1], F32, isOutput=False).ap()
    p_outy = nc.declare_dram_parameter("outy", [N, GPC, D], F32, isOutput=True).ap()

    def bcast(dram_ap, parts, inner, extra_off=0):
        return bass.AP(tensor=dram_ap.tensor, offset=dram_ap.offset + extra_off,
                       ap=[[0, parts]] + inner)

    with tile.TileContext(nc) as tc:
        import contextlib
        ctx = contextlib.ExitStack()
        with ctx:
            const = ctx.enter_context(tc.tile_pool(name="const", bufs=1))
            persist = ctx.enter_context(tc.tile_pool(name="persist", bufs=1))
            ps = ctx.enter_context(tc.tile_pool(name="ps", bufs=2, space="PSUM"))
            dram = ctx.enter_context(tc.tile_pool(name="dram", bufs=1, space="DRAM"))

            # ---------------- constants ----------------
            ones_col = const.tile([128, 1], F32)
            nc.vector.memset(ones_col, 1.0)
            ones_r = const.tile([128, 1], F32R)
            nc.vector.tensor_copy(ones_r, ones_col)
            ones2 = const.tile([128, 2], F32)
            nc.vector.memset(ones2, 1.0)
            ones2_r = const.tile([128, 2], F32R)
            nc.vector.tensor_copy(ones2_r, ones2)
            identity = const.tile([128, 128], F32)
            make_identity(nc, identity)
            wcheb_sb = const.tile([DH, ORDER * DH], F32)
            nc.sync.dma_start(out=wcheb_sb, in_=p_wcheb)
            wcat_sb = const.tile([128, 8 * D], F32R)
            bcat2_bc = const.tile([128, D], F32)
            nc.sync.dma_start(out=bcat2_bc, in_=bcast(p_bcat2, 128, [[1, D]]))
            gamma_bc = const.tile([128, D], F32)
            nc.sync.dma_start(out=gamma_bc, in_=bcast(p_gamma, 128, [[1, D]]))
            beta_bc = const.tile([128, D], F32)
            nc.sync.dma_start(out=beta_bc, in_=bcast(p_beta, 128, [[1, D]]))
            wrow_bc = const.tile([128, ORDER], F32)
            nc.sync.dma_start(out=wrow_bc, in_=bcast(p_wrow, 128, [[1, ORDER]]))
            bgcn_bc = const.tile([128, ORDER], F32)
            nc.sync.dma_start(out=bgcn_bc, in_=bcast(p_bgcn, 128, [[1, ORDER]]))
            mask_sb = const.tile([H, H * N], F32)
            nc.sync.dma_start(out=mask_sb, in_=p_mask)
            wlin_sb = const.tile([ORDER, ORDER], F32)
            nc.sync.dma_start(out=wlin_sb, in_=p_wlin)
            kcorr_sb = const.tile([ORDER, 1], F32)
            nc.sync.dma_start(out=kcorr_sb, in_=p_kcorr)
            blin4_sb = const.tile([ORDER, 1], F32)
            nc.sync.dma_start(out=blin4_sb, in_=p_blin4)
            eps_col = const.tile([128, 1], F32)
            nc.vector.memset(eps_col, 1e-5)

            # persistent big tiles
            acc_all = persist.tile([128, GPC * NCHUNK * D], F32)    # 4 MB
            outT = persist.tile([128, 4 * GPC * N], F32R)           # 4 MB
            dcrow2 = dram.tile([GPC, H * ORDER], F32)
            dcrow_exp = dram.tile([GPC, DH * H * ORDER], F32)
            ddinv = dram.tile([GPC, H * N], F32)

            # 'output' input, loaded node-major then PE-transposed to
            # feature-major: outT[f in fb-block, fb*2048 + g*256 + n].
            with tc.tile_pool(name="oin", bufs=3) as oinp:
                for fb in range(8):
                    wb = oinp.tile([128, D], F32, tag="wb")
                    nc.sync.dma_start(out=wb, in_=p_wcat[fb * 128:(fb + 1) * 128, :])
                    nc.scalar.copy(out=wcat_sb[:, fb * D:(fb + 1) * D], in_=wb)
                for nch in range(GPC * NCHUNK):
                    g, dc = divmod(nch, NCHUNK)
                    on = oinp.tile([128, D], F32, tag="on")
                    nc.sync.dma_start(out=on, in_=p_oin[dc * 128:(dc + 1) * 128, g, :])
                    pso = ps.tile([128, 2048], F32, tag="big")
                    for fb in range(4):
                        nc.tensor.transpose(pso[:, fb * 128:(fb + 1) * 128],
                                            on[:, fb * 128:(fb + 1) * 128], identity)
                    nc.scalar.copy(
                        out=outT.rearrange("p (f x) -> p f x", f=4)
                            [:, :, nch * 128:(nch + 1) * 128],
                        in_=pso[:, 0:512].rearrange("p (f n) -> p f n", f=4))

            # =====================================================
            # Phase A: stage-1 GCN coefficients (streaming form)
            # =====================================================
            with tc.tile_pool(name="stage1", bufs=3) as s1p, \
                 tc.tile_pool(name="stage1s", bufs=4) as s1s, \
                 tc.tile_pool(name="s1w", bufs=1) as s1w:
                for g in range(GPC):
                    Atr = []
                    for ic in range(2):
                        a = s1p.tile([128, H * N], F32, tag="attn")
                        nc.sync.dma_start(
                            out=a.rearrange("p (h j) -> p h j", h=H),
                            in_=p_attn[g, :, ic * 128:(ic + 1) * 128, :]
                                .rearrange("h i j -> i h j"))
                        ar = s1p.tile([128, H * N], F32R, tag="attnr")
                        nc.scalar.copy(out=ar, in_=a)
                        Atr.append(ar)
                    # r1: colsum rows [1, (h, j)]  (one psum tile per graph)
                    psA = ps.tile([128, 2048], F32, tag="big")
                    psT = psA
                    psD = psA
                    psC = psA
                    for w in range(4):
                        for ic in range(2):
                            nc.tensor.matmul(
                                out=psA[0:1, w * 512:(w + 1) * 512],
                                lhsT=ones_r, rhs=Atr[ic][:, w * 512:(w + 1) * 512],
                                start=(ic == 0), stop=(ic == 1))
                    # dinv row = 1/sqrt(colsum + 1)
                    sqrow = s1w.tile([1, H * N], F32, tag="sqrow")
                    nc.scalar.activation(out=sqrow, in_=psA[0:1, 0:2048],
                                         func=AF.Sqrt, bias=1.0)
                    dinvrow = s1w.tile([1, H * N], F32, tag="dinvrow")
                    nc.vector.reciprocal(out=dinvrow, in_=sqrow)
                    nc.sync.dma_start(out=ddinv[g:g + 1, :], in_=dinvrow)
                    # transpose dinv row-chunks -> columns [128, (ic, h)]
                    for ic in range(2):
                        for h in range(H):
                            nc.tensor.transpose(
                                psT[:, (ic * 8 + h) * 128:(ic * 8 + h) * 128 + 1],
                                dinvrow[0:1, h * N + ic * 128:h * N + (ic + 1) * 128],
                                identity[0:1, 0:1])
                    dinvT = s1s.tile([128, 16], F32R, tag="dinvT")
                    nc.scalar.copy(
                        out=dinvT,
                        in_=psT[:, 0:2048].rearrange("p (c x) -> p c x", c=16)[:, :, 0])
                    # r2: dot rows [8, (h', j)] via block-diag dinv lhsT
                    for w in range(4):
                        for ic in range(2):
                            nc.tensor.matmul(
                                out=psD[0:8, w * 512:(w + 1) * 512],
                                lhsT=dinvT[:, ic * 8:(ic + 1) * 8],
                                rhs=Atr[ic][:, w * 512:(w + 1) * 512],
                                start=(ic == 0), stop=(ic == 1))
                    # s = (dot + dinv) * dinv, diag-masked
                    dinvB = s1w.tile([H, H * N], F32, tag="dinvB")
                    nc.sync.dma_start(out=dinvB,
                                      in_=bcast(ddinv[0], H, [[1, H * N]],
                                                extra_off=g * H * N))
                    maskD = s1w.tile([H, H * N], F32, tag="maskD")
                    nc.vector.tensor_mul(maskD, dinvB, mask_sb)
                    t8 = s1w.tile([H, H * N], F32, tag="t8")
                    nc.vector.tensor_add(t8, psD[0:8, 0:2048], dinvB)
                    sm = s1w.tile([H, H * N], F32, tag="sm")
                    nc.vector.tensor_mul(sm, t8, maskD)
                    # tanh + GAP (accumulate); off-diag contributes tanh(b_c)
                    gacc = s1s.tile([H, ORDER], F32, tag="gacc")
                    for c in range(ORDER):
                        junk = s1w.tile([H, H * N], F32, tag="junk")
                        nc.scalar.activation(
                            out=junk, in_=sm, func=AF.Tanh,
                            scale=wrow_bc[0:H, c:c + 1], bias=bgcn_bc[0:H, c:c + 1],
                            accum_out=gacc[:, c:c + 1])
                    # coeff = (gacc - corr)/256 @ W_lin + b_lin
                    nc.tensor.transpose(psC[0:ORDER, 0:H], gacc, identity[0:H, 0:H])
                    gapT = s1s.tile([ORDER, H], F32, tag="gapT")
                    nc.vector.tensor_scalar_sub(gapT, psC[0:ORDER, 0:H], kcorr_sb)
                    nc.tensor.matmul(out=psC[0:ORDER, 512:512 + H], lhsT=wlin_sb,
                                     rhs=gapT, start=True, stop=True)
                    coefT = s1s.tile([ORDER, H], F32, tag="coefT")
                    nc.vector.tensor_scalar_add(coefT, psC[0:ORDER, 512:512 + H],
                                                blin4_sb)
                    nc.tensor.transpose(psC[0:H, 1024:1024 + ORDER], coefT,
                                        identity[0:ORDER, 0:ORDER])
                    crow_sb = s1s.tile([H, ORDER], F32, tag="crow_sb")
                    nc.scalar.copy(out=crow_sb, in_=psC[0:H, 1024:1024 + ORDER])
                    nc.sync.dma_start(
                        out=dcrow2[g:g + 1, :].rearrange("o (h k) -> o h k", k=ORDER),
                        in_=crow_sb)

            # expand coefficients: dcrow_exp[g, (dh', h, k)] = c[g, h, k]
            nc.sync.dma_start(
                out=dcrow_exp.rearrange("g (d hk) -> g d hk", d=DH),
                in_=bass.AP(tensor=dcrow2.tensor, offset=dcrow2.offset,
                            ap=[[H * ORDER, GPC], [0, DH], [1, H * ORDER]]))

            # =====================================================
            # Phase B: Chebyshev propagation per graph
            # =====================================================
            with tc.tile_pool(name="st2a", bufs=2) as st2a, \
                 tc.tile_pool(name="st2b", bufs=2) as st2b, \
                 tc.tile_pool(name="st2s", bufs=4) as st2s:
                for g in range(GPC):
                    # --- C_g (cast to f32r during DMA) and degree columns ---
                    cm = []
                    for sc in range(2):
                        tf = st2a.tile([128, N], F32, tag="cmf")
                        nc.sync.dma_start(out=tf,
                                          in_=p_cmat[g, sc * 128:(sc + 1) * 128, :])
                        t = st2a.tile([128, N], F32R, tag="cm")
                        nc.scalar.copy(out=t, in_=tf)
                        cm.append(t)
                    psd = ps.tile([128, 2048], F32, tag="big")
                    for dc in range(2):
                        for sc in range(2):
                            nc.tensor.matmul(
                                out=psd[:, dc * 2:dc * 2 + 2],
                                lhsT=cm[sc][:, dc * 128:(dc + 1) * 128],
                                rhs=ones2_r, start=(sc == 0), stop=(sc == 1))
                    # masked rsqrt: dinv = m / (sqrt(deg) + 1 - m), m = sign(deg)
                    # dvar layout [128, (kind, dc)]: kinds
                    # 0=dinv 1=dsq 2=dneg 3=d3 4=d2 5=dm4
                    dvar = st2s.tile([128, 12], F32, tag="dvar")
                    sg = st2s.tile([128, 2], F32, tag="sg")
                    psdv = psd[:, 0:4].rearrange('p (d two) -> p d two', two=2)[:, :, 0]
                    nc.scalar.activation(out=sg, in_=psdv, func=AF.Sign)
                    sq = st2s.tile([128, 2], F32, tag="sq")
                    nc.scalar.activation(out=sq, in_=psdv, func=AF.Sqrt)
                    den = st2s.tile([128, 2], F32, tag="den")
                    nc.vector.scalar_tensor_tensor(
                        out=den, in0=sq, scalar=1.0, in1=sg,
                        op0=ALU.add, op1=ALU.subtract)
                    rec = st2s.tile([128, 2], F32, tag="rec")
                    nc.vector.reciprocal(out=rec, in_=den)
                    dv = dvar[:, 0:2]
                    nc.vector.tensor_mul(dv, rec, sg)
                    nc.scalar.square(dvar[:, 2:4], dv)
                    nc.scalar.mul(dvar[:, 4:6], dv, -1.0)
                    nc.scalar.mul(dvar[:, 6:8], dv, 3.0)
                    nc.scalar.mul(dvar[:, 8:10], dv, 2.0)
                    nc.scalar.mul(dvar[:, 10:12], dv, -4.0)

                    # --- x0^T via PE transpose (cast to f32r on eviction) ---
                    x0t = st2b.tile([DH, H * N], F32R, tag="x0t")
                    for nc2 in range(2):
                        x0n = st2s.tile([128, H * DH], F32, tag="x0n")
                        nc.sync.dma_start(
                            out=x0n.rearrange("p (h d) -> p h d", h=H),
                            in_=p_x0[g, nc2 * 128:(nc2 + 1) * 128, :, :])
                        psx = ps.tile([128, 2048], F32, tag="big")
                        for h in range(H):
                            nc.tensor.transpose(psx[0:DH, h * 128:(h + 1) * 128],
                                                x0n[:, h * DH:(h + 1) * DH], identity)
                        nc.vector.tensor_copy(
                            x0t.rearrange("p (h x) -> p h x", h=H)
                               [:, :, nc2 * 128:(nc2 + 1) * 128],
                            psx[0:DH, 0:1024].rearrange("p (h n) -> p h n", h=H))

                    # --- c-scaled W_cheb:
                    # cexp[p, (dh', hk)] broadcast from dcrow_exp[g]
                    cexp = st2b.tile([DH, DH * H * ORDER], F32, tag="cexp")
                    nc.sync.dma_start(
                        out=cexp,
                        in_=bcast(dcrow_exp[0], DH, [[1, DH * H * ORDER]],
                                  extra_off=g * DH * H * ORDER))
                    wsc = st2b.tile([DH, H * ORDER * DH], F32R, tag="wsc")
                    cexpv = cexp.rearrange("p (d h2 k) -> p h2 k d", d=DH, k=ORDER)
                    for h in range(H):
                        nc.vector.tensor_mul(
                            wsc[:, h * 256:(h + 1) * 256]
                                .rearrange("p (k d) -> p k d", k=ORDER),
                            cexpv[:, h, :, :],
                            wcheb_sb.rearrange("p (k d) -> p k d", k=ORDER))

                    # --- Z matmuls:  Z[(n),(k,dh')] = x0 @ (c_k W_k) ---
                    accv = []
                    v0 = []
                    for dc in range(2):
                        pz = ps.tile([128, 2048], F32, tag="big")
                        for h in range(H):
                            nc.tensor.matmul(
                                out=pz[:, h * 256:(h + 1) * 256],
                                lhsT=x0t[:, h * N + dc * 128:h * N + (dc + 1) * 128],
                                rhs=wsc[:, h * 256:(h + 1) * 256],
                                start=True, stop=True)
                        pzv = pz.rearrange("p (h k d) -> p h k d", h=H, k=ORDER)
                        a = acc_all[:, (g * 2 + dc) * D:(g * 2 + dc + 1) * D] \
                            .rearrange("p (h d) -> p h d", h=H)
                        accv.append(a)
                        # acc = Z0 - Z2  (one PSUM operand per instruction)
                        nc.scalar.mul(a, pzv[:, :, 2, :], -1.0)
                        nc.vector.tensor_tensor(
                            out=a, in0=pzv[:, :, 0, :], in1=a, op=ALU.add)
                        # v0 = dinv * Z[1:4]
                        v = st2a.tile([128, H * 3 * DH], F32R, tag="v0")
                        nc.scalar.activation(
                            out=v.rearrange("p (h k d) -> p h k d", h=H, k=3),
                            in_=pzv[:, :, 1:4, :], func=AF.Copy,
                            scale=dvar[:, dc:dc + 1])
                        v0.append(v)

                    # --- propagation round 1: y1 = C^T v0 ---
                    v1 = []
                    for dc in range(2):
                        py = ps.tile([128, 2048], F32, tag="big")
                        for w in range(3):
                            for sc in range(2):
                                nc.tensor.matmul(
                                    out=py[:, w * 512:(w + 1) * 512],
                                    lhsT=cm[sc][:, dc * 128:(dc + 1) * 128],
                                    rhs=v0[sc][:, w * 512:(w + 1) * 512],
                                    start=(sc == 0), stop=(sc == 1))
                        pyv = py[:, 0:1536].rearrange("p (h k d) -> p h k d", h=H, k=3)
                        a = accv[dc]
                        # acc += -dinv * y1[k=1]
                        nc.vector.scalar_tensor_tensor(
                            out=a, in0=pyv[:, :, 0, :],
                            scalar=dvar[:, 4 + dc:5 + dc],
                            in1=a, op0=ALU.mult, op1=ALU.add)
                        # acc += 3 dinv * y1[k=3]
                        nc.vector.scalar_tensor_tensor(
                            out=a, in0=pyv[:, :, 2, :],
                            scalar=dvar[:, 6 + dc:7 + dc],
                            in1=a, op0=ALU.mult, op1=ALU.add)
                        # v1 = dinv^2 * y1[k=2,3]
                        v = st2a.tile([128, H * 2 * DH], F32R, tag="v1")
                        nc.scalar.activation(
                            out=v.rearrange("p (h k d) -> p h k d", h=H, k=2),
                            in_=pyv[:, :, 1:3, :], func=AF.Copy,
                            scale=dvar[:, 2 + dc:3 + dc])
                        v1.append(v)

                    # --- round 2 ---
                    v2 = []
                    for dc in range(2):
                        py = ps.tile([128, 2048], F32, tag="big")
                        for w in range(2):
                            for sc in range(2):
                                nc.tensor.matmul(
                                    out=py[:, w * 512:(w + 1) * 512],
                                    lhsT=cm[sc][:, dc * 128:(dc + 1) * 128],
                                    rhs=v1[sc][:, w * 512:(w + 1) * 512],
                                    start=(sc == 0), stop=(sc == 1))
                        pyv = py[:, 0:1024].rearrange("p (h k d) -> p h k d", h=H, k=2)
                        a = accv[dc]
                        # acc += 2 dinv * y2[k=2]
                        nc.vector.scalar_tensor_tensor(
                            out=a, in0=pyv[:, :, 0, :],
                            scalar=dvar[:, 8 + dc:9 + dc],
                            in1=a, op0=ALU.mult, op1=ALU.add)
                        v = st2a.tile([128, H * DH], F32R, tag="v2")
                        nc.scalar.activation(
                            out=v.rearrange("p (h d) -> p h d", h=H),
                            in_=pyv[:, :, 1, :], func=AF.Copy,
                            scale=dvar[:, 2 + dc:3 + dc])
                        v2.append(v)

                    # --- round 3 ---
                    for dc in range(2):
                        py = ps.tile([128, 2048], F32, tag="big")
                        for sc in range(2):
                            nc.tensor.matmul(
                                out=py[:, 0:512],
                                lhsT=cm[sc][:, dc * 128:(dc + 1) * 128],
                                rhs=v2[sc], start=(sc == 0), stop=(sc == 1))
                        a = accv[dc]
                        # acc += -4 dinv * y3
                        nc.vector.scalar_tensor_tensor(
                            out=a,
                            in0=py[:, 0:512].rearrange("p (h d) -> p h d", h=H),
                            scalar=dvar[:, 10 + dc:11 + dc],
                            in1=a, op0=ALU.mult, op1=ALU.add)

            # =====================================================
            # Phase C: transpose acc, cat-matmul, LayerNorm, store
            # =====================================================
            with tc.tile_pool(name="ph_c", bufs=1) as phc, \
                 tc.tile_pool(name="ln", bufs=3) as lnp, \
                 tc.tile_pool(name="lns", bufs=4) as lns:
                accT = phc.tile([128, 4 * GPC * N], F32R)
                for nch in range(GPC * NCHUNK):
                    pst = ps.tile([128, 2048], F32, tag="big")
                    for fb in range(4):
                        nc.tensor.transpose(
                            pst[:, fb * 128:(fb + 1) * 128],
                            acc_all[:, nch * D + fb * 128:nch * D + (fb + 1) * 128],
                            identity)
                    nc.scalar.copy(
                        out=accT.rearrange("p (f x) -> p f x", f=4)
                            [:, :, nch * 128:(nch + 1) * 128],
                        in_=pst[:, 0:512].rearrange("p (f n) -> p f n", f=4))

                for nch in range(GPC * NCHUNK):
                    g, dc = divmod(nch, NCHUNK)
                    po = ps.tile([128, 2048], F32, tag="big")
                    for fb in range(4):
                        nc.tensor.matmul(
                            out=po[:, 0:512],
                            lhsT=outT[:, fb * (GPC * N) + nch * 128:
                                      fb * (GPC * N) + (nch + 1) * 128],
                            rhs=wcat_sb[:, fb * D:(fb + 1) * D],
                            start=(fb == 0), stop=False)
                    for fb in range(4):
                        nc.tensor.matmul(
                            out=po[:, 0:512],
                            lhsT=accT[:, fb * (GPC * N) + nch * 128:
                                      fb * (GPC * N) + (nch + 1) * 128],
                            rhs=wcat_sb[:, (4 + fb) * D:(5 + fb) * D],
                            start=False, stop=(fb == 3))

                    # LayerNorm
                    t0 = lnp.tile([128, D], F32, tag="t0")
                    musum = lns.tile([128, 1], F32, tag="musum")
                    nc.vector.scalar_tensor_tensor(
                        out=t0, in0=po[:, 0:512], scalar=1.0, in1=bcat2_bc,
                        op0=ALU.mult, op1=ALU.add, accum_out=musum)
                    negmu = lns.tile([128, 1], F32, tag="negmu")
                    nc.scalar.mul(negmu, musum, -1.0 / D)
                    sqs = lns.tile([128, 1], F32, tag="sqs")
                    sq = lnp.tile([128, D], F32, tag="sq")
                    nc.scalar.activation(out=sq, in_=t0, func=AF.Square,
                                         bias=negmu, accum_out=sqs)
                    sd = lns.tile([128, 1], F32, tag="sd")
                    nc.scalar.activation(out=sd, in_=sqs, func=AF.Sqrt,
                                         scale=1.0 / D, bias=eps_col)
                    rstd = lns.tile([128, 1], F32, tag="rstd")
                    nc.vector.reciprocal(out=rstd, in_=sd)
                    nmr = lns.tile([128, 1], F32, tag="nmr")
                    nc.vector.tensor_mul(nmr, negmu, rstd)
                    t1 = lnp.tile([128, D], F32, tag="t1")
                    nc.scalar.activation(out=t1, in_=t0, func=AF.Identity,
                                         scale=rstd, bias=nmr)
                    t2 = lnp.tile([128, D], F32, tag="t2")
                    nc.vector.tensor_mul(t2, t1, gamma_bc)
                    nc.vector.tensor_add(t2, t2, beta_bc)
                    nc.sync.dma_start(out=p_outy[dc * 128:(dc + 1) * 128, g, :], in_=t2)

    if not nc.is_finalized():
        nc.finalize()
    return nc


# ---------------------------------------------------------------------------
# host side
# ---------------------------------------------------------------------------

def _canonical_indices(feature_indices, batch):
    fi = np.asarray(feature_indices)
    bt = np.asarray(batch)
    want0 = np.repeat(np.arange(B), N)
    want1 = np.tile(np.arange(N), B)
    return (fi.shape == (B * N, 2) and bt.shape == (B * N,)
            and np.array_equal(fi[:, 0], want0) and np.array_equal(fi[:, 1], want1)
            and np.array_equal(bt, want0))


def _prep(inputs):
    """Host-side sharding + index preprocessing. Returns per-core input maps."""
    attn = np.ascontiguousarray(np.asarray(inputs["attn"], np.float32))
    oeh = np.ascontiguousarray(np.asarray(inputs["out_each_head"], np.float32))
    outp = np.ascontiguousarray(np.asarray(inputs["output"], np.float32))
    ei = np.asarray(inputs["edge_index"])

    W_gcn = np.asarray(inputs["W_gcn"], np.float32)
    b_gcn = np.asarray(inputs["b_gcn"], np.float32)
    W_lin = np.asarray(inputs["W_lin"], np.float32)
    b_lin = np.asarray(inputs["b_lin"], np.float32)
    W_cheb = np.asarray(inputs["W_cheb"], np.float32)
    b_cheb = np.asarray(inputs["b_cheb"], np.float32)
    W_cat = np.asarray(inputs["W_cat"], np.float32)
    b_cat = np.asarray(inputs["b_cat"], np.float32)
    gamma = np.asarray(inputs["gamma"], np.float32)
    beta = np.asarray(inputs["beta"], np.float32)

    # dense per-graph edge-count matrices (pure integer indexing)
    s_g = (ei[0] // N).astype(np.int64)
    s_l = (ei[0] % N).astype(np.int64)
    d_l = (ei[1] % N).astype(np.int64)
    flat = np.zeros(B * N * N, np.float32)
    np.add.at(flat, s_g * (N * N) + s_l * N + d_l, 1.0)
    cmat = flat.reshape(B, N, N)

    wcheb = np.ascontiguousarray(
        W_cheb.transpose(1, 0, 2).reshape(DH, ORDER * DH))   # [dh, (k, dh')]
    wrow = W_gcn.sum(axis=0).reshape(1, ORDER).astype(np.float32)
    bcat2 = (b_cat + np.tile(b_cheb, H) @ W_cat[D:, :]).reshape(1, D).astype(np.float32)

    # stage-1 streaming-form constants
    mask8 = np.zeros((H, H * N), np.float32)
    for h in range(H):
        mask8[h, h * N:(h + 1) * N] = 1.0
    kcorr = ((H * N - N) * np.tanh(b_gcn)).reshape(ORDER, 1).astype(np.float32)
    wlin4 = (W_lin / N).astype(np.float32)
    blin4 = b_lin.reshape(ORDER, 1).astype(np.float32)

    shared = dict(
        wcheb=wcheb, wcat=np.ascontiguousarray(W_cat),
        bcat2=bcat2, gamma=gamma.reshape(1, D).astype(np.float32),
        beta=beta.reshape(1, D).astype(np.float32),
        wrow=wrow, bgcn=b_gcn.reshape(1, ORDER).astype(np.float32),
        mask8=mask8, wlin4=wlin4, kcorr=kcorr, blin4=blin4)

    in_maps = []
    for c in range(NCORES):
        G = slice(c * GPC, (c + 1) * GPC)
        in_maps.append(dict(
            attn_s=np.ascontiguousarray(attn[G]),
            x0_s=np.ascontiguousarray(oeh[G]),
            out_in_s=np.ascontiguousarray(outp[:, G, :]),
            cmat_s=np.ascontiguousarray(cmat[G]),
            **shared))
    return in_maps


def _fallback_numpy(inputs):
    """Generic (slow) numpy path, used only if the index tensors deviate from
    the canonical layout produced by setup_inputs()."""
    output = np.asarray(inputs["output"], np.float32)
    attn = np.asarray(inputs["attn"], np.float32)
    oeh = np.asarray(inputs["out_each_head"], np.float32)
    ei = np.asarray(inputs["edge_index"])
    fi = np.asarray(inputs["feature_indices"])
    batch = np.asarray(inputs["batch"])
    W_gcn = np.asarray(inputs["W_gcn"], np.float32); b_gcn = np.asarray(inputs["b_gcn"], np.float32)
    W_lin = np.asarray(inputs["W_lin"], np.float32); b_lin = np.asarray(inputs["b_lin"], np.float32)
    W_cheb = np.asarray(inputs["W_cheb"], np.float32); b_cheb = np.asarray(inputs["b_cheb"], np.float32)
    W_cat = np.asarray(inputs["W_cat"], np.float32); b_cat = np.asarray(inputs["b_cat"], np.float32)
    gamma = np.asarray(inputs["gamma"], np.float32); beta = np.asarray(inputs["beta"], np.float32)

    Bn, Hn, Nn, _ = attn.shape
    C = W_gcn.shape[0]
    total = Bn * Nn
    NT = Hn * total
    A = attn.transpose(1, 0, 2, 3).reshape(Hn * Bn, Nn, Nn)
    A_hat = A + np.eye(Nn, dtype=A.dtype)
    deg = A_hat.sum(axis=1)
    dinv = np.where(deg > 0, 1.0 / np.sqrt(deg), 0.0).astype(np.float32)
    w_row = W_gcn.sum(axis=0)
    s = np.einsum('bi,bij->bj', dinv, A_hat) * dinv
    x_c = np.tanh(s[:, :, None] * w_row + b_gcn)
    gap = x_c.mean(axis=1)
    coeff = gap @ W_lin + b_lin

    offsets = (np.arange(Hn) * total).astype(ei.dtype)
    src = (ei[0][None, :] + offsets[:, None]).reshape(-1)
    dst = (ei[1][None, :] + offsets[:, None]).reshape(-1)
    deg_n = np.zeros(NT, np.float32)
    np.add.at(deg_n, dst, 1.0)
    dinv_n = np.where(deg_n > 0, 1.0 / np.sqrt(np.maximum(deg_n, 1e-30)), 0.0).astype(np.float32)
    norm_e = -(dinv_n[src] * dinv_n[dst])

    def prop(x):
        out = np.zeros((NT, x.shape[1]), np.float32)
        np.add.at(out, dst, norm_e[:, None] * x[src])
        return out

    x0 = oeh.transpose(2, 0, 1, 3).reshape(NT, DH)
    batch_all = (batch[None, :] + (np.arange(Hn) * Bn)[:, None]).reshape(-1)
    c_node = coeff[batch_all]
    T_prev, T_cur = x0, prop(x0)
    acc = c_node[:, 0:1] * (T_prev @ W_cheb[0]) + c_node[:, 1:2] * (T_cur @ W_cheb[1])
    for k in range(2, C):
        T_next = 2.0 * prop(T_cur) - T_prev
        acc = acc + c_node[:, k:k + 1] * (T_next @ W_cheb[k])
        T_prev, T_cur = T_cur, T_next
    acc = acc + b_cheb
    filtered = acc.reshape(Hn, total, DH).transpose(1, 0, 2).reshape(total, Hn * DH)
    out_filtered = np.zeros_like(output)
    out_filtered[fi[:, 1], fi[:, 0], :] = filtered
    out_cat = np.concatenate([output, out_filtered], axis=-1)
    out = out_cat @ W_cat + b_cat
    mu = out.mean(axis=-1, keepdims=True)
    var = ((out - mu) ** 2).mean(axis=-1, keepdims=True)
    return ((out - mu) / np.sqrt(var + 1e-5) * gamma + beta).astype(np.float32)


def _get_nc():
    if "nc" not in _CACHE:
        _CACHE["nc"] = _build_module(use_f32r=_CACHE.get("use_f32r", True))
    return _CACHE["nc"]


def kernel(**inputs) -> np.ndarray:
    if not _canonical_indices(inputs["feature_indices"], inputs["batch"]):
        return _fallback_numpy(inputs)

    from concourse.bass_utils import run_bass_kernel_spmd

    nc = _get_nc()
    in_maps = _prep(inputs)
    res = run_bass_kernel_spmd(nc, in_maps, list(range(NCORES)))
    out = np.empty((N, B, D), np.float32)
    for c in range(NCORES):
        out[:, c * GPC:(c + 1) * GPC, :] = res.results[c]["outy"]
    return out


# revision 34
# speedup vs baseline: 2.0309x; 1.0100x over previous
"""Trainium2 Bass kernel for nn_DiffGraphTransformerGenGCN.

Strategy (see spec sharding hint): data-parallel over the 64 graphs, 8 graphs
per NeuronCore.  Per core everything is reformulated as dense per-graph
matmuls:

  stage 1 (GCN filter coefficients over the attention graphs):
    streaming colsum / dinv-weighted colsum of each 256x256 attention matrix
    (ones / block-diagonal dinv as the stationary operand, attention tiles
    streamed once in float32r), masked tanh with accumulate for the global
    average pool, tiny matmuls for the final linear -> 4 coefficients per
    (graph, head).

  stage 2 (dynamic Chebyshev filtering on the input graph):
    The edge list is converted (host-side, pure integer indexing) into a
    dense 256x256 edge-count matrix C_g per graph.  The scaled Laplacian
    action is  P x = -D C^T D x  with D = diag(rsqrt(indeg)).  The Chebyshev
    coefficients c_(g,h,k) (stage-1 output) are folded into per-(g,h)
    scaled copies of W_cheb, so  acc = sum_k c_k T_k(P) (x0 W_k)  is computed
    with 6 dense propagation matmuls per graph and no transposes.

  final: out = LayerNorm(concat(output, filtered) @ W_cat + b_cat') computed
    as a feat-major matmul (lhsT = PE-transposed operands).

All large matmuls run in float32r (1 cycle/row vs 4 for fp32); operands are
rounded to f32r at their producers (engine copy-casts / SWDGE cast-DMA).

kernel() takes FULL inputs and returns the FULL output; it shards across the
8 cores internally.
"""

import numpy as np

B, N, H, DH, ORDER = 64, 256, 8, 64, 4
D = H * DH
NCORES = 8
GPC = B // NCORES          # graphs per core
NCHUNK = N // 128          # node chunks per graph (2)

_CACHE = {}


# ---------------------------------------------------------------------------
# device kernel construction
# ---------------------------------------------------------------------------

def _build_module(use_f32r=True):
    import concourse.bass as bass
    import concourse.bacc as bacc
    import concourse.tile as tile
    from concourse import mybir
    from concourse.masks import make_identity

    F32 = mybir.dt.float32
    F32R = mybir.dt.float32r if use_f32r else mybir.dt.float32
    AF = mybir.ActivationFunctionType
    ALU = mybir.AluOpType

    nc = bacc.Bacc(None, target_bir_lowering=False, debug=False)

    # ---- DRAM parameters (per-core shard) ----
    p_attn = nc.declare_dram_parameter("attn_s", [GPC, H, N, N], F32, isOutput=False).ap()
    p_x0 = nc.declare_dram_parameter("x0_s", [GPC, N, H, DH], F32, isOutput=False).ap()
    p_oin = nc.declare_dram_parameter("out_in_s", [N, GPC, D], F32, isOutput=False).ap()
    p_cmat = nc.declare_dram_parameter("cmat_s", [GPC, N, N], F32, isOutput=False).ap()
    p_wcheb = nc.declare_dram_parameter("wcheb", [DH, ORDER * DH], F32, isOutput=False).ap()
    p_wcat = nc.declare_dram_parameter("wcat", [2 * D, D], F32, isOutput=False).ap()
    p_bcat2 = nc.declare_dram_parameter("bcat2", [1, D], F32, isOutput=False).ap()
    p_gamma = nc.declare_dram_parameter("gamma", [1, D], F32, isOutput=False).ap()
    p_beta = nc.declare_dram_parameter("beta", [1, D], F32, isOutput=False).ap()
    p_wrow = nc.declare_dram_parameter("wrow", [1, ORDER], F32, isOutput=False).ap()
    p_bgcn = nc.declare_dram_parameter("bgcn", [1, ORDER], F32, isOutput=False).ap()
    p_mask = nc.declare_dram_parameter("mask8", [H, H * N], F32, isOutput=False).ap()
    p_wlin = nc.declare_dram_parameter("wlin4", [ORDER, ORDER], F32, isOutput=False).ap()
    p_kcorr = nc.declare_dram_parameter("kcorr", [ORDER, 1], F32, isOutput=False).ap()
    p_blin4 = nc.declare_dram_parameter("blin4", [ORDER, 1], F32, isOutput=False).ap()
    p_outy = nc.declare_dram_parameter("outy", [N, GPC, D], F32, isOutput=True).ap()

    def bcast(dram_ap, parts, inner, extra_off=0):
        return bass.AP(tensor=dram_ap.tensor, offset=dram_ap.offset + extra_off,
                       ap=[[0, parts]] + inner)

    with tile.TileContext(nc) as tc:
        import contextlib
        ctx = contextlib.ExitStack()
        with ctx:
            const = ctx.enter_context(tc.tile_pool(name="const", bufs=1))
            persist = ctx.enter_context(tc.tile_pool(name="persist", bufs=1))
            ps = ctx.enter_context(tc.tile_pool(name="ps", bufs=2, space="PSUM"))
            dram = ctx.enter_context(tc.tile_pool(name="dram", bufs=1, space="DRAM"))

            # ---------------- constants ----------------
            ones_col = const.tile([128, 1], F32)
            nc.vector.memset(ones_col, 1.0)
            ones_r = const.tile([128, 1], F32R)
            nc.vector.tensor_copy(ones_r, ones_col)
            ones2 = const.tile([128, 2], F32)
            nc.vector.memset(ones2, 1.0)
            ones2_r = const.tile([128, 2], F32R)
            nc.vector.tensor_copy(ones2_r, ones2)
            identity = const.tile([128, 128], F32)
            make_identity(nc, identity)
            wcheb_sb = const.tile([DH, ORDER * DH], F32)
            nc.sync.dma_start(out=wcheb_sb, in_=p_wcheb)
            wcat_sb = const.tile([128, 8 * D], F32R)
            bcat2_bc = const.tile([128, D], F32)
            nc.sync.dma_start(out=bcat2_bc, in_=bcast(p_bcat2, 128, [[1, D]]))
            gamma_bc = const.tile([128, D], F32)
            nc.sync.dma_start(out=gamma_bc, in_=bcast(p_gamma, 128, [[1, D]]))
            beta_bc = const.tile([128, D], F32)
            nc.sync.dma_start(out=beta_bc, in_=bcast(p_beta, 128, [[1, D]]))
            wrow_bc = const.tile([128, ORDER], F32)
            nc.sync.dma_start(out=wrow_bc, in_=bcast(p_wrow, 128, [[1, ORDER]]))
            bgcn_bc = const.tile([128, ORDER], F32)
            nc.sync.dma_start(out=bgcn_bc, in_=bcast(p_bgcn, 128, [[1, ORDER]]))
            mask_sb = const.tile([H, H * N], F32)
            nc.sync.dma_start(out=mask_sb, in_=p_mask)
            wlin_sb = const.tile([ORDER, ORDER], F32)
            nc.sync.dma_start(out=wlin_sb, in_=p_wlin)
            kcorr_sb = const.tile([ORDER, 1], F32)
            nc.sync.dma_start(out=kcorr_sb, in_=p_kcorr)
            blin4_sb = const.tile([ORDER, 1], F32)
            nc.sync.dma_start(out=blin4_sb, in_=p_blin4)
            eps_col = const.tile([128, 1], F32)
            nc.vector.memset(eps_col, 1e-5)

            # persistent big tiles
            acc_all = persist.tile([128, GPC * NCHUNK * D], F32)    # 4 MB
            outT = persist.tile([128, 4 * GPC * N], F32R)           # 4 MB
            dcrow2 = dram.tile([GPC, H * ORDER], F32)
            dcrow_exp = dram.tile([GPC, DH * H * ORDER], F32)
            ddinv = dram.tile([GPC, H * N], F32)

            # 'output' input, loaded node-major then PE-transposed to
            # feature-major: outT[f in fb-block, fb*2048 + g*256 + n].
            with tc.tile_pool(name="oin", bufs=3) as oinp:
                for fb in range(8):
                    wb = oinp.tile([128, D], F32, tag="wb")
                    nc.sync.dma_start(out=wb, in_=p_wcat[fb * 128:(fb + 1) * 128, :])
                    nc.scalar.copy(out=wcat_sb[:, fb * D:(fb + 1) * D], in_=wb)
                for nch in range(GPC * NCHUNK):
                    g, dc = divmod(nch, NCHUNK)
                    on = oinp.tile([128, D], F32, tag="on")
                    nc.sync.dma_start(out=on, in_=p_oin[dc * 128:(dc + 1) * 128, g, :])
                    pso = ps.tile([128, 2048], F32, tag="big")
                    for fb in range(4):
                        nc.tensor.transpose(pso[:, fb * 128:(fb + 1) * 128],
                                            on[:, fb * 128:(fb + 1) * 128], identity)
                    nc.scalar.copy(
                        out=outT.rearrange("p (f x) -> p f x", f=4)
                            [:, :, nch * 128:(nch + 1) * 128],
                        in_=pso[:, 0:512].rearrange("p (f n) -> p f n", f=4))

            # =====================================================
            # Phase A: stage-1 GCN coefficients (streaming form)
            # =====================================================
            with tc.tile_pool(name="stage1", bufs=3) as s1p, \
                 tc.tile_pool(name="stage1s", bufs=4) as s1s, \
                 tc.tile_pool(name="s1w", bufs=1) as s1w:
                for g in range(GPC):
                    Atr = []
                    for ic in range(2):
                        a = s1p.tile([128, H * N], F32, tag="attn")
                        (nc.sync if ic == 0 else nc.scalar).dma_start(
                            out=a.rearrange("p (h j) -> p h j", h=H),
                            in_=p_attn[g, :, ic * 128:(ic + 1) * 128, :]
                                .rearrange("h i j -> i h j"))
                        ar = s1p.tile([128, H * N], F32R, tag="attnr")
                        nc.scalar.copy(out=ar, in_=a)
                        Atr.append(ar)
                    # r1: colsum rows [1, (h, j)]  (one psum tile per graph)
                    psA = ps.tile([128, 2048], F32, tag="big")
                    psT = psA
                    psD = psA
                    psC = psA
                    for w in range(4):
                        for ic in range(2):
                            nc.tensor.matmul(
                                out=psA[0:1, w * 512:(w + 1) * 512],
                                lhsT=ones_r, rhs=Atr[ic][:, w * 512:(w + 1) * 512],
                                start=(ic == 0), stop=(ic == 1))
                    # dinv row = 1/sqrt(colsum + 1)
                    sqrow = s1w.tile([1, H * N], F32, tag="sqrow")
                    nc.scalar.activation(out=sqrow, in_=psA[0:1, 0:2048],
                                         func=AF.Sqrt, bias=1.0)
                    dinvrow = s1w.tile([1, H * N], F32, tag="dinvrow")
                    rscr = s1w.tile([1, H * N], F32, tag="junk")
                    nc.vector.reciprocal_approx_accurate(out=dinvrow, in_=sqrow,
                                                         scratch=rscr)
                    nc.sync.dma_start(out=ddinv[g:g + 1, :], in_=dinvrow)
                    # transpose dinv row-chunks -> columns [128, (ic, h)]
                    for ic in range(2):
                        for h in range(H):
                            nc.tensor.transpose(
                                psT[:, (ic * 8 + h) * 128:(ic * 8 + h) * 128 + 1],
                                dinvrow[0:1, h * N + ic * 128:h * N + (ic + 1) * 128],
                                identity[0:1, 0:1])
                    dinvT = s1s.tile([128, 16], F32R, tag="dinvT")
                    nc.scalar.copy(
                        out=dinvT,
                        in_=psT[:, 0:2048].rearrange("p (c x) -> p c x", c=16)[:, :, 0])
                    # r2: dot rows [8, (h', j)] via block-diag dinv lhsT
                    for w in range(4):
                        for ic in range(2):
                            nc.tensor.matmul(
                                out=psD[0:8, w * 512:(w + 1) * 512],
                                lhsT=dinvT[:, ic * 8:(ic + 1) * 8],
                                rhs=Atr[ic][:, w * 512:(w + 1) * 512],
                                start=(ic == 0), stop=(ic == 1))
                    # s = (dot + dinv) * dinv, diag-masked
                    dinvB = s1w.tile([H, H * N], F32, tag="dinvB")
                    nc.sync.dma_start(out=dinvB,
                                      in_=bcast(ddinv[0], H, [[1, H * N]],
                                                extra_off=g * H * N))
                    maskD = s1w.tile([H, H * N], F32, tag="maskD")
                    nc.vector.tensor_mul(maskD, dinvB, mask_sb)
                    t8 = s1w.tile([H, H * N], F32, tag="t8")
                    nc.vector.tensor_add(t8, psD[0:8, 0:2048], dinvB)
                    sm = s1w.tile([H, H * N], F32, tag="sm")
                    nc.vector.tensor_mul(sm, t8, maskD)
                    # tanh + GAP (accumulate); off-diag contributes tanh(b_c)
                    gacc = s1s.tile([H, ORDER], F32, tag="gacc")
                    for c in range(ORDER):
                        junk = s1w.tile([H, H * N], F32, tag="junk")
                        nc.scalar.activation(
                            out=junk, in_=sm, func=AF.Tanh,
                            scale=wrow_bc[0:H, c:c + 1], bias=bgcn_bc[0:H, c:c + 1],
                            accum_out=gacc[:, c:c + 1])
                    # coeff = (gacc - corr)/256 @ W_lin + b_lin
                    nc.tensor.transpose(psC[0:ORDER, 0:H], gacc, identity[0:H, 0:H])
                    gapT = s1s.tile([ORDER, H], F32, tag="gapT")
                    nc.vector.tensor_scalar_sub(gapT, psC[0:ORDER, 0:H], kcorr_sb)
                    nc.tensor.matmul(out=psC[0:ORDER, 512:512 + H], lhsT=wlin_sb,
                                     rhs=gapT, start=True, stop=True)
                    coefT = s1s.tile([ORDER, H], F32, tag="coefT")
                    nc.vector.tensor_scalar_add(coefT, psC[0:ORDER, 512:512 + H],
                                                blin4_sb)
                    nc.tensor.transpose(psC[0:H, 1024:1024 + ORDER], coefT,
                                        identity[0:ORDER, 0:ORDER])
                    crow_sb = s1s.tile([H, ORDER], F32, tag="crow_sb")
                    nc.scalar.copy(out=crow_sb, in_=psC[0:H, 1024:1024 + ORDER])
                    nc.sync.dma_start(
                        out=dcrow2[g:g + 1, :].rearrange("o (h k) -> o h k", k=ORDER),
                        in_=crow_sb)

            # expand coefficients: dcrow_exp[g, (dh', h, k)] = c[g, h, k]
            nc.sync.dma_start(
                out=dcrow_exp.rearrange("g (d hk) -> g d hk", d=DH),
                in_=bass.AP(tensor=dcrow2.tensor, offset=dcrow2.offset,
                            ap=[[H * ORDER, GPC], [0, DH], [1, H * ORDER]]))

            # =====================================================
            # Phase B: Chebyshev propagation per graph
            # =====================================================
            with tc.tile_pool(name="st2a", bufs=2) as st2a, \
                 tc.tile_pool(name="st2b", bufs=2) as st2b, \
                 tc.tile_pool(name="st2s", bufs=4) as st2s:
                for g in range(GPC):
                    # --- C_g (cast to f32r during DMA) and degree columns ---
                    cm = []
                    for sc in range(2):
                        tf = st2a.tile([128, N], F32, tag="cmf")
                        nc.sync.dma_start(out=tf,
                                          in_=p_cmat[g, sc * 128:(sc + 1) * 128, :])
                        t = st2a.tile([128, N], F32R, tag="cm")
                        nc.scalar.copy(out=t, in_=tf)
                        cm.append(t)
                    psd = ps.tile([128, 2048], F32, tag="big")
                    for dc in range(2):
                        for sc in range(2):
                            nc.tensor.matmul(
                                out=psd[:, dc * 2:dc * 2 + 2],
                                lhsT=cm[sc][:, dc * 128:(dc + 1) * 128],
                                rhs=ones2_r, start=(sc == 0), stop=(sc == 1))
                    # masked rsqrt: dinv = m / (sqrt(deg) + 1 - m), m = sign(deg)
                    # dvar layout [128, (kind, dc)]: kinds
                    # 0=dinv 1=dsq 2=dneg 3=d3 4=d2 5=dm4
                    dvar = st2s.tile([128, 12], F32, tag="dvar")
                    sg = st2s.tile([128, 2], F32, tag="sg")
                    psdv = psd[:, 0:4].rearrange('p (d two) -> p d two', two=2)[:, :, 0]
                    nc.scalar.activation(out=sg, in_=psdv, func=AF.Sign)
                    sq = st2s.tile([128, 2], F32, tag="sq")
                    nc.scalar.activation(out=sq, in_=psdv, func=AF.Sqrt)
                    den = st2s.tile([128, 2], F32, tag="den")
                    nc.vector.scalar_tensor_tensor(
                        out=den, in0=sq, scalar=1.0, in1=sg,
                        op0=ALU.add, op1=ALU.subtract)
                    rec = st2s.tile([128, 2], F32, tag="rec")
                    nc.vector.reciprocal(out=rec, in_=den)
                    dv = dvar[:, 0:2]
                    nc.vector.tensor_mul(dv, rec, sg)
                    nc.scalar.square(dvar[:, 2:4], dv)
                    nc.scalar.mul(dvar[:, 4:6], dv, -1.0)
                    nc.scalar.mul(dvar[:, 6:8], dv, 3.0)
                    nc.scalar.mul(dvar[:, 8:10], dv, 2.0)
                    nc.scalar.mul(dvar[:, 10:12], dv, -4.0)

                    # --- x0^T via PE transpose (cast to f32r on eviction) ---
                    x0t = st2b.tile([DH, H * N], F32R, tag="x0t")
                    for nc2 in range(2):
                        x0n = st2s.tile([128, H * DH], F32, tag="x0n")
                        nc.scalar.dma_start(
                            out=x0n.rearrange("p (h d) -> p h d", h=H),
                            in_=p_x0[g, nc2 * 128:(nc2 + 1) * 128, :, :])
                        psx = ps.tile([128, 2048], F32, tag="big")
                        for h in range(H):
                            nc.tensor.transpose(psx[0:DH, h * 128:(h + 1) * 128],
                                                x0n[:, h * DH:(h + 1) * DH], identity)
                        nc.vector.tensor_copy(
                            x0t.rearrange("p (h x) -> p h x", h=H)
                               [:, :, nc2 * 128:(nc2 + 1) * 128],
                            psx[0:DH, 0:1024].rearrange("p (h n) -> p h n", h=H))

                    # --- c-scaled W_cheb:
                    # cexp[p, (dh', hk)] broadcast from dcrow_exp[g]
                    cexp = st2b.tile([DH, DH * H * ORDER], F32, tag="cexp")
                    nc.sync.dma_start(
                        out=cexp,
                        in_=bcast(dcrow_exp[0], DH, [[1, DH * H * ORDER]],
                                  extra_off=g * DH * H * ORDER))
                    wsc = st2b.tile([DH, H * ORDER * DH], F32R, tag="wsc")
                    cexpv = cexp.rearrange("p (d h2 k) -> p h2 k d", d=DH, k=ORDER)
                    for h in range(H):
                        nc.vector.tensor_mul(
                            wsc[:, h * 256:(h + 1) * 256]
                                .rearrange("p (k d) -> p k d", k=ORDER),
                            cexpv[:, h, :, :],
                            wcheb_sb.rearrange("p (k d) -> p k d", k=ORDER))

                    # --- Z matmuls:  Z[(n),(k,dh')] = x0 @ (c_k W_k) ---
                    accv = []
                    v0 = []
                    for dc in range(2):
                        pz = ps.tile([128, 2048], F32, tag="big")
                        for h in range(H):
                            nc.tensor.matmul(
                                out=pz[:, h * 256:(h + 1) * 256],
                                lhsT=x0t[:, h * N + dc * 128:h * N + (dc + 1) * 128],
                                rhs=wsc[:, h * 256:(h + 1) * 256],
                                start=True, stop=True)
                        pzv = pz.rearrange("p (h k d) -> p h k d", h=H, k=ORDER)
                        a = acc_all[:, (g * 2 + dc) * D:(g * 2 + dc + 1) * D] \
                            .rearrange("p (h d) -> p h d", h=H)
                        accv.append(a)
                        # acc = Z0 - Z2  (one PSUM operand per instruction)
                        nc.scalar.mul(a, pzv[:, :, 2, :], -1.0)
                        nc.vector.tensor_tensor(
                            out=a, in0=pzv[:, :, 0, :], in1=a, op=ALU.add)
                        # v0 = dinv * Z[1:4]
                        v = st2a.tile([128, H * 3 * DH], F32R, tag="v0")
                        nc.scalar.activation(
                            out=v.rearrange("p (h k d) -> p h k d", h=H, k=3),
                            in_=pzv[:, :, 1:4, :], func=AF.Copy,
                            scale=dvar[:, dc:dc + 1])
                        v0.append(v)

                    # --- propagation round 1: y1 = C^T v0 ---
                    v1 = []
                    for dc in range(2):
                        py = ps.tile([128, 2048], F32, tag="big")
                        for w in range(3):
                            for sc in range(2):
                                nc.tensor.matmul(
                                    out=py[:, w * 512:(w + 1) * 512],
                                    lhsT=cm[sc][:, dc * 128:(dc + 1) * 128],
                                    rhs=v0[sc][:, w * 512:(w + 1) * 512],
                                    start=(sc == 0), stop=(sc == 1))
                        pyv = py[:, 0:1536].rearrange("p (h k d) -> p h k d", h=H, k=3)
                        a = accv[dc]
                        # acc += -dinv * y1[k=1]
                        nc.vector.scalar_tensor_tensor(
                            out=a, in0=pyv[:, :, 0, :],
                            scalar=dvar[:, 4 + dc:5 + dc],
                            in1=a, op0=ALU.mult, op1=ALU.add)
                        # acc += 3 dinv * y1[k=3]
                        nc.vector.scalar_tensor_tensor(
                            out=a, in0=pyv[:, :, 2, :],
                            scalar=dvar[:, 6 + dc:7 + dc],
                            in1=a, op0=ALU.mult, op1=ALU.add)
                        # v1 = dinv^2 * y1[k=2,3]
                        v = st2a.tile([128, H * 2 * DH], F32R, tag="v1")
                        nc.scalar.activation(
                            out=v.rearrange("p (h k d) -> p h k d", h=H, k=2),
                            in_=pyv[:, :, 1:3, :], func=AF.Copy,
                            scale=dvar[:, 2 + dc:3 + dc])
                        v1.append(v)

                    # --- round 2 ---
                    v2 = []
                    for dc in range(2):
                        py = ps.tile([128, 2048], F32, tag="big")
                        for w in range(2):
                            for sc in range(2):
                                nc.tensor.matmul(
                                    out=py[:, w * 512:(w + 1) * 512],
                                    lhsT=cm[sc][:, dc * 128:(dc + 1) * 128],
                                    rhs=v1[sc][:, w * 512:(w + 1) * 512],
                                    start=(sc == 0), stop=(sc == 1))
                        pyv = py[:, 0:1024].rearrange("p (h k d) -> p h k d", h=H, k=2)
                        a = accv[dc]
                        # acc += 2 dinv * y2[k=2]
                        nc.vector.scalar_tensor_tensor(
                            out=a, in0=pyv[:, :, 0, :],
                            scalar=dvar[:, 8 + dc:9 + dc],
                            in1=a, op0=ALU.mult, op1=ALU.add)
                        v = st2a.tile([128, H * DH], F32R, tag="v2")
                        nc.scalar.activation(
                            out=v.rearrange("p (h d) -> p h d", h=H),
                            in_=pyv[:, :, 1, :], func=AF.Copy,
                            scale=dvar[:, 2 + dc:3 + dc])
                        v2.append(v)

                    # --- round 3 ---
                    for dc in range(2):
                        py = ps.tile([128, 2048], F32, tag="big")
                        for sc in range(2):
                            nc.tensor.matmul(
                                out=py[:, 0:512],
                                lhsT=cm[sc][:, dc * 128:(dc + 1) * 128],
                                rhs=v2[sc], start=(sc == 0), stop=(sc == 1))
                        a = accv[dc]
                        # acc += -4 dinv * y3
                        nc.vector.scalar_tensor_tensor(
                            out=a,
                            in0=py[:, 0:512].rearrange("p (h d) -> p h d", h=H),
                            scalar=dvar[:, 10 + dc:11 + dc],
                            in1=a, op0=ALU.mult, op1=ALU.add)

            # =====================================================
            # Phase C: transpose acc, cat-matmul, LayerNorm, store
            # =====================================================
            with tc.tile_pool(name="ph_c", bufs=1) as phc, \
                 tc.tile_pool(name="ln", bufs=3) as lnp, \
                 tc.tile_pool(name="lns", bufs=4) as lns:
                accT = phc.tile([128, 4 * GPC * N], F32R)
                for nch in range(GPC * NCHUNK):
                    pst = ps.tile([128, 2048], F32, tag="big")
                    for fb in range(4):
                        nc.tensor.transpose(
                            pst[:, fb * 128:(fb + 1) * 128],
                            acc_all[:, nch * D + fb * 128:nch * D + (fb + 1) * 128],
                            identity)
                    nc.scalar.copy(
                        out=accT.rearrange("p (f x) -> p f x", f=4)
                            [:, :, nch * 128:(nch + 1) * 128],
                        in_=pst[:, 0:512].rearrange("p (f n) -> p f n", f=4))

                for nch in range(GPC * NCHUNK):
                    g, dc = divmod(nch, NCHUNK)
                    po = ps.tile([128, 2048], F32, tag="big")
                    for fb in range(4):
                        nc.tensor.matmul(
                            out=po[:, 0:512],
                            lhsT=outT[:, fb * (GPC * N) + nch * 128:
                                      fb * (GPC * N) + (nch + 1) * 128],
                            rhs=wcat_sb[:, fb * D:(fb + 1) * D],
                            start=(fb == 0), stop=False)
                    for fb in range(4):
                        nc.tensor.matmul(
                            out=po[:, 0:512],
                            lhsT=accT[:, fb * (GPC * N) + nch * 128:
                                      fb * (GPC * N) + (nch + 1) * 128],
                            rhs=wcat_sb[:, (4 + fb) * D:(5 + fb) * D],
                            start=False, stop=(fb == 3))

                    # LayerNorm
                    t0 = lnp.tile([128, D], F32, tag="t0")
                    musum = lns.tile([128, 1], F32, tag="musum")
                    nc.vector.scalar_tensor_tensor(
                        out=t0, in0=po[:, 0:512], scalar=1.0, in1=bcat2_bc,
                        op0=ALU.mult, op1=ALU.add, accum_out=musum)
                    negmu = lns.tile([128, 1], F32, tag="negmu")
                    nc.scalar.mul(negmu, musum, -1.0 / D)
                    sqs = lns.tile([128, 1], F32, tag="sqs")
                    sq = lnp.tile([128, D], F32, tag="sq")
                    nc.scalar.activation(out=sq, in_=t0, func=AF.Square,
                                         bias=negmu, accum_out=sqs)
                    sd = lns.tile([128, 1], F32, tag="sd")
                    nc.scalar.activation(out=sd, in_=sqs, func=AF.Sqrt,
                                         scale=1.0 / D, bias=eps_col)
                    rstd = lns.tile([128, 1], F32, tag="rstd")
                    nc.vector.reciprocal(out=rstd, in_=sd)
                    nmr = lns.tile([128, 1], F32, tag="nmr")
                    nc.vector.tensor_mul(nmr, negmu, rstd)
                    t1 = lnp.tile([128, D], F32, tag="t1")
                    nc.scalar.activation(out=t1, in_=t0, func=AF.Identity,
                                         scale=rstd, bias=nmr)
                    t2 = lnp.tile([128, D], F32, tag="t2")
                    nc.vector.tensor_mul(t2, t1, gamma_bc)
                    nc.vector.tensor_add(t2, t2, beta_bc)
                    nc.gpsimd.dma_start(out=p_outy[dc * 128:(dc + 1) * 128, g, :], in_=t2)

    if not nc.is_finalized():
        nc.finalize()
    return nc


# ---------------------------------------------------------------------------
# host side
# ---------------------------------------------------------------------------

def _canonical_indices(feature_indices, batch):
    fi = np.asarray(feature_indices)
    bt = np.asarray(batch)
    want0 = np.repeat(np.arange(B), N)
    want1 = np.tile(np.arange(N), B)
    return (fi.shape == (B * N, 2) and bt.shape == (B * N,)
            and np.array_equal(fi[:, 0], want0) and np.array_equal(fi[:, 1], want1)
            and np.array_equal(bt, want0))


def _prep(inputs):
    """Host-side sharding + index preprocessing. Returns per-core input maps."""
    attn = np.ascontiguousarray(np.asarray(inputs["attn"], np.float32))
    oeh = np.ascontiguousarray(np.asarray(inputs["out_each_head"], np.float32))
    outp = np.ascontiguousarray(np.asarray(inputs["output"], np.float32))
    ei = np.asarray(inputs["edge_index"])

    W_gcn = np.asarray(inputs["W_gcn"], np.float32)
    b_gcn = np.asarray(inputs["b_gcn"], np.float32)
    W_lin = np.asarray(inputs["W_lin"], np.float32)
    b_lin = np.asarray(inputs["b_lin"], np.float32)
    W_cheb = np.asarray(inputs["W_cheb"], np.float32)
    b_cheb = np.asarray(inputs["b_cheb"], np.float32)
    W_cat = np.asarray(inputs["W_cat"], np.float32)
    b_cat = np.asarray(inputs["b_cat"], np.float32)
    gamma = np.asarray(inputs["gamma"], np.float32)
    beta = np.asarray(inputs["beta"], np.float32)

    # dense per-graph edge-count matrices (pure integer indexing)
    s_g = (ei[0] // N).astype(np.int64)
    s_l = (ei[0] % N).astype(np.int64)
    d_l = (ei[1] % N).astype(np.int64)
    flat = np.zeros(B * N * N, np.float32)
    np.add.at(flat, s_g * (N * N) + s_l * N + d_l, 1.0)
    cmat = flat.reshape(B, N, N)

    wcheb = np.ascontiguousarray(
        W_cheb.transpose(1, 0, 2).reshape(DH, ORDER * DH))   # [dh, (k, dh')]
    wrow = W_gcn.sum(axis=0).reshape(1, ORDER).astype(np.float32)
    bcat2 = (b_cat + np.tile(b_cheb, H) @ W_cat[D:, :]).reshape(1, D).astype(np.float32)

    # stage-1 streaming-form constants
    mask8 = np.zeros((H, H * N), np.float32)
    for h in range(H):
        mask8[h, h * N:(h + 1) * N] = 1.0
    kcorr = ((H * N - N) * np.tanh(b_gcn)).reshape(ORDER, 1).astype(np.float32)
    wlin4 = (W_lin / N).astype(np.float32)
    blin4 = b_lin.reshape(ORDER, 1).astype(np.float32)

    shared = dict(
        wcheb=wcheb, wcat=np.ascontiguousarray(W_cat),
        bcat2=bcat2, gamma=gamma.reshape(1, D).astype(np.float32),
        beta=beta.reshape(1, D).astype(np.float32),
        wrow=wrow, bgcn=b_gcn.reshape(1, ORDER).astype(np.float32),
        mask8=mask8, wlin4=wlin4, kcorr=kcorr, blin4=blin4)

    in_maps = []
    for c in range(NCORES):
        G = slice(c * GPC, (c + 1) * GPC)
        in_maps.append(dict(
            attn_s=np.ascontiguousarray(attn[G]),
            x0_s=np.ascontiguousarray(oeh[G]),
            out_in_s=np.ascontiguousarray(outp[:, G, :]),
            cmat_s=np.ascontiguousarray(cmat[G]),
            **shared))
    return in_maps


def _fallback_numpy(inputs):
    """Generic (slow) numpy path, used only if the index tensors deviate from
    the canonical layout produced by setup_inputs()."""
    output = np.asarray(inputs["output"], np.float32)
    attn = np.asarray(inputs["attn"], np.float32)
    oeh = np.asarray(inputs["out_each_head"], np.float32)
    ei = np.asarray(inputs["edge_index"])
    fi = np.asarray(inputs["feature_indices"])
    batch = np.asarray(inputs["batch"])
    W_gcn = np.asarray(inputs["W_gcn"], np.float32); b_gcn = np.asarray(inputs["b_gcn"], np.float32)
    W_lin = np.asarray(inputs["W_lin"], np.float32); b_lin = np.asarray(inputs["b_lin"], np.float32)
    W_cheb = np.asarray(inputs["W_cheb"], np.float32); b_cheb = np.asarray(inputs["b_cheb"], np.float32)
    W_cat = np.asarray(inputs["W_cat"], np.float32); b_cat = np.asarray(inputs["b_cat"], np.float32)
    gamma = np.asarray(inputs["gamma"], np.float32); beta = np.asarray(inputs["beta"], np.float32)

    Bn, Hn, Nn, _ = attn.shape
    C = W_gcn.shape[0]
    total = Bn * Nn
    NT = Hn * total
    A = attn.transpose(1, 0, 2, 3).reshape(Hn * Bn, Nn, Nn)
    A_hat = A + np.eye(Nn, dtype=A.dtype)
    deg = A_hat.sum(axis=1)
    dinv = np.where(deg > 0, 1.0 / np.sqrt(deg), 0.0).astype(np.float32)
    w_row = W_gcn.sum(axis=0)
    s = np.einsum('bi,bij->bj', dinv, A_hat) * dinv
    x_c = np.tanh(s[:, :, None] * w_row + b_gcn)
    gap = x_c.mean(axis=1)
    coeff = gap @ W_lin + b_lin

    offsets = (np.arange(Hn) * total).astype(ei.dtype)
    src = (ei[0][None, :] + offsets[:, None]).reshape(-1)
    dst = (ei[1][None, :] + offsets[:, None]).reshape(-1)
    deg_n = np.zeros(NT, np.float32)
    np.add.at(deg_n, dst, 1.0)
    dinv_n = np.where(deg_n > 0, 1.0 / np.sqrt(np.maximum(deg_n, 1e-30)), 0.0).astype(np.float32)
    norm_e = -(dinv_n[src] * dinv_n[dst])

    def prop(x):
        out = np.zeros((NT, x.shape[1]), np.float32)
        np.add.at(out, dst, norm_e[:, None] * x[src])
        return out

    x0 = oeh.transpose(2, 0, 1, 3).reshape(NT, DH)
    batch_all = (batch[None, :] + (np.arange(Hn) * Bn)[:, None]).reshape(-1)
    c_node = coeff[batch_all]
    T_prev, T_cur = x0, prop(x0)
    acc = c_node[:, 0:1] * (T_prev @ W_cheb[0]) + c_node[:, 1:2] * (T_cur @ W_cheb[1])
    for k in range(2, C):
        T_next = 2.0 * prop(T_cur) - T_prev
        acc = acc + c_node[:, k:k + 1] * (T_next @ W_cheb[k])
        T_prev, T_cur = T_cur, T_next
    acc = acc + b_cheb
    filtered = acc.reshape(Hn, total, DH).transpose(1, 0, 2).reshape(total, Hn * DH)
    out_filtered = np.zeros_like(output)
    out_filtered[fi[:, 1], fi[:, 0], :] = filtered
    out_cat = np.concatenate([output, out_filtered], axis=-1)
    out = out_cat @ W_cat + b_cat
    mu = out.mean(axis=-1, keepdims=True)
    var = ((out - mu) ** 2).mean(axis=-1, keepdims=True)
    return ((out - mu) / np.sqrt(var + 1e-5) * gamma + beta).astype(np.float32)


def _get_nc():
    if "nc" not in _CACHE:
        _CACHE["nc"] = _build_module(use_f32r=_CACHE.get("use_f32r", True))
    return _CACHE["nc"]


def kernel(**inputs) -> np.ndarray:
    if not _canonical_indices(inputs["feature_indices"], inputs["batch"]):
        return _fallback_numpy(inputs)

    from concourse.bass_utils import run_bass_kernel_spmd

    nc = _get_nc()
    in_maps = _prep(inputs)
    res = run_bass_kernel_spmd(nc, in_maps, list(range(NCORES)))
    out = np.empty((N, B, D), np.float32)
    for c in range(NCORES):
        out[:, c * GPC:(c + 1) * GPC, :] = res.results[c]["outy"]
    return out


# revision 36
# speedup vs baseline: 2.0316x; 1.0003x over previous
"""Trainium2 Bass kernel for nn_DiffGraphTransformerGenGCN.

Strategy (see spec sharding hint): data-parallel over the 64 graphs, 8 graphs
per NeuronCore.  Per core everything is reformulated as dense per-graph
matmuls:

  stage 1 (GCN filter coefficients over the attention graphs):
    streaming colsum / dinv-weighted colsum of each 256x256 attention matrix
    (ones / block-diagonal dinv as the stationary operand, attention tiles
    streamed once in float32r), masked tanh with accumulate for the global
    average pool, tiny matmuls for the final linear -> 4 coefficients per
    (graph, head).

  stage 2 (dynamic Chebyshev filtering on the input graph):
    The edge list is converted (host-side, pure integer indexing) into a
    dense 256x256 edge-count matrix C_g per graph.  The scaled Laplacian
    action is  P x = -D C^T D x  with D = diag(rsqrt(indeg)).  The Chebyshev
    coefficients c_(g,h,k) (stage-1 output) are folded into per-(g,h)
    scaled copies of W_cheb, so  acc = sum_k c_k T_k(P) (x0 W_k)  is computed
    with 6 dense propagation matmuls per graph and no transposes.

  final: out = LayerNorm(concat(output, filtered) @ W_cat + b_cat') computed
    as a feat-major matmul (lhsT = PE-transposed operands).

All large matmuls run in float32r (1 cycle/row vs 4 for fp32); operands are
rounded to f32r at their producers (engine copy-casts / SWDGE cast-DMA).

kernel() takes FULL inputs and returns the FULL output; it shards across the
8 cores internally.
"""

import numpy as np

B, N, H, DH, ORDER = 64, 256, 8, 64, 4
D = H * DH
NCORES = 8
GPC = B // NCORES          # graphs per core
NCHUNK = N // 128          # node chunks per graph (2)

_CACHE = {}


# ---------------------------------------------------------------------------
# device kernel construction
# ---------------------------------------------------------------------------

def _build_module(use_f32r=True):
    import concourse.bass as bass
    import concourse.bacc as bacc
    import concourse.tile as tile
    from concourse import mybir
    from concourse.masks import make_identity

    F32 = mybir.dt.float32
    F32R = mybir.dt.float32r if use_f32r else mybir.dt.float32
    AF = mybir.ActivationFunctionType
    ALU = mybir.AluOpType

    nc = bacc.Bacc(None, target_bir_lowering=False, debug=False)

    # ---- DRAM parameters (per-core shard) ----
    p_attn = nc.declare_dram_parameter("attn_s", [GPC, H, N, N], F32, isOutput=False).ap()
    p_x0 = nc.declare_dram_parameter("x0_s", [GPC, N, H, DH], F32, isOutput=False).ap()
    p_oin = nc.declare_dram_parameter("out_in_s", [N, GPC, D], F32, isOutput=False).ap()
    p_cmat = nc.declare_dram_parameter("cmat_s", [GPC, N, N], F32, isOutput=False).ap()
    p_wcheb = nc.declare_dram_parameter("wcheb", [DH, ORDER * DH], F32, isOutput=False).ap()
    p_wcat = nc.declare_dram_parameter("wcat", [2 * D, D], F32, isOutput=False).ap()
    p_bcat2 = nc.declare_dram_parameter("bcat2", [1, D], F32, isOutput=False).ap()
    p_gamma = nc.declare_dram_parameter("gamma", [1, D], F32, isOutput=False).ap()
    p_beta = nc.declare_dram_parameter("beta", [1, D], F32, isOutput=False).ap()
    p_wrow = nc.declare_dram_parameter("wrow", [1, ORDER], F32, isOutput=False).ap()
    p_bgcn = nc.declare_dram_parameter("bgcn", [1, ORDER], F32, isOutput=False).ap()
    p_mask = nc.declare_dram_parameter("mask8", [H, H * N], F32, isOutput=False).ap()
    p_wlin = nc.declare_dram_parameter("wlin4", [ORDER, ORDER], F32, isOutput=False).ap()
    p_kcorr = nc.declare_dram_parameter("kcorr", [ORDER, 1], F32, isOutput=False).ap()
    p_blin4 = nc.declare_dram_parameter("blin4", [ORDER, 1], F32, isOutput=False).ap()
    p_outy = nc.declare_dram_parameter("outy", [N, GPC, D], F32, isOutput=True).ap()

    def bcast(dram_ap, parts, inner, extra_off=0):
        return bass.AP(tensor=dram_ap.tensor, offset=dram_ap.offset + extra_off,
                       ap=[[0, parts]] + inner)

    with tile.TileContext(nc) as tc:
        import contextlib
        ctx = contextlib.ExitStack()
        with ctx:
            const = ctx.enter_context(tc.tile_pool(name="const", bufs=1))
            persist = ctx.enter_context(tc.tile_pool(name="persist", bufs=1))
            ps = ctx.enter_context(tc.tile_pool(name="ps", bufs=2, space="PSUM"))
            dram = ctx.enter_context(tc.tile_pool(name="dram", bufs=1, space="DRAM"))

            # ---------------- constants ----------------
            ones_col = const.tile([128, 1], F32)
            nc.vector.memset(ones_col, 1.0)
            ones_r = const.tile([128, 1], F32R)
            nc.vector.tensor_copy(ones_r, ones_col)
            ones2 = const.tile([128, 2], F32)
            nc.vector.memset(ones2, 1.0)
            ones2_r = const.tile([128, 2], F32R)
            nc.vector.tensor_copy(ones2_r, ones2)
            identity = const.tile([128, 128], F32)
            make_identity(nc, identity)
            wcheb_sb = const.tile([DH, ORDER * DH], F32)
            nc.sync.dma_start(out=wcheb_sb, in_=p_wcheb)
            wcat_sb = const.tile([128, 8 * D], F32R)
            bcat2_bc = const.tile([128, D], F32)
            nc.sync.dma_start(out=bcat2_bc, in_=bcast(p_bcat2, 128, [[1, D]]))
            gamma_bc = const.tile([128, D], F32)
            nc.sync.dma_start(out=gamma_bc, in_=bcast(p_gamma, 128, [[1, D]]))
            beta_bc = const.tile([128, D], F32)
            nc.sync.dma_start(out=beta_bc, in_=bcast(p_beta, 128, [[1, D]]))
            wrow_bc = const.tile([128, ORDER], F32)
            nc.sync.dma_start(out=wrow_bc, in_=bcast(p_wrow, 128, [[1, ORDER]]))
            bgcn_bc = const.tile([128, ORDER], F32)
            nc.sync.dma_start(out=bgcn_bc, in_=bcast(p_bgcn, 128, [[1, ORDER]]))
            mask_sb = const.tile([H, H * N], F32)
            nc.sync.dma_start(out=mask_sb, in_=p_mask)
            wlin_sb = const.tile([ORDER, ORDER], F32)
            nc.sync.dma_start(out=wlin_sb, in_=p_wlin)
            kcorr_sb = const.tile([ORDER, 1], F32)
            nc.sync.dma_start(out=kcorr_sb, in_=p_kcorr)
            blin4_sb = const.tile([ORDER, 1], F32)
            nc.sync.dma_start(out=blin4_sb, in_=p_blin4)
            eps_col = const.tile([128, 1], F32)
            nc.vector.memset(eps_col, 1e-5)

            # persistent big tiles
            acc_all = persist.tile([128, GPC * NCHUNK * D], F32)    # 4 MB
            outT = persist.tile([128, 4 * GPC * N], F32R)           # 4 MB
            dcrow2 = dram.tile([GPC, H * ORDER], F32)
            dcrow_exp = dram.tile([GPC, DH * H * ORDER], F32)
            ddinv = dram.tile([GPC, H * N], F32)

            # 'output' input, loaded node-major then PE-transposed to
            # feature-major: outT[f in fb-block, fb*2048 + g*256 + n].
            # Emitted interleaved with stage-1 (2 chunks per graph) so the
            # transposes fill PE gaps in stage-1's serial tail.
            oinp = ctx.enter_context(tc.tile_pool(name="oin", bufs=2))
            for fb in range(8):
                wb = oinp.tile([128, D], F32, tag="on")
                nc.sync.dma_start(out=wb, in_=p_wcat[fb * 128:(fb + 1) * 128, :])
                nc.scalar.copy(out=wcat_sb[:, fb * D:(fb + 1) * D], in_=wb)

            def emit_outT_chunk(nch):
                g, dc = divmod(nch, NCHUNK)
                on = oinp.tile([128, D], F32, tag="on", name=f"on_{nch}")
                nc.sync.dma_start(out=on, in_=p_oin[dc * 128:(dc + 1) * 128, g, :])
                pso = ps.tile([128, 2048], F32, tag="big", name=f"pso_{nch}")
                for fb in range(4):
                    nc.tensor.transpose(pso[:, fb * 128:(fb + 1) * 128],
                                        on[:, fb * 128:(fb + 1) * 128], identity)
                nc.vector.tensor_copy(
                    outT.rearrange("p (f x) -> p f x", f=4)
                        [:, :, nch * 128:(nch + 1) * 128],
                    pso[:, 0:512].rearrange("p (f n) -> p f n", f=4))

            # =====================================================
            # Phase A: stage-1 GCN coefficients (streaming form)
            # =====================================================
            with tc.tile_pool(name="stage1", bufs=3) as s1p, \
                 tc.tile_pool(name="stage1s", bufs=4) as s1s, \
                 tc.tile_pool(name="s1w", bufs=1) as s1w:
                for g in range(GPC):
                    emit_outT_chunk(2 * g)
                    emit_outT_chunk(2 * g + 1)
                    Atr = []
                    for ic in range(2):
                        a = s1p.tile([128, H * N], F32, tag="attn")
                        (nc.sync if ic == 0 else nc.scalar).dma_start(
                            out=a.rearrange("p (h j) -> p h j", h=H),
                            in_=p_attn[g, :, ic * 128:(ic + 1) * 128, :]
                                .rearrange("h i j -> i h j"))
                        ar = s1p.tile([128, H * N], F32R, tag="attnr")
                        if ic == 0:
                            nc.scalar.copy(out=ar, in_=a)
                        else:
                            nc.vector.tensor_copy(ar, a)
                        Atr.append(ar)
                    # r1: colsum rows [1, (h, j)]  (one psum tile per graph)
                    psA = ps.tile([128, 2048], F32, tag="big")
                    psT = psA
                    psD = psA
                    psC = psA
                    for w in range(4):
                        for ic in range(2):
                            nc.tensor.matmul(
                                out=psA[0:1, w * 512:(w + 1) * 512],
                                lhsT=ones_r, rhs=Atr[ic][:, w * 512:(w + 1) * 512],
                                start=(ic == 0), stop=(ic == 1))
                    # dinv row = 1/sqrt(colsum + 1)
                    sqrow = s1w.tile([1, H * N], F32, tag="sqrow")
                    nc.scalar.activation(out=sqrow, in_=psA[0:1, 0:2048],
                                         func=AF.Sqrt, bias=1.0)
                    dinvrow = s1w.tile([1, H * N], F32, tag="dinvrow")
                    rscr = s1w.tile([1, H * N], F32, tag="junk")
                    nc.vector.reciprocal_approx_accurate(out=dinvrow, in_=sqrow,
                                                         scratch=rscr)
                    nc.sync.dma_start(out=ddinv[g:g + 1, :], in_=dinvrow)
                    # transpose dinv row-chunks -> columns [128, (ic, h)]
                    for ic in range(2):
                        for h in range(H):
                            nc.tensor.transpose(
                                psT[:, (ic * 8 + h) * 128:(ic * 8 + h) * 128 + 1],
                                dinvrow[0:1, h * N + ic * 128:h * N + (ic + 1) * 128],
                                identity[0:1, 0:1])
                    dinvT = s1s.tile([128, 16], F32R, tag="dinvT")
                    nc.scalar.copy(
                        out=dinvT,
                        in_=psT[:, 0:2048].rearrange("p (c x) -> p c x", c=16)[:, :, 0])
                    # r2: dot rows [8, (h', j)] via block-diag dinv lhsT
                    for w in range(4):
                        for ic in range(2):
                            nc.tensor.matmul(
                                out=psD[0:8, w * 512:(w + 1) * 512],
                                lhsT=dinvT[:, ic * 8:(ic + 1) * 8],
                                rhs=Atr[ic][:, w * 512:(w + 1) * 512],
                                start=(ic == 0), stop=(ic == 1))
                    # s = (dot + dinv) * dinv, diag-masked
                    dinvB = s1w.tile([H, H * N], F32, tag="dinvB")
                    nc.sync.dma_start(out=dinvB,
                                      in_=bcast(ddinv[0], H, [[1, H * N]],
                                                extra_off=g * H * N))
                    maskD = s1w.tile([H, H * N], F32, tag="maskD")
                    nc.vector.tensor_mul(maskD, dinvB, mask_sb)
                    t8 = s1w.tile([H, H * N], F32, tag="t8")
                    nc.vector.tensor_add(t8, psD[0:8, 0:2048], dinvB)
                    sm = s1w.tile([H, H * N], F32, tag="sm")
                    nc.vector.tensor_mul(sm, t8, maskD)
                    # tanh + GAP (accumulate); off-diag contributes tanh(b_c)
                    gacc = s1s.tile([H, ORDER], F32, tag="gacc")
                    for c in range(ORDER):
                        junk = s1w.tile([H, H * N], F32, tag="junk")
                        nc.scalar.activation(
                            out=junk, in_=sm, func=AF.Tanh,
                            scale=wrow_bc[0:H, c:c + 1], bias=bgcn_bc[0:H, c:c + 1],
                            accum_out=gacc[:, c:c + 1])
                    # coeff = (gacc - corr)/256 @ W_lin + b_lin
                    nc.tensor.transpose(psC[0:ORDER, 0:H], gacc, identity[0:H, 0:H])
                    gapT = s1s.tile([ORDER, H], F32, tag="gapT")
                    nc.vector.tensor_scalar_sub(gapT, psC[0:ORDER, 0:H], kcorr_sb)
                    nc.tensor.matmul(out=psC[0:ORDER, 512:512 + H], lhsT=wlin_sb,
                                     rhs=gapT, start=True, stop=True)
                    coefT = s1s.tile([ORDER, H], F32, tag="coefT")
                    nc.vector.tensor_scalar_add(coefT, psC[0:ORDER, 512:512 + H],
                                                blin4_sb)
                    nc.tensor.transpose(psC[0:H, 1024:1024 + ORDER], coefT,
                                        identity[0:ORDER, 0:ORDER])
                    crow_sb = s1s.tile([H, ORDER], F32, tag="crow_sb")
                    nc.scalar.copy(out=crow_sb, in_=psC[0:H, 1024:1024 + ORDER])
                    nc.sync.dma_start(
                        out=dcrow2[g:g + 1, :].rearrange("o (h k) -> o h k", k=ORDER),
                        in_=crow_sb)

            # expand coefficients: dcrow_exp[g, (dh', h, k)] = c[g, h, k]
            nc.sync.dma_start(
                out=dcrow_exp.rearrange("g (d hk) -> g d hk", d=DH),
                in_=bass.AP(tensor=dcrow2.tensor, offset=dcrow2.offset,
                            ap=[[H * ORDER, GPC], [0, DH], [1, H * ORDER]]))

            # =====================================================
            # Phase B: Chebyshev propagation per graph
            # =====================================================
            with tc.tile_pool(name="st2a", bufs=2) as st2a, \
                 tc.tile_pool(name="st2b", bufs=2) as st2b, \
                 tc.tile_pool(name="st2s", bufs=4) as st2s:
                for g in range(GPC):
                    # --- C_g (cast to f32r during DMA) and degree columns ---
                    cm = []
                    for sc in range(2):
                        tf = st2a.tile([128, N], F32, tag="cmf")
                        nc.sync.dma_start(out=tf,
                                          in_=p_cmat[g, sc * 128:(sc + 1) * 128, :])
                        t = st2a.tile([128, N], F32R, tag="cm")
                        nc.scalar.copy(out=t, in_=tf)
                        cm.append(t)
                    psd = ps.tile([128, 2048], F32, tag="big")
                    for dc in range(2):
                        for sc in range(2):
                            nc.tensor.matmul(
                                out=psd[:, dc * 2:dc * 2 + 2],
                                lhsT=cm[sc][:, dc * 128:(dc + 1) * 128],
                                rhs=ones2_r, start=(sc == 0), stop=(sc == 1))
                    # masked rsqrt: dinv = m / (sqrt(deg) + 1 - m), m = sign(deg)
                    # dvar layout [128, (kind, dc)]: kinds
                    # 0=dinv 1=dsq 2=dneg 3=d3 4=d2 5=dm4
                    dvar = st2s.tile([128, 12], F32, tag="dvar")
                    sg = st2s.tile([128, 2], F32, tag="sg")
                    psdv = psd[:, 0:4].rearrange('p (d two) -> p d two', two=2)[:, :, 0]
                    nc.scalar.activation(out=sg, in_=psdv, func=AF.Sign)
                    sq = st2s.tile([128, 2], F32, tag="sq")
                    nc.scalar.activation(out=sq, in_=psdv, func=AF.Sqrt)
                    den = st2s.tile([128, 2], F32, tag="den")
                    nc.vector.scalar_tensor_tensor(
                        out=den, in0=sq, scalar=1.0, in1=sg,
                        op0=ALU.add, op1=ALU.subtract)
                    rec = st2s.tile([128, 2], F32, tag="rec")
                    nc.vector.reciprocal(out=rec, in_=den)
                    dv = dvar[:, 0:2]
                    nc.vector.tensor_mul(dv, rec, sg)
                    nc.scalar.square(dvar[:, 2:4], dv)
                    nc.scalar.mul(dvar[:, 4:6], dv, -1.0)
                    nc.scalar.mul(dvar[:, 6:8], dv, 3.0)
                    nc.scalar.mul(dvar[:, 8:10], dv, 2.0)
                    nc.scalar.mul(dvar[:, 10:12], dv, -4.0)

                    # --- x0^T via PE transpose (cast to f32r on eviction) ---
                    x0t = st2b.tile([DH, H * N], F32R, tag="x0t")
                    for nc2 in range(2):
                        x0n = st2s.tile([128, H * DH], F32, tag="x0n")
                        nc.scalar.dma_start(
                            out=x0n.rearrange("p (h d) -> p h d", h=H),
                            in_=p_x0[g, nc2 * 128:(nc2 + 1) * 128, :, :])
                        psx = ps.tile([128, 2048], F32, tag="big")
                        for h in range(H):
                            nc.tensor.transpose(psx[0:DH, h * 128:(h + 1) * 128],
                                                x0n[:, h * DH:(h + 1) * DH], identity)
                        nc.vector.tensor_copy(
                            x0t.rearrange("p (h x) -> p h x", h=H)
                               [:, :, nc2 * 128:(nc2 + 1) * 128],
                            psx[0:DH, 0:1024].rearrange("p (h n) -> p h n", h=H))

                    # --- c-scaled W_cheb:
                    # cexp[p, (dh', hk)] broadcast from dcrow_exp[g]
                    cexp = st2b.tile([DH, DH * H * ORDER], F32, tag="cexp")
                    nc.sync.dma_start(
                        out=cexp,
                        in_=bcast(dcrow_exp[0], DH, [[1, DH * H * ORDER]],
                                  extra_off=g * DH * H * ORDER))
                    wsc = st2b.tile([DH, H * ORDER * DH], F32R, tag="wsc")
                    cexpv = cexp.rearrange("p (d h2 k) -> p h2 k d", d=DH, k=ORDER)
                    for h in range(H):
                        nc.vector.tensor_mul(
                            wsc[:, h * 256:(h + 1) * 256]
                                .rearrange("p (k d) -> p k d", k=ORDER),
                            cexpv[:, h, :, :],
                            wcheb_sb.rearrange("p (k d) -> p k d", k=ORDER))

                    # --- Z matmuls:  Z[(n),(k,dh')] = x0 @ (c_k W_k) ---
                    accv = []
                    v0 = []
                    for dc in range(2):
                        pz = ps.tile([128, 2048], F32, tag="big")
                        for h in range(H):
                            nc.tensor.matmul(
                                out=pz[:, h * 256:(h + 1) * 256],
                                lhsT=x0t[:, h * N + dc * 128:h * N + (dc + 1) * 128],
                                rhs=wsc[:, h * 256:(h + 1) * 256],
                                start=True, stop=True)
                        pzv = pz.rearrange("p (h k d) -> p h k d", h=H, k=ORDER)
                        a = acc_all[:, (g * 2 + dc) * D:(g * 2 + dc + 1) * D] \
                            .rearrange("p (h d) -> p h d", h=H)
                        accv.append(a)
                        # acc = Z0 - Z2  (one PSUM operand per instruction)
                        nc.scalar.mul(a, pzv[:, :, 2, :], -1.0)
                        nc.vector.tensor_tensor(
                            out=a, in0=pzv[:, :, 0, :], in1=a, op=ALU.add)
                        # v0 = dinv * Z[1:4]
                        v = st2a.tile([128, H * 3 * DH], F32R, tag="v0")
                        nc.scalar.activation(
                            out=v.rearrange("p (h k d) -> p h k d", h=H, k=3),
                            in_=pzv[:, :, 1:4, :], func=AF.Copy,
                            scale=dvar[:, dc:dc + 1])
                        v0.append(v)

                    # --- propagation round 1: y1 = C^T v0 ---
                    v1 = []
                    for dc in range(2):
                        py = ps.tile([128, 2048], F32, tag="big")
                        for w in range(3):
                            for sc in range(2):
                                nc.tensor.matmul(
                                    out=py[:, w * 512:(w + 1) * 512],
                                    lhsT=cm[sc][:, dc * 128:(dc + 1) * 128],
                                    rhs=v0[sc][:, w * 512:(w + 1) * 512],
                                    start=(sc == 0), stop=(sc == 1))
                        pyv = py[:, 0:1536].rearrange("p (h k d) -> p h k d", h=H, k=3)
                        a = accv[dc]
                        # acc += -dinv * y1[k=1]
                        nc.vector.scalar_tensor_tensor(
                            out=a, in0=pyv[:, :, 0, :],
                            scalar=dvar[:, 4 + dc:5 + dc],
                            in1=a, op0=ALU.mult, op1=ALU.add)
                        # acc += 3 dinv * y1[k=3]
                        nc.vector.scalar_tensor_tensor(
                            out=a, in0=pyv[:, :, 2, :],
                            scalar=dvar[:, 6 + dc:7 + dc],
                            in1=a, op0=ALU.mult, op1=ALU.add)
                        # v1 = dinv^2 * y1[k=2,3]
                        v = st2a.tile([128, H * 2 * DH], F32R, tag="v1")
                        nc.scalar.activation(
                            out=v.rearrange("p (h k d) -> p h k d", h=H, k=2),
                            in_=pyv[:, :, 1:3, :], func=AF.Copy,
                            scale=dvar[:, 2 + dc:3 + dc])
                        v1.append(v)

                    # --- round 2 ---
                    v2 = []
                    for dc in range(2):
                        py = ps.tile([128, 2048], F32, tag="big")
                        for w in range(2):
                            for sc in range(2):
                                nc.tensor.matmul(
                                    out=py[:, w * 512:(w + 1) * 512],
                                    lhsT=cm[sc][:, dc * 128:(dc + 1) * 128],
                                    rhs=v1[sc][:, w * 512:(w + 1) * 512],
                                    start=(sc == 0), stop=(sc == 1))
                        pyv = py[:, 0:1024].rearrange("p (h k d) -> p h k d", h=H, k=2)
                        a = accv[dc]
                        # acc += 2 dinv * y2[k=2]
                        nc.vector.scalar_tensor_tensor(
                            out=a, in0=pyv[:, :, 0, :],
                            scalar=dvar[:, 8 + dc:9 + dc],
                            in1=a, op0=ALU.mult, op1=ALU.add)
                        v = st2a.tile([128, H * DH], F32R, tag="v2")
                        nc.scalar.activation(
                            out=v.rearrange("p (h d) -> p h d", h=H),
                            in_=pyv[:, :, 1, :], func=AF.Copy,
                            scale=dvar[:, 2 + dc:3 + dc])
                        v2.append(v)

                    # --- round 3 ---
                    for dc in range(2):
                        py = ps.tile([128, 2048], F32, tag="big")
                        for sc in range(2):
                            nc.tensor.matmul(
                                out=py[:, 0:512],
                                lhsT=cm[sc][:, dc * 128:(dc + 1) * 128],
                                rhs=v2[sc], start=(sc == 0), stop=(sc == 1))
                        a = accv[dc]
                        # acc += -4 dinv * y3
                        nc.vector.scalar_tensor_tensor(
                            out=a,
                            in0=py[:, 0:512].rearrange("p (h d) -> p h d", h=H),
                            scalar=dvar[:, 10 + dc:11 + dc],
                            in1=a, op0=ALU.mult, op1=ALU.add)

            # =====================================================
            # Phase C: transpose acc, cat-matmul, LayerNorm, store
            # =====================================================
            with tc.tile_pool(name="ph_c", bufs=1) as phc, \
                 tc.tile_pool(name="ln", bufs=3) as lnp, \
                 tc.tile_pool(name="lns", bufs=4) as lns:
                accT = phc.tile([128, 4 * GPC * N], F32R)
                for nch in range(GPC * NCHUNK):
                    pst = ps.tile([128, 2048], F32, tag="big")
                    for fb in range(4):
                        nc.tensor.transpose(
                            pst[:, fb * 128:(fb + 1) * 128],
                            acc_all[:, nch * D + fb * 128:nch * D + (fb + 1) * 128],
                            identity)
                    nc.scalar.copy(
                        out=accT.rearrange("p (f x) -> p f x", f=4)
                            [:, :, nch * 128:(nch + 1) * 128],
                        in_=pst[:, 0:512].rearrange("p (f n) -> p f n", f=4))

                for nch in range(GPC * NCHUNK):
                    g, dc = divmod(nch, NCHUNK)
                    po = ps.tile([128, 2048], F32, tag="big")
                    for fb in range(4):
                        nc.tensor.matmul(
                            out=po[:, 0:512],
                            lhsT=outT[:, fb * (GPC * N) + nch * 128:
                                      fb * (GPC * N) + (nch + 1) * 128],
                            rhs=wcat_sb[:, fb * D:(fb + 1) * D],
                            start=(fb == 0), stop=False)
                    for fb in range(4):
                        nc.tensor.matmul(
                            out=po[:, 0:512],
                            lhsT=accT[:, fb * (GPC * N) + nch * 128:
                                      fb * (GPC * N) + (nch + 1) * 128],
                            rhs=wcat_sb[:, (4 + fb) * D:(5 + fb) * D],
                            start=False, stop=(fb == 3))

                    # LayerNorm
                    t0 = lnp.tile([128, D], F32, tag="t0")
                    musum = lns.tile([128, 1], F32, tag="musum")
                    nc.vector.scalar_tensor_tensor(
                        out=t0, in0=po[:, 0:512], scalar=1.0, in1=bcat2_bc,
                        op0=ALU.mult, op1=ALU.add, accum_out=musum)
                    negmu = lns.tile([128, 1], F32, tag="negmu")
                    nc.scalar.mul(negmu, musum, -1.0 / D)
                    sqs = lns.tile([128, 1], F32, tag="sqs")
                    sq = lnp.tile([128, D], F32, tag="sq")
                    nc.scalar.activation(out=sq, in_=t0, func=AF.Square,
                                         bias=negmu, accum_out=sqs)
                    sd = lns.tile([128, 1], F32, tag="sd")
                    nc.scalar.activation(out=sd, in_=sqs, func=AF.Sqrt,
                                         scale=1.0 / D, bias=eps_col)
                    rstd = lns.tile([128, 1], F32, tag="rstd")
                    nc.vector.reciprocal(out=rstd, in_=sd)
                    nmr = lns.tile([128, 1], F32, tag="nmr")
                    nc.vector.tensor_mul(nmr, negmu, rstd)
                    t1 = lnp.tile([128, D], F32, tag="t1")
                    nc.scalar.activation(out=t1, in_=t0, func=AF.Identity,
                                         scale=rstd, bias=nmr)
                    t2 = lnp.tile([128, D], F32, tag="t2")
                    nc.vector.tensor_mul(t2, t1, gamma_bc)
                    nc.vector.tensor_add(t2, t2, beta_bc)
                    nc.gpsimd.dma_start(out=p_outy[dc * 128:(dc + 1) * 128, g, :], in_=t2)

    if not nc.is_finalized():
        nc.finalize()
    return nc


# ---------------------------------------------------------------------------
# host side
# ---------------------------------------------------------------------------

def _canonical_indices(feature_indices, batch):
    fi = np.asarray(feature_indices)
    bt = np.asarray(batch)
    want0 = np.repeat(np.arange(B), N)
    want1 = np.tile(np.arange(N), B)
    return (fi.shape == (B * N, 2) and bt.shape == (B * N,)
            and np.array_equal(fi[:, 0], want0) and np.array_equal(fi[:, 1], want1)
            and np.array_equal(bt, want0))


def _prep(inputs):
    """Host-side sharding + index preprocessing. Returns per-core input maps."""
    attn = np.ascontiguousarray(np.asarray(inputs["attn"], np.float32))
    oeh = np.ascontiguousarray(np.asarray(inputs["out_each_head"], np.float32))
    outp = np.ascontiguousarray(np.asarray(inputs["output"], np.float32))
    ei = np.asarray(inputs["edge_index"])

    W_gcn = np.asarray(inputs["W_gcn"], np.float32)
    b_gcn = np.asarray(inputs["b_gcn"], np.float32)
    W_lin = np.asarray(inputs["W_lin"], np.float32)
    b_lin = np.asarray(inputs["b_lin"], np.float32)
    W_cheb = np.asarray(inputs["W_cheb"], np.float32)
    b_cheb = np.asarray(inputs["b_cheb"], np.float32)
    W_cat = np.asarray(inputs["W_cat"], np.float32)
    b_cat = np.asarray(inputs["b_cat"], np.float32)
    gamma = np.asarray(inputs["gamma"], np.float32)
    beta = np.asarray(inputs["beta"], np.float32)

    # dense per-graph edge-count matrices (pure integer indexing)
    s_g = (ei[0] // N).astype(np.int64)
    s_l = (ei[0] % N).astype(np.int64)
    d_l = (ei[1] % N).astype(np.int64)
    flat = np.zeros(B * N * N, np.float32)
    np.add.at(flat, s_g * (N * N) + s_l * N + d_l, 1.0)
    cmat = flat.reshape(B, N, N)

    wcheb = np.ascontiguousarray(
        W_cheb.transpose(1, 0, 2).reshape(DH, ORDER * DH))   # [dh, (k, dh')]
    wrow = W_gcn.sum(axis=0).reshape(1, ORDER).astype(np.float32)
    bcat2 = (b_cat + np.tile(b_cheb, H) @ W_cat[D:, :]).reshape(1, D).astype(np.float32)

    # stage-1 streaming-form constants
    mask8 = np.zeros((H, H * N), np.float32)
    for h in range(H):
        mask8[h, h * N:(h + 1) * N] = 1.0
    kcorr = ((H * N - N) * np.tanh(b_gcn)).reshape(ORDER, 1).astype(np.float32)
    wlin4 = (W_lin / N).astype(np.float32)
    blin4 = b_lin.reshape(ORDER, 1).astype(np.float32)

    shared = dict(
        wcheb=wcheb, wcat=np.ascontiguousarray(W_cat),
        bcat2=bcat2, gamma=gamma.reshape(1, D).astype(np.float32),
        beta=beta.reshape(1, D).astype(np.float32),
        wrow=wrow, bgcn=b_gcn.reshape(1, ORDER).astype(np.float32),
        mask8=mask8, wlin4=wlin4, kcorr=kcorr, blin4=blin4)

    in_maps = []
    for c in range(NCORES):
        G = slice(c * GPC, (c + 1) * GPC)
        in_maps.append(dict(
            attn_s=np.ascontiguousarray(attn[G]),
            x0_s=np.ascontiguousarray(oeh[G]),
            out_in_s=np.ascontiguousarray(outp[:, G, :]),
            cmat_s=np.ascontiguousarray(cmat[G]),
            **shared))
    return in_maps


def _fallback_numpy(inputs):
    """Generic (slow) numpy path, used only if the index tensors deviate from
    the canonical layout produced by setup_inputs()."""
    output = np.asarray(inputs["output"], np.float32)
    attn = np.asarray(inputs["attn"], np.float32)
    oeh = np.asarray(inputs["out_each_head"], np.float32)
    ei = np.asarray(inputs["edge_index"])
    fi = np.asarray(inputs["feature_indices"])
    batch = np.asarray(inputs["batch"])
    W_gcn = np.asarray(inputs["W_gcn"], np.float32); b_gcn = np.asarray(inputs["b_gcn"], np.float32)
    W_lin = np.asarray(inputs["W_lin"], np.float32); b_lin = np.asarray(inputs["b_lin"], np.float32)
    W_cheb = np.asarray(inputs["W_cheb"], np.float32); b_cheb = np.asarray(inputs["b_cheb"], np.float32)
    W_cat = np.asarray(inputs["W_cat"], np.float32); b_cat = np.asarray(inputs["b_cat"], np.float32)
    gamma = np.asarray(inputs["gamma"], np.float32); beta = np.asarray(inputs["beta"], np.float32)

    Bn, Hn, Nn, _ = attn.shape
    C = W_gcn.shape[0]
    total = Bn * Nn
    NT = Hn * total
    A = attn.transpose(1, 0, 2, 3).reshape(Hn * Bn, Nn, Nn)
    A_hat = A + np.eye(Nn, dtype=A.dtype)
    deg = A_hat.sum(axis=1)
    dinv = np.where(deg > 0, 1.0 / np.sqrt(deg), 0.0).astype(np.float32)
    w_row = W_gcn.sum(axis=0)
    s = np.einsum('bi,bij->bj', dinv, A_hat) * dinv
    x_c = np.tanh(s[:, :, None] * w_row + b_gcn)
    gap = x_c.mean(axis=1)
    coeff = gap @ W_lin + b_lin

    offsets = (np.arange(Hn) * total).astype(ei.dtype)
    src = (ei[0][None, :] + offsets[:, None]).reshape(-1)
    dst = (ei[1][None, :] + offsets[:, None]).reshape(-1)
    deg_n = np.zeros(NT, np.float32)
    np.add.at(deg_n, dst, 1.0)
    dinv_n = np.where(deg_n > 0, 1.0 / np.sqrt(np.maximum(deg_n, 1e-30)), 0.0).astype(np.float32)
    norm_e = -(dinv_n[src] * dinv_n[dst])

    def prop(x):
        out = np.zeros((NT, x.shape[1]), np.float32)
        np.add.at(out, dst, norm_e[:, None] * x[src])
        return out

    x0 = oeh.transpose(2, 0, 1, 3).reshape(NT, DH)
    batch_all = (batch[None, :] + (np.arange(Hn) * Bn)[:, None]).reshape(-1)
    c_node = coeff[batch_all]
    T_prev, T_cur = x0, prop(x0)
    acc = c_node[:, 0:1] * (T_prev @ W_cheb[0]) + c_node[:, 1:2] * (T_cur @ W_cheb[1])
    for k in range(2, C):
        T_next = 2.0 * prop(T_cur) - T_prev
        acc = acc + c_node[:, k:k + 1] * (T_next @ W_cheb[k])
        T_prev, T_cur = T_cur, T_next
    acc = acc + b_cheb
    filtered = acc.reshape(Hn, total, DH).transpose(1, 0, 2).reshape(total, Hn * DH)
    out_filtered = np.zeros_like(output)
    out_filtered[fi[:, 1], fi[:, 0], :] = filtered
    out_cat = np.concatenate([output, out_filtered], axis=-1)
    out = out_cat @ W_cat + b_cat
    mu = out.mean(axis=-1, keepdims=True)
    var = ((out - mu) ** 2).mean(axis=-1, keepdims=True)
    return ((out - mu) / np.sqrt(var + 1e-5) * gamma + beta).astype(np.float32)


def _get_nc():
    if "nc" not in _CACHE:
        _CACHE["nc"] = _build_module(use_f32r=_CACHE.get("use_f32r", True))
    return _CACHE["nc"]


def kernel(**inputs) -> np.ndarray:
    if not _canonical_indices(inputs["feature_indices"], inputs["batch"]):
        return _fallback_numpy(inputs)

    from concourse.bass_utils import run_bass_kernel_spmd

    nc = _get_nc()
    in_maps = _prep(inputs)
    res = run_bass_kernel_spmd(nc, in_maps, list(range(NCORES)))
    out = np.empty((N, B, D), np.float32)
    for c in range(NCORES):
        out[:, c * GPC:(c + 1) * GPC, :] = res.results[c]["outy"]
    return out


# revision 37
# speedup vs baseline: 2.0360x; 1.0022x over previous
"""Trainium2 Bass kernel for nn_DiffGraphTransformerGenGCN.

Strategy (see spec sharding hint): data-parallel over the 64 graphs, 8 graphs
per NeuronCore.  Per core everything is reformulated as dense per-graph
matmuls:

  stage 1 (GCN filter coefficients over the attention graphs):
    streaming colsum / dinv-weighted colsum of each 256x256 attention matrix
    (ones / block-diagonal dinv as the stationary operand, attention tiles
    streamed once in float32r), masked tanh with accumulate for the global
    average pool, tiny matmuls for the final linear -> 4 coefficients per
    (graph, head).

  stage 2 (dynamic Chebyshev filtering on the input graph):
    The edge list is converted (host-side, pure integer indexing) into a
    dense 256x256 edge-count matrix C_g per graph.  The scaled Laplacian
    action is  P x = -D C^T D x  with D = diag(rsqrt(indeg)).  The Chebyshev
    coefficients c_(g,h,k) (stage-1 output) are folded into per-(g,h)
    scaled copies of W_cheb, so  acc = sum_k c_k T_k(P) (x0 W_k)  is computed
    with 6 dense propagation matmuls per graph and no transposes.

  final: out = LayerNorm(concat(output, filtered) @ W_cat + b_cat') computed
    as a feat-major matmul (lhsT = PE-transposed operands).

All large matmuls run in float32r (1 cycle/row vs 4 for fp32); operands are
rounded to f32r at their producers (engine copy-casts / SWDGE cast-DMA).

kernel() takes FULL inputs and returns the FULL output; it shards across the
8 cores internally.
"""

import numpy as np

B, N, H, DH, ORDER = 64, 256, 8, 64, 4
D = H * DH
NCORES = 8
GPC = B // NCORES          # graphs per core
NCHUNK = N // 128          # node chunks per graph (2)

_CACHE = {}


# ---------------------------------------------------------------------------
# device kernel construction
# ---------------------------------------------------------------------------

def _build_module(use_f32r=True):
    import concourse.bass as bass
    import concourse.bacc as bacc
    import concourse.tile as tile
    from concourse import mybir
    from concourse.masks import make_identity

    F32 = mybir.dt.float32
    F32R = mybir.dt.float32r if use_f32r else mybir.dt.float32
    AF = mybir.ActivationFunctionType
    ALU = mybir.AluOpType

    nc = bacc.Bacc(None, target_bir_lowering=False, debug=False)

    # ---- DRAM parameters (per-core shard) ----
    p_attn = nc.declare_dram_parameter("attn_s", [GPC, H, N, N], F32, isOutput=False).ap()
    p_x0 = nc.declare_dram_parameter("x0_s", [GPC, N, H, DH], F32, isOutput=False).ap()
    p_oin = nc.declare_dram_parameter("out_in_s", [N, GPC, D], F32, isOutput=False).ap()
    p_cmat = nc.declare_dram_parameter("cmat_s", [GPC, N, N], F32, isOutput=False).ap()
    p_wcheb = nc.declare_dram_parameter("wcheb", [DH, ORDER * DH], F32, isOutput=False).ap()
    p_wcat = nc.declare_dram_parameter("wcat", [2 * D, D], F32, isOutput=False).ap()
    p_bcat2 = nc.declare_dram_parameter("bcat2", [1, D], F32, isOutput=False).ap()
    p_gamma = nc.declare_dram_parameter("gamma", [1, D], F32, isOutput=False).ap()
    p_beta = nc.declare_dram_parameter("beta", [1, D], F32, isOutput=False).ap()
    p_wrow = nc.declare_dram_parameter("wrow", [1, ORDER], F32, isOutput=False).ap()
    p_bgcn = nc.declare_dram_parameter("bgcn", [1, ORDER], F32, isOutput=False).ap()
    p_mask = nc.declare_dram_parameter("mask8", [H, H * N], F32, isOutput=False).ap()
    p_wlin = nc.declare_dram_parameter("wlin4", [ORDER, ORDER], F32, isOutput=False).ap()
    p_kcorr = nc.declare_dram_parameter("kcorr", [ORDER, 1], F32, isOutput=False).ap()
    p_blin4 = nc.declare_dram_parameter("blin4", [ORDER, 1], F32, isOutput=False).ap()
    p_outy = nc.declare_dram_parameter("outy", [N, GPC, D], F32, isOutput=True).ap()

    def bcast(dram_ap, parts, inner, extra_off=0):
        return bass.AP(tensor=dram_ap.tensor, offset=dram_ap.offset + extra_off,
                       ap=[[0, parts]] + inner)

    with tile.TileContext(nc, pool_alloc_mode="queue") as tc:
        import contextlib
        ctx = contextlib.ExitStack()
        with ctx:
            const = ctx.enter_context(tc.tile_pool(name="const", bufs=1))
            persist = ctx.enter_context(tc.tile_pool(name="persist", bufs=1))
            ps = ctx.enter_context(tc.tile_pool(name="ps", bufs=2, space="PSUM"))
            dram = ctx.enter_context(tc.tile_pool(name="dram", bufs=1, space="DRAM"))

            # ---------------- constants ----------------
            ones_col = const.tile([128, 1], F32)
            nc.vector.memset(ones_col, 1.0)
            ones_r = const.tile([128, 1], F32R)
            nc.vector.tensor_copy(ones_r, ones_col)
            ones2 = const.tile([128, 2], F32)
            nc.vector.memset(ones2, 1.0)
            ones2_r = const.tile([128, 2], F32R)
            nc.vector.tensor_copy(ones2_r, ones2)
            identity = const.tile([128, 128], F32)
            make_identity(nc, identity)
            wcheb_sb = const.tile([DH, ORDER * DH], F32)
            nc.sync.dma_start(out=wcheb_sb, in_=p_wcheb)
            wcat_sb = const.tile([128, 8 * D], F32R)
            bcat2_bc = const.tile([128, D], F32)
            nc.sync.dma_start(out=bcat2_bc, in_=bcast(p_bcat2, 128, [[1, D]]))
            gamma_bc = const.tile([128, D], F32)
            nc.sync.dma_start(out=gamma_bc, in_=bcast(p_gamma, 128, [[1, D]]))
            beta_bc = const.tile([128, D], F32)
            nc.sync.dma_start(out=beta_bc, in_=bcast(p_beta, 128, [[1, D]]))
            wrow_bc = const.tile([128, ORDER], F32)
            nc.sync.dma_start(out=wrow_bc, in_=bcast(p_wrow, 128, [[1, ORDER]]))
            bgcn_bc = const.tile([128, ORDER], F32)
            nc.sync.dma_start(out=bgcn_bc, in_=bcast(p_bgcn, 128, [[1, ORDER]]))
            mask_sb = const.tile([H, H * N], F32)
            nc.sync.dma_start(out=mask_sb, in_=p_mask)
            wlin_sb = const.tile([ORDER, ORDER], F32)
            nc.sync.dma_start(out=wlin_sb, in_=p_wlin)
            kcorr_sb = const.tile([ORDER, 1], F32)
            nc.sync.dma_start(out=kcorr_sb, in_=p_kcorr)
            blin4_sb = const.tile([ORDER, 1], F32)
            nc.sync.dma_start(out=blin4_sb, in_=p_blin4)
            eps_col = const.tile([128, 1], F32)
            nc.vector.memset(eps_col, 1e-5)

            # persistent big tiles
            acc_all = persist.tile([128, GPC * NCHUNK * D], F32)    # 4 MB
            outT = persist.tile([128, 4 * GPC * N], F32R)           # 4 MB
            dcrow2 = dram.tile([GPC, H * ORDER], F32)
            dcrow_exp = dram.tile([GPC, DH * H * ORDER], F32)
            ddinv = dram.tile([GPC, H * N], F32)

            # 'output' input, loaded node-major then PE-transposed to
            # feature-major: outT[f in fb-block, fb*2048 + g*256 + n].
            # Emitted interleaved with stage-1 (2 chunks per graph) so the
            # transposes fill PE gaps in stage-1's serial tail.
            oinp = ctx.enter_context(tc.tile_pool(name="oin", bufs=2))
            for fb in range(8):
                wb = oinp.tile([128, D], F32, tag="on")
                nc.sync.dma_start(out=wb, in_=p_wcat[fb * 128:(fb + 1) * 128, :])
                nc.scalar.copy(out=wcat_sb[:, fb * D:(fb + 1) * D], in_=wb)

            def emit_outT_chunk(nch):
                g, dc = divmod(nch, NCHUNK)
                on = oinp.tile([128, D], F32, tag="on", name=f"on_{nch}")
                nc.sync.dma_start(out=on, in_=p_oin[dc * 128:(dc + 1) * 128, g, :])
                pso = ps.tile([128, 2048], F32, tag="big", name=f"pso_{nch}")
                for fb in range(4):
                    nc.tensor.transpose(pso[:, fb * 128:(fb + 1) * 128],
                                        on[:, fb * 128:(fb + 1) * 128], identity)
                nc.vector.tensor_copy(
                    outT.rearrange("p (f x) -> p f x", f=4)
                        [:, :, nch * 128:(nch + 1) * 128],
                    pso[:, 0:512].rearrange("p (f n) -> p f n", f=4))

            # =====================================================
            # Phase A: stage-1 GCN coefficients (streaming form)
            # =====================================================
            with tc.tile_pool(name="stage1", bufs=3) as s1p, \
                 tc.tile_pool(name="stage1s", bufs=4) as s1s, \
                 tc.tile_pool(name="s1w", bufs=1) as s1w:
                for g in range(GPC):
                    emit_outT_chunk(2 * g)
                    emit_outT_chunk(2 * g + 1)
                    Atr = []
                    for ic in range(2):
                        a = s1p.tile([128, H * N], F32, tag="attn")
                        (nc.sync if ic == 0 else nc.scalar).dma_start(
                            out=a.rearrange("p (h j) -> p h j", h=H),
                            in_=p_attn[g, :, ic * 128:(ic + 1) * 128, :]
                                .rearrange("h i j -> i h j"))
                        ar = s1p.tile([128, H * N], F32R, tag="attnr")
                        if ic == 0:
                            nc.scalar.copy(out=ar, in_=a)
                        else:
                            nc.vector.tensor_copy(ar, a)
                        Atr.append(ar)
                    # r1: colsum rows [1, (h, j)]  (one psum tile per graph)
                    psA = ps.tile([128, 2048], F32, tag="big")
                    psT = psA
                    psD = psA
                    psC = psA
                    for w in range(4):
                        for ic in range(2):
                            nc.tensor.matmul(
                                out=psA[0:1, w * 512:(w + 1) * 512],
                                lhsT=ones_r, rhs=Atr[ic][:, w * 512:(w + 1) * 512],
                                start=(ic == 0), stop=(ic == 1))
                    # dinv row = 1/sqrt(colsum + 1)
                    sqrow = s1w.tile([1, H * N], F32, tag="sqrow")
                    nc.scalar.activation(out=sqrow, in_=psA[0:1, 0:2048],
                                         func=AF.Sqrt, bias=1.0)
                    dinvrow = s1w.tile([1, H * N], F32, tag="dinvrow")
                    rscr = s1w.tile([1, H * N], F32, tag="junk")
                    nc.vector.reciprocal_approx_accurate(out=dinvrow, in_=sqrow,
                                                         scratch=rscr)
                    nc.sync.dma_start(out=ddinv[g:g + 1, :], in_=dinvrow)
                    # transpose dinv row-chunks -> columns [128, (ic, h)]
                    for ic in range(2):
                        for h in range(H):
                            nc.tensor.transpose(
                                psT[:, (ic * 8 + h) * 128:(ic * 8 + h) * 128 + 1],
                                dinvrow[0:1, h * N + ic * 128:h * N + (ic + 1) * 128],
                                identity[0:1, 0:1])
                    dinvT = s1s.tile([128, 16], F32R, tag="dinvT")
                    nc.scalar.copy(
                        out=dinvT,
                        in_=psT[:, 0:2048].rearrange("p (c x) -> p c x", c=16)[:, :, 0])
                    # r2: dot rows [8, (h', j)] via block-diag dinv lhsT
                    for w in range(4):
                        for ic in range(2):
                            nc.tensor.matmul(
                                out=psD[0:8, w * 512:(w + 1) * 512],
                                lhsT=dinvT[:, ic * 8:(ic + 1) * 8],
                                rhs=Atr[ic][:, w * 512:(w + 1) * 512],
                                start=(ic == 0), stop=(ic == 1))
                    # s = (dot + dinv) * dinv, diag-masked
                    dinvB = s1w.tile([H, H * N], F32, tag="dinvB")
                    nc.sync.dma_start(out=dinvB,
                                      in_=bcast(ddinv[0], H, [[1, H * N]],
                                                extra_off=g * H * N))
                    maskD = s1w.tile([H, H * N], F32, tag="maskD")
                    nc.vector.tensor_mul(maskD, dinvB, mask_sb)
                    t8 = s1w.tile([H, H * N], F32, tag="t8")
                    nc.vector.tensor_add(t8, psD[0:8, 0:2048], dinvB)
                    sm = s1w.tile([H, H * N], F32, tag="sm")
                    nc.vector.tensor_mul(sm, t8, maskD)
                    # tanh + GAP (accumulate); off-diag contributes tanh(b_c)
                    gacc = s1s.tile([H, ORDER], F32, tag="gacc")
                    for c in range(ORDER):
                        junk = s1w.tile([H, H * N], F32, tag="junk")
                        nc.scalar.activation(
                            out=junk, in_=sm, func=AF.Tanh,
                            scale=wrow_bc[0:H, c:c + 1], bias=bgcn_bc[0:H, c:c + 1],
                            accum_out=gacc[:, c:c + 1])
                    # coeff = (gacc - corr)/256 @ W_lin + b_lin
                    nc.tensor.transpose(psC[0:ORDER, 0:H], gacc, identity[0:H, 0:H])
                    gapT = s1s.tile([ORDER, H], F32, tag="gapT")
                    nc.vector.tensor_scalar_sub(gapT, psC[0:ORDER, 0:H], kcorr_sb)
                    nc.tensor.matmul(out=psC[0:ORDER, 512:512 + H], lhsT=wlin_sb,
                                     rhs=gapT, start=True, stop=True)
                    coefT = s1s.tile([ORDER, H], F32, tag="coefT")
                    nc.vector.tensor_scalar_add(coefT, psC[0:ORDER, 512:512 + H],
                                                blin4_sb)
                    nc.tensor.transpose(psC[0:H, 1024:1024 + ORDER], coefT,
                                        identity[0:ORDER, 0:ORDER])
                    crow_sb = s1s.tile([H, ORDER], F32, tag="crow_sb")
                    nc.scalar.copy(out=crow_sb, in_=psC[0:H, 1024:1024 + ORDER])
                    nc.sync.dma_start(
                        out=dcrow2[g:g + 1, :].rearrange("o (h k) -> o h k", k=ORDER),
                        in_=crow_sb)

            # expand coefficients: dcrow_exp[g, (dh', h, k)] = c[g, h, k]
            nc.sync.dma_start(
                out=dcrow_exp.rearrange("g (d hk) -> g d hk", d=DH),
                in_=bass.AP(tensor=dcrow2.tensor, offset=dcrow2.offset,
                            ap=[[H * ORDER, GPC], [0, DH], [1, H * ORDER]]))

            # =====================================================
            # Phase B: Chebyshev propagation per graph
            # =====================================================
            with tc.tile_pool(name="st2a", bufs=2) as st2a, \
                 tc.tile_pool(name="st2b", bufs=2) as st2b, \
                 tc.tile_pool(name="st2s", bufs=4) as st2s:
                for g in range(GPC):
                    # --- C_g (cast to f32r during DMA) and degree columns ---
                    cm = []
                    for sc in range(2):
                        tf = st2a.tile([128, N], F32, tag="cmf")
                        nc.sync.dma_start(out=tf,
                                          in_=p_cmat[g, sc * 128:(sc + 1) * 128, :])
                        t = st2a.tile([128, N], F32R, tag="cm")
                        nc.scalar.copy(out=t, in_=tf)
                        cm.append(t)
                    psd = ps.tile([128, 2048], F32, tag="big")
                    for dc in range(2):
                        for sc in range(2):
                            nc.tensor.matmul(
                                out=psd[:, dc * 2:dc * 2 + 2],
                                lhsT=cm[sc][:, dc * 128:(dc + 1) * 128],
                                rhs=ones2_r, start=(sc == 0), stop=(sc == 1))
                    # masked rsqrt: dinv = m / (sqrt(deg) + 1 - m), m = sign(deg)
                    # dvar layout [128, (kind, dc)]: kinds
                    # 0=dinv 1=dsq 2=dneg 3=d3 4=d2 5=dm4
                    dvar = st2s.tile([128, 12], F32, tag="dvar")
                    sg = st2s.tile([128, 2], F32, tag="sg")
                    psdv = psd[:, 0:4].rearrange('p (d two) -> p d two', two=2)[:, :, 0]
                    nc.scalar.activation(out=sg, in_=psdv, func=AF.Sign)
                    sq = st2s.tile([128, 2], F32, tag="sq")
                    nc.scalar.activation(out=sq, in_=psdv, func=AF.Sqrt)
                    den = st2s.tile([128, 2], F32, tag="den")
                    nc.vector.scalar_tensor_tensor(
                        out=den, in0=sq, scalar=1.0, in1=sg,
                        op0=ALU.add, op1=ALU.subtract)
                    rec = st2s.tile([128, 2], F32, tag="rec")
                    nc.vector.reciprocal(out=rec, in_=den)
                    dv = dvar[:, 0:2]
                    nc.vector.tensor_mul(dv, rec, sg)
                    nc.scalar.square(dvar[:, 2:4], dv)
                    nc.scalar.mul(dvar[:, 4:6], dv, -1.0)
                    nc.scalar.mul(dvar[:, 6:8], dv, 3.0)
                    nc.scalar.mul(dvar[:, 8:10], dv, 2.0)
                    nc.scalar.mul(dvar[:, 10:12], dv, -4.0)

                    # --- x0^T via PE transpose (cast to f32r on eviction) ---
                    x0t = st2b.tile([DH, H * N], F32R, tag="x0t")
                    for nc2 in range(2):
                        x0n = st2s.tile([128, H * DH], F32, tag="x0n")
                        nc.scalar.dma_start(
                            out=x0n.rearrange("p (h d) -> p h d", h=H),
                            in_=p_x0[g, nc2 * 128:(nc2 + 1) * 128, :, :])
                        psx = ps.tile([128, 2048], F32, tag="big")
                        for h in range(H):
                            nc.tensor.transpose(psx[0:DH, h * 128:(h + 1) * 128],
                                                x0n[:, h * DH:(h + 1) * DH], identity)
                        nc.vector.tensor_copy(
                            x0t.rearrange("p (h x) -> p h x", h=H)
                               [:, :, nc2 * 128:(nc2 + 1) * 128],
                            psx[0:DH, 0:1024].rearrange("p (h n) -> p h n", h=H))

                    # --- c-scaled W_cheb:
                    # cexp[p, (dh', hk)] broadcast from dcrow_exp[g]
                    cexp = st2b.tile([DH, DH * H * ORDER], F32, tag="cexp")
                    nc.sync.dma_start(
                        out=cexp,
                        in_=bcast(dcrow_exp[0], DH, [[1, DH * H * ORDER]],
                                  extra_off=g * DH * H * ORDER))
                    wsc = st2b.tile([DH, H * ORDER * DH], F32R, tag="wsc")
                    cexpv = cexp.rearrange("p (d h2 k) -> p h2 k d", d=DH, k=ORDER)
                    for h in range(H):
                        nc.vector.tensor_mul(
                            wsc[:, h * 256:(h + 1) * 256]
                                .rearrange("p (k d) -> p k d", k=ORDER),
                            cexpv[:, h, :, :],
                            wcheb_sb.rearrange("p (k d) -> p k d", k=ORDER))

                    # --- Z matmuls:  Z[(n),(k,dh')] = x0 @ (c_k W_k) ---
                    accv = []
                    v0 = []
                    for dc in range(2):
                        pz = ps.tile([128, 2048], F32, tag="big")
                        for h in range(H):
                            nc.tensor.matmul(
                                out=pz[:, h * 256:(h + 1) * 256],
                                lhsT=x0t[:, h * N + dc * 128:h * N + (dc + 1) * 128],
                                rhs=wsc[:, h * 256:(h + 1) * 256],
                                start=True, stop=True)
                        pzv = pz.rearrange("p (h k d) -> p h k d", h=H, k=ORDER)
                        a = acc_all[:, (g * 2 + dc) * D:(g * 2 + dc + 1) * D] \
                            .rearrange("p (h d) -> p h d", h=H)
                        accv.append(a)
                        # acc = Z0 - Z2  (one PSUM operand per instruction)
                        nc.scalar.mul(a, pzv[:, :, 2, :], -1.0)
                        nc.vector.tensor_tensor(
                            out=a, in0=pzv[:, :, 0, :], in1=a, op=ALU.add)
                        # v0 = dinv * Z[1:4]
                        v = st2a.tile([128, H * 3 * DH], F32R, tag="v0")
                        nc.scalar.activation(
                            out=v.rearrange("p (h k d) -> p h k d", h=H, k=3),
                            in_=pzv[:, :, 1:4, :], func=AF.Copy,
                            scale=dvar[:, dc:dc + 1])
                        v0.append(v)

                    # --- propagation round 1: y1 = C^T v0 ---
                    v1 = []
                    for dc in range(2):
                        py = ps.tile([128, 2048], F32, tag="big")
                        for w in range(3):
                            for sc in range(2):
                                nc.tensor.matmul(
                                    out=py[:, w * 512:(w + 1) * 512],
                                    lhsT=cm[sc][:, dc * 128:(dc + 1) * 128],
                                    rhs=v0[sc][:, w * 512:(w + 1) * 512],
                                    start=(sc == 0), stop=(sc == 1))
                        pyv = py[:, 0:1536].rearrange("p (h k d) -> p h k d", h=H, k=3)
                        a = accv[dc]
                        # acc += -dinv * y1[k=1]
                        nc.vector.scalar_tensor_tensor(
                            out=a, in0=pyv[:, :, 0, :],
                            scalar=dvar[:, 4 + dc:5 + dc],
                            in1=a, op0=ALU.mult, op1=ALU.add)
                        # acc += 3 dinv * y1[k=3]
                        nc.vector.scalar_tensor_tensor(
                            out=a, in0=pyv[:, :, 2, :],
                            scalar=dvar[:, 6 + dc:7 + dc],
                            in1=a, op0=ALU.mult, op1=ALU.add)
                        # v1 = dinv^2 * y1[k=2,3]
                        v = st2a.tile([128, H * 2 * DH], F32R, tag="v1")
                        nc.scalar.activation(
                            out=v.rearrange("p (h k d) -> p h k d", h=H, k=2),
                            in_=pyv[:, :, 1:3, :], func=AF.Copy,
                            scale=dvar[:, 2 + dc:3 + dc])
                        v1.append(v)

                    # --- round 2 ---
                    v2 = []
                    for dc in range(2):
                        py = ps.tile([128, 2048], F32, tag="big")
                        for w in range(2):
                            for sc in range(2):
                                nc.tensor.matmul(
                                    out=py[:, w * 512:(w + 1) * 512],
                                    lhsT=cm[sc][:, dc * 128:(dc + 1) * 128],
                                    rhs=v1[sc][:, w * 512:(w + 1) * 512],
                                    start=(sc == 0), stop=(sc == 1))
                        pyv = py[:, 0:1024].rearrange("p (h k d) -> p h k d", h=H, k=2)
                        a = accv[dc]
                        # acc += 2 dinv * y2[k=2]
                        nc.vector.scalar_tensor_tensor(
                            out=a, in0=pyv[:, :, 0, :],
                            scalar=dvar[:, 8 + dc:9 + dc],
                            in1=a, op0=ALU.mult, op1=ALU.add)
                        v = st2a.tile([128, H * DH], F32R, tag="v2")
                        nc.scalar.activation(
                            out=v.rearrange("p (h d) -> p h d", h=H),
                            in_=pyv[:, :, 1, :], func=AF.Copy,
                            scale=dvar[:, 2 + dc:3 + dc])
                        v2.append(v)

                    # --- round 3 ---
                    for dc in range(2):
                        py = ps.tile([128, 2048], F32, tag="big")
                        for sc in range(2):
                            nc.tensor.matmul(
                                out=py[:, 0:512],
                                lhsT=cm[sc][:, dc * 128:(dc + 1) * 128],
                                rhs=v2[sc], start=(sc == 0), stop=(sc == 1))
                        a = accv[dc]
                        # acc += -4 dinv * y3
                        nc.vector.scalar_tensor_tensor(
                            out=a,
                            in0=py[:, 0:512].rearrange("p (h d) -> p h d", h=H),
                            scalar=dvar[:, 10 + dc:11 + dc],
                            in1=a, op0=ALU.mult, op1=ALU.add)

            # =====================================================
            # Phase C: transpose acc, cat-matmul, LayerNorm, store
            # =====================================================
            with tc.tile_pool(name="ph_c", bufs=1) as phc, \
                 tc.tile_pool(name="ln", bufs=3) as lnp, \
                 tc.tile_pool(name="lns", bufs=4) as lns:
                accT = phc.tile([128, 4 * GPC * N], F32R)
                for nch in range(GPC * NCHUNK):
                    pst = ps.tile([128, 2048], F32, tag="big")
                    for fb in range(4):
                        nc.tensor.transpose(
                            pst[:, fb * 128:(fb + 1) * 128],
                            acc_all[:, nch * D + fb * 128:nch * D + (fb + 1) * 128],
                            identity)
                    nc.scalar.copy(
                        out=accT.rearrange("p (f x) -> p f x", f=4)
                            [:, :, nch * 128:(nch + 1) * 128],
                        in_=pst[:, 0:512].rearrange("p (f n) -> p f n", f=4))

                for nch in range(GPC * NCHUNK):
                    g, dc = divmod(nch, NCHUNK)
                    po = ps.tile([128, 2048], F32, tag="big")
                    for fb in range(4):
                        nc.tensor.matmul(
                            out=po[:, 0:512],
                            lhsT=outT[:, fb * (GPC * N) + nch * 128:
                                      fb * (GPC * N) + (nch + 1) * 128],
                            rhs=wcat_sb[:, fb * D:(fb + 1) * D],
                            start=(fb == 0), stop=False)
                    for fb in range(4):
                        nc.tensor.matmul(
                            out=po[:, 0:512],
                            lhsT=accT[:, fb * (GPC * N) + nch * 128:
                                      fb * (GPC * N) + (nch + 1) * 128],
                            rhs=wcat_sb[:, (4 + fb) * D:(5 + fb) * D],
                            start=False, stop=(fb == 3))

                    # LayerNorm
                    t0 = lnp.tile([128, D], F32, tag="t0")
                    musum = lns.tile([128, 1], F32, tag="musum")
                    nc.vector.scalar_tensor_tensor(
                        out=t0, in0=po[:, 0:512], scalar=1.0, in1=bcat2_bc,
                        op0=ALU.mult, op1=ALU.add, accum_out=musum)
                    negmu = lns.tile([128, 1], F32, tag="negmu")
                    nc.scalar.mul(negmu, musum, -1.0 / D)
                    sqs = lns.tile([128, 1], F32, tag="sqs")
                    sq = lnp.tile([128, D], F32, tag="sq")
                    nc.scalar.activation(out=sq, in_=t0, func=AF.Square,
                                         bias=negmu, accum_out=sqs)
                    sd = lns.tile([128, 1], F32, tag="sd")
                    nc.scalar.activation(out=sd, in_=sqs, func=AF.Sqrt,
                                         scale=1.0 / D, bias=eps_col)
                    rstd = lns.tile([128, 1], F32, tag="rstd")
                    nc.vector.reciprocal(out=rstd, in_=sd)
                    nmr = lns.tile([128, 1], F32, tag="nmr")
                    nc.vector.tensor_mul(nmr, negmu, rstd)
                    t1 = lnp.tile([128, D], F32, tag="t1")
                    nc.scalar.activation(out=t1, in_=t0, func=AF.Identity,
                                         scale=rstd, bias=nmr)
                    t2 = lnp.tile([128, D], F32, tag="t2")
                    nc.vector.tensor_mul(t2, t1, gamma_bc)
                    nc.vector.tensor_add(t2, t2, beta_bc)
                    nc.gpsimd.dma_start(out=p_outy[dc * 128:(dc + 1) * 128, g, :], in_=t2)

    if not nc.is_finalized():
        nc.finalize()
    return nc


# ---------------------------------------------------------------------------
# host side
# ---------------------------------------------------------------------------

def _canonical_indices(feature_indices, batch):
    fi = np.asarray(feature_indices)
    bt = np.asarray(batch)
    want0 = np.repeat(np.arange(B), N)
    want1 = np.tile(np.arange(N), B)
    return (fi.shape == (B * N, 2) and bt.shape == (B * N,)
            and np.array_equal(fi[:, 0], want0) and np.array_equal(fi[:, 1], want1)
            and np.array_equal(bt, want0))


def _prep(inputs):
    """Host-side sharding + index preprocessing. Returns per-core input maps."""
    attn = np.ascontiguousarray(np.asarray(inputs["attn"], np.float32))
    oeh = np.ascontiguousarray(np.asarray(inputs["out_each_head"], np.float32))
    outp = np.ascontiguousarray(np.asarray(inputs["output"], np.float32))
    ei = np.asarray(inputs["edge_index"])

    W_gcn = np.asarray(inputs["W_gcn"], np.float32)
    b_gcn = np.asarray(inputs["b_gcn"], np.float32)
    W_lin = np.asarray(inputs["W_lin"], np.float32)
    b_lin = np.asarray(inputs["b_lin"], np.float32)
    W_cheb = np.asarray(inputs["W_cheb"], np.float32)
    b_cheb = np.asarray(inputs["b_cheb"], np.float32)
    W_cat = np.asarray(inputs["W_cat"], np.float32)
    b_cat = np.asarray(inputs["b_cat"], np.float32)
    gamma = np.asarray(inputs["gamma"], np.float32)
    beta = np.asarray(inputs["beta"], np.float32)

    # dense per-graph edge-count matrices (pure integer indexing)
    s_g = (ei[0] // N).astype(np.int64)
    s_l = (ei[0] % N).astype(np.int64)
    d_l = (ei[1] % N).astype(np.int64)
    flat = np.zeros(B * N * N, np.float32)
    np.add.at(flat, s_g * (N * N) + s_l * N + d_l, 1.0)
    cmat = flat.reshape(B, N, N)

    wcheb = np.ascontiguousarray(
        W_cheb.transpose(1, 0, 2).reshape(DH, ORDER * DH))   # [dh, (k, dh')]
    wrow = W_gcn.sum(axis=0).reshape(1, ORDER).astype(np.float32)
    bcat2 = (b_cat + np.tile(b_cheb, H) @ W_cat[D:, :]).reshape(1, D).astype(np.float32)

    # stage-1 streaming-form constants
    mask8 = np.zeros((H, H * N), np.float32)
    for h in range(H):
        mask8[h, h * N:(h + 1) * N] = 1.0
    kcorr = ((H * N - N) * np.tanh(b_gcn)).reshape(ORDER, 1).astype(np.float32)
    wlin4 = (W_lin / N).astype(np.float32)
    blin4 = b_lin.reshape(ORDER, 1).astype(np.float32)

    shared = dict(
        wcheb=wcheb, wcat=np.ascontiguousarray(W_cat),
        bcat2=bcat2, gamma=gamma.reshape(1, D).astype(np.float32),
        beta=beta.reshape(1, D).astype(np.float32),
        wrow=wrow, bgcn=b_gcn.reshape(1, ORDER).astype(np.float32),
        mask8=mask8, wlin4=wlin4, kcorr=kcorr, blin4=blin4)

    in_maps = []
    for c in range(NCORES):
        G = slice(c * GPC, (c + 1) * GPC)
        in_maps.append(dict(
            attn_s=np.ascontiguousarray(attn[G]),
            x0_s=np.ascontiguousarray(oeh[G]),
            out_in_s=np.ascontiguousarray(outp[:, G, :]),
            cmat_s=np.ascontiguousarray(cmat[G]),
            **shared))
    return in_maps


def _fallback_numpy(inputs):
    """Generic (slow) numpy path, used only if the index tensors deviate from
    the canonical layout produced by setup_inputs()."""
    output = np.asarray(inputs["output"], np.float32)
    attn = np.asarray(inputs["attn"], np.float32)
    oeh = np.asarray(inputs["out_each_head"], np.float32)
    ei = np.asarray(inputs["edge_index"])
    fi = np.asarray(inputs["feature_indices"])
    batch = np.asarray(inputs["batch"])
    W_gcn = np.asarray(inputs["W_gcn"], np.float32); b_gcn = np.asarray(inputs["b_gcn"], np.float32)
    W_lin = np.asarray(inputs["W_lin"], np.float32); b_lin = np.asarray(inputs["b_lin"], np.float32)
    W_cheb = np.asarray(inputs["W_cheb"], np.float32); b_cheb = np.asarray(inputs["b_cheb"], np.float32)
    W_cat = np.asarray(inputs["W_cat"], np.float32); b_cat = np.asarray(inputs["b_cat"], np.float32)
    gamma = np.asarray(inputs["gamma"], np.float32); beta = np.asarray(inputs["beta"], np.float32)

    Bn, Hn, Nn, _ = attn.shape
    C = W_gcn.shape[0]
    total = Bn * Nn
    NT = Hn * total
    A = attn.transpose(1, 0, 2, 3).reshape(Hn * Bn, Nn, Nn)
    A_hat = A + np.eye(Nn, dtype=A.dtype)
    deg = A_hat.sum(axis=1)
    dinv = np.where(deg > 0, 1.0 / np.sqrt(deg), 0.0).astype(np.float32)
    w_row = W_gcn.sum(axis=0)
    s = np.einsum('bi,bij->bj', dinv, A_hat) * dinv
    x_c = np.tanh(s[:, :, None] * w_row + b_gcn)
    gap = x_c.mean(axis=1)
    coeff = gap @ W_lin + b_lin

    offsets = (np.arange(Hn) * total).astype(ei.dtype)
    src = (ei[0][None, :] + offsets[:, None]).reshape(-1)
    dst = (ei[1][None, :] + offsets[:, None]).reshape(-1)
    deg_n = np.zeros(NT, np.float32)
    np.add.at(deg_n, dst, 1.0)
    dinv_n = np.where(deg_n > 0, 1.0 / np.sqrt(np.maximum(deg_n, 1e-30)), 0.0).astype(np.float32)
    norm_e = -(dinv_n[src] * dinv_n[dst])

    def prop(x):
        out = np.zeros((NT, x.shape[1]), np.float32)
        np.add.at(out, dst, norm_e[:, None] * x[src])
        return out

    x0 = oeh.transpose(2, 0, 1, 3).reshape(NT, DH)
    batch_all = (batch[None, :] + (np.arange(Hn) * Bn)[:, None]).reshape(-1)
    c_node = coeff[batch_all]
    T_prev, T_cur = x0, prop(x0)
    acc = c_node[:, 0:1] * (T_prev @ W_cheb[0]) + c_node[:, 1:2] * (T_cur @ W_cheb[1])
    for k in range(2, C):
        T_next = 2.0 * prop(T_cur) - T_prev
        acc = acc + c_node[:, k:k + 1] * (T_next @ W_cheb[k])
        T_prev, T_cur = T_cur, T_next
    acc = acc + b_cheb
    filtered = acc.reshape(Hn, total, DH).transpose(1, 0, 2).reshape(total, Hn * DH)
    out_filtered = np.zeros_like(output)
    out_filtered[fi[:, 1], fi[:, 0], :] = filtered
    out_cat = np.concatenate([output, out_filtered], axis=-1)
    out = out_cat @ W_cat + b_cat
    mu = out.mean(axis=-1, keepdims=True)
    var = ((out - mu) ** 2).mean(axis=-1, keepdims=True)
    return ((out - mu) / np.sqrt(var + 1e-5) * gamma + beta).astype(np.float32)


def _get_nc():
    if "nc" not in _CACHE:
        _CACHE["nc"] = _build_module(use_f32r=_CACHE.get("use_f32r", True))
    return _CACHE["nc"]


def kernel(**inputs) -> np.ndarray:
    if not _canonical_indices(inputs["feature_indices"], inputs["batch"]):
        return _fallback_numpy(inputs)

    from concourse.bass_utils import run_bass_kernel_spmd

    nc = _get_nc()
    in_maps = _prep(inputs)
    res = run_bass_kernel_spmd(nc, in_maps, list(range(NCORES)))
    out = np.empty((N, B, D), np.float32)
    for c in range(NCORES):
        out[:, c * GPC:(c + 1) * GPC, :] = res.results[c]["outy"]
    return out
